# revision 1
# baseline (speedup 1.0000x reference)
# Trainium2 Bass kernel for nn_CapLayer (CapsNet grouped 1x1 conv + dynamic routing).
#
# Key algebraic restructuring: the huge intermediate pred[b, i=(g,s), (j,d)]
# (188MB for the full batch) is NEVER materialized. Routing is computed in a
# factored form:
#   pred[b,(g,s),(j,d)] = sum_c Wa[g,j,d,c] * xga[b,g,c,s]     (c augmented with
#                                                               a ones channel to
#                                                               absorb the bias)
#   t[b,j,g,c]  = sum_s c[b,j,(g,s)] * xga[b,g,c,s]
#   s[b,j,d]    = sum_{g,c} t[b,j,g,c] * Wa[g,j,d,c]
#   u[b,j,g,c]  = sum_d v[b,j,d] * Wa[g,j,d,c]
#   db[b,j,g,s] = sum_c u[b,j,g,c] * xga[b,g,c,s]
# Iteration 1 collapses (softmax of zeros is uniform): t1 = xsum / J.
#
# Sharding: pure data parallel, 32 samples per core across 8 cores.
# On-chip layout: partition p = (b4, g) with 4 samples x 32 groups = 128
# partitions; 8 chunks cover the 32 local samples. The g-contraction for
# s[b,(j,d)] is done on the TensorEngine with a block-diagonal ones matrix,
# which also replicates s across the g-partitions for free (so v and u stay
# in the same partition layout).
#
# Engine split: broadcast-products run in bf16 (DVE 2x mode / GPSIMD),
# segmented reductions and small elementwise stay on DVE in fp32 accuracy,
# exp/sqrt/copies ride the Scalar engine, the g-sum is a TensorE matmul.

import sys

import numpy as np

# concourse (Bass/Tile) ships with the container; make sure it's importable
# when the grader runs kernel.py from a bare directory.
for _p in ("/opt/trn_rl_repo", "/root/.axon_site/_ro/trn_rl_repo"):
    if _p not in sys.path:
        sys.path.insert(0, _p)

NS, J, D, C_IN, H, WID, RN = 32, 10, 16, 8, 6, 6, 3
S = H * WID            # 36 spatial positions
CA = C_IN + 1          # 9 channels including the ones channel
CP = 10                # padded channel stride (4B alignment for bf16 rows)
NCORES = 8
BLOC = 32              # samples per core
B4 = 4                 # samples per chunk
NCH = BLOC // B4       # 8 chunks

_CACHE = {}


def _build_program(split_waits=True, dve_chunks=8, dma_eng="sync"):
    from contextlib import ExitStack

    import concourse.bass as bass
    import concourse.tile as tile
    from concourse import mybir

    f32 = mybir.dt.float32
    bf16 = mybir.dt.float16
    Alu = mybir.AluOpType
    Act = mybir.ActivationFunctionType
    AxX = mybir.AxisListType.X

    nc = bass.Bass("TRN2", target_bir_lowering=True, debug=False,
                   num_devices=NCORES)

    xcs_d = nc.dram_tensor("xcs", [NCH, 128, CA * S], bf16,
                           kind="ExternalInput").ap()      # free = (c, s)
    xsc_d = nc.dram_tensor("xsc", [NCH, 128, S * CP], bf16,
                           kind="ExternalInput").ap()      # free = (s, c10)
    wc_d = nc.dram_tensor("wc", [128, J * D * CP], bf16,
                          kind="ExternalInput").ap()       # free = (j, d, c10)
    wu_d = nc.dram_tensor("wu", [128, J * CA * D], bf16,
                          kind="ExternalInput").ap()       # free = (j, c, d)
    ones_d = nc.dram_tensor("onesb", [128, 128], bf16,
                            kind="ExternalInput").ap()     # blockdiag over b4
    v_d = nc.dram_tensor("v", [BLOC, J * D], f32,
                         kind="ExternalOutput").ap()

    dmae = {"gpsimd": nc.gpsimd, "sync": nc.sync}[dma_eng]
    with tile.TileContext(nc) as tc, ExitStack() as ctx:
        consts = ctx.enter_context(tc.tile_pool(name="consts", bufs=1))
        xpool = ctx.enter_context(tc.tile_pool(name="xpool", bufs=1))
        lpool = ctx.enter_context(tc.tile_pool(name="lpool", bufs=1))
        spool = ctx.enter_context(tc.tile_pool(name="scratch", bufs=2))
        small = ctx.enter_context(tc.tile_pool(name="small", bufs=3))
        vpool = ctx.enter_context(tc.tile_pool(name="vpool", bufs=2))
        psum = ctx.enter_context(tc.tile_pool(name="psum", bufs=2,
                                              space="PSUM"))

        wc_t = consts.tile([128, J * D * CP], bf16, tag="wc")
        dmae.dma_start(wc_t[:, :], wc_d[:, :])
        wu_t = consts.tile([128, J * CA * D], bf16, tag="wu")
        dmae.dma_start(wu_t[:, :], wu_d[:, :])
        ones_t = consts.tile([128, 128], bf16, tag="onesb")
        dmae.dma_start(ones_t[:, :], ones_d[:, :])

        # Persistent per-chunk tiles.
        Xcs = []   # xga [p, (c, s)] bf16
        Xsc = []   # xga [p, (s, c)] bf16
        L = []     # routing logits b, layout [p, (j, s)] fp32
        for ch in range(NCH):
            xt = xpool.tile([128, CA * S], bf16, tag=f"Xcs{ch}",
                            name=f"Xcs{ch}")
            dmae.dma_start(xt[:, :], xcs_d[ch, :, :])
            Xcs.append(xt)
            xt2 = xpool.tile([128, S * CP], bf16, tag=f"Xsc{ch}",
                             name=f"Xsc{ch}")
            dmae.dma_start(xt2[:, :], xsc_d[ch, :, :])
            Xsc.append(xt2)
            L.append(lpool.tile([128, J * S], f32, tag=f"L{ch}",
                                name=f"L{ch}"))

        def prod_engine(ch):
            # Split the broadcast-product work between DVE and GPSIMD by
            # chunk so both engines stay busy.
            return nc.vector if (ch % 8) < dve_chunks else nc.gpsimd

        def c_step(ch, t_in0_bcast):
            """t x Wa summed over (g, c) -> replicated s [p, (j,d)].

            t_in0_bcast: AP broadcast to [p, J, D, CA] (bf16).
            Returns an SBUF tile [128, J*D] fp32 with s replicated over g
            within each b4 partition block.
            """
            eng = prod_engine(ch)
            pc = spool.tile([128, J * D * CP], bf16, tag="prodC")
            pc4 = (pc[:, :].rearrange("p (j d c) -> p j d c", j=J, d=D)
                   [:, :, :, 0:CA])
            wc4 = (wc_t[:, :].rearrange("p (j d c) -> p j d c", j=J, d=D)
                   [:, :, :, 0:CA])
            eng.tensor_tensor(pc4, t_in0_bcast, wc4, Alu.mult)
            # PE contracts g (partitions, via blockdiag ones) AND c (PSUM
            # accumulation over the 9 channel slices) in one group -- no
            # DVE reduction needed at all.
            pcz = pc[:, :].rearrange("p (a c) -> p a c", c=CP)
            ps = psum.tile([128, J * D], f32, tag="psum_s")
            for c in range(CA):
                nc.tensor.matmul(ps[:, :], ones_t[:, :], pcz[:, :, c],
                                 start=(c == 0), stop=(c == CA - 1))
            s_sb = small.tile([128, J * D], f32, tag="s_sb")
            nc.scalar.copy(s_sb[:, :], ps[:, :])
            return ps, s_sb

        def squash(ch, s_ps, s_sb, want_bf16):
            """v = s * |s| / (1 + |s|^2), norm over d."""
            s2 = small.tile([128, J * D], f32, tag="s2")
            nc.scalar.activation(s2[:, :], s_ps[:, :], Act.Square)
            n2 = small.tile([128, J], f32, tag="n2")
            nc.vector.tensor_reduce(
                n2[:, :], s2[:, :].rearrange("p (j d) -> p j d", j=J), AxX,
                Alu.add)
            n2p1 = small.tile([128, J], f32, tag="n2p1")
            nc.scalar.add(n2p1[:, :], n2[:, :], 1.0)
            r = small.tile([128, J], f32, tag="rcp")
            nc.vector.reciprocal(r[:, :], n2p1[:, :])
            nr = small.tile([128, J], f32, tag="nrm")
            nc.scalar.activation(nr[:, :], n2[:, :], Act.Sqrt)
            f = small.tile([128, J], f32, tag="fac")
            nc.vector.tensor_tensor(f[:, :], nr[:, :], r[:, :], Alu.mult)
            fb = f[:, :].unsqueeze(2).broadcast_to([128, J, D])
            if want_bf16:
                vt = vpool.tile([128, J * D], bf16, tag="vtb")
            else:
                vt = vpool.tile([128, J * D], f32, tag="vtf")
            nc.vector.tensor_tensor(
                vt[:, :].rearrange("p (j d) -> p j d", j=J),
                s_sb[:, :].rearrange("p (j d) -> p j d", j=J), fb, Alu.mult)
            return vt

        def u_step(ch, vt):
            """u[p,(j,c)] = sum_d v[p,(j,d)] * Wa[p,(j,c,d)]. Out bf16
            padded to stride CP."""
            eng = prod_engine(ch)
            pu = spool.tile([128, J * CA * D], bf16, tag="produ")
            pu4 = pu[:, :].rearrange("p (j c d) -> p j c d", j=J, c=CA)
            wu4 = wu_t[:, :].rearrange("p (j c d) -> p j c d", j=J, c=CA)
            vb = (vt[:, :].rearrange("p (j d) -> p j d", j=J)
                  .unsqueeze(2).broadcast_to([128, J, CA, D]))
            eng.tensor_tensor(pu4, vb, wu4, Alu.mult)
            puz = pu[:, :].rearrange("p (a d) -> p a d", d=D)
            uA = spool.tile([128, 90 * 8], bf16, tag="treeUA")
            uA3 = uA[:, :].rearrange("p (a c) -> p a c", c=8)
            nc.vector.tensor_tensor(uA3, puz[:, :, 0:8], puz[:, :, 8:16],
                                    Alu.add)
            uB = spool.tile([128, 90 * 4], bf16, tag="treeUB")
            uB3 = uB[:, :].rearrange("p (a c) -> p a c", c=4)
            nc.vector.tensor_tensor(uB3, uA3[:, :, 0:4], uA3[:, :, 4:8],
                                    Alu.add)
            uC = spool.tile([128, 90 * 2], bf16, tag="treeUC")
            uC3 = uC[:, :].rearrange("p (a c) -> p a c", c=2)
            nc.vector.tensor_tensor(uC3, uB3[:, :, 0:2], uB3[:, :, 2:4],
                                    Alu.add)
            u = small.tile([128, J * CP], bf16, tag="u")
            u3 = u[:, :].rearrange("p (j c) -> p j c", j=J)[:, :, 0:CA]
            nc.vector.tensor_tensor(u3, uC3[:, :, 0], uC3[:, :, 1],
                                    Alu.add)
            return u

        def e_heavy(ch, u, out_js):
            """db[p,(j,s)] = sum_c u[p,(j,c)] * x[p,(s,c)] -> out_js fp32."""
            eng = prod_engine(ch)
            pe = spool.tile([128, J * S * CP], bf16, tag="prodE")
            pe4 = (pe[:, :].rearrange("p (j s c) -> p j s c", j=J, s=S)
                   [:, :, :, 0:CA])
            ub = (u[:, :].rearrange("p (j c) -> p j c", j=J)[:, :, 0:CA]
                  .unsqueeze(2).broadcast_to([128, J, S, CA]))
            xb = (Xsc[ch][:, :].rearrange("p (s c) -> p s c", s=S)
                  [:, :, 0:CA].unsqueeze(1)
                  .broadcast_to([128, J, S, CA]))
            eng.tensor_tensor(pe4, ub, xb, Alu.mult)
            # pe layout (j, s, c10): (j,s) merges; tree-sum over c
            pez = pe[:, :].rearrange("p (a c) -> p a c", c=CP)
            eA = spool.tile([128, 360 * 4], bf16, tag="treeEA")
            eA3 = eA[:, :].rearrange("p (a c) -> p a c", c=4)
            nc.vector.tensor_tensor(eA3, pez[:, :, 0:4], pez[:, :, 4:8],
                                    Alu.add)
            eB = spool.tile([128, 360 * 2], bf16, tag="treeEB")
            eB3 = eB[:, :].rearrange("p (a c) -> p a c", c=2)
            nc.vector.tensor_tensor(eB3, eA3[:, :, 0:2], eA3[:, :, 2:4],
                                    Alu.add)
            nc.vector.tensor_tensor(out_js, eB3[:, :, 0], eB3[:, :, 1],
                                    Alu.add)
            nc.vector.scalar_tensor_tensor(out_js, pez[:, :, 8], 1.0,
                                           out_js, Alu.mult, Alu.add)

        def softmax(ch):
            """c[p,(j,s)] = softmax_j(L). Returns bf16 C tile."""
            et = spool.tile([128, J * S], f32, tag="expt")
            nc.scalar.activation(et[:, :], L[ch][:, :], Act.Exp)
            z = small.tile([128, S], f32, tag="z")
            # reduce over j: view [p, s(outer, stride 1), j(inner, stride S)]
            ejs = (et[:, :].rearrange("p (j s) -> p j s", j=J)
                   .transpose([0, 2, 1]))
            nc.vector.tensor_reduce(z[:, :], ejs, AxX, Alu.add)
            zr = small.tile([128, S], f32, tag="zr")
            nc.vector.reciprocal(zr[:, :], z[:, :])
            ct = spool.tile([128, J * S], bf16, tag="ct")
            zb = zr[:, :].unsqueeze(1).broadcast_to([128, J, S])
            nc.vector.tensor_tensor(
                ct[:, :].rearrange("p (j s) -> p j s", j=J),
                et[:, :].rearrange("p (j s) -> p j s", j=J), zb, Alu.mult)
            return ct

        def b_heavy(ch, ct):
            """t[p,(j,c)] = sum_s c[p,(j,s)] * x[p,(c,s)]. Out bf16 padded
            to stride CP."""
            eng = prod_engine(ch)
            pb = spool.tile([128, J * CA * S], bf16, tag="prodB")
            pb4 = pb[:, :].rearrange("p (j c s) -> p j c s", j=J, c=CA)
            cb = (ct[:, :].rearrange("p (j s) -> p j s", j=J)
                  .unsqueeze(2).broadcast_to([128, J, CA, S]))
            xb = (Xcs[ch][:, :].rearrange("p (c s) -> p c s", c=CA)
                  .unsqueeze(1).broadcast_to([128, J, CA, S]))
            eng.tensor_tensor(pb4, cb, xb, Alu.mult)
            pbz = pb[:, :].rearrange("p (a s) -> p a s", s=S)
            bA = spool.tile([128, 90 * 16], bf16, tag="treeBA")
            bA3 = bA[:, :].rearrange("p (a c) -> p a c", c=16)
            nc.vector.tensor_tensor(bA3, pbz[:, :, 0:16], pbz[:, :, 16:32],
                                    Alu.add)
            bB = spool.tile([128, 90 * 8], bf16, tag="treeBB")
            bB3 = bB[:, :].rearrange("p (a c) -> p a c", c=8)
            nc.vector.tensor_tensor(bB3, bA3[:, :, 0:8], bA3[:, :, 8:16],
                                    Alu.add)
            bC = spool.tile([128, 90 * 4], bf16, tag="treeBC")
            bC3 = bC[:, :].rearrange("p (a c) -> p a c", c=4)
            nc.vector.tensor_tensor(bC3, bB3[:, :, 0:4], bB3[:, :, 4:8],
                                    Alu.add)
            # tail s=32..35 pairs
            bT = spool.tile([128, 90 * 2], bf16, tag="treeBT")
            bT3 = bT[:, :].rearrange("p (a c) -> p a c", c=2)
            nc.vector.tensor_tensor(bT3, pbz[:, :, 32:34], pbz[:, :, 34:36],
                                    Alu.add)
            bD = spool.tile([128, 90 * 2], bf16, tag="treeBD")
            bD3 = bD[:, :].rearrange("p (a c) -> p a c", c=2)
            nc.vector.tensor_tensor(bD3, bC3[:, :, 0:2], bC3[:, :, 2:4],
                                    Alu.add)
            bE = spool.tile([128, 90 * 2], f32, tag="treeBE")
            bE3 = bE[:, :].rearrange("p (a c) -> p a c", c=2)
            nc.vector.tensor_tensor(bE3, bD3[:, :, :], bT3[:, :, :],
                                    Alu.add)
            t = small.tile([128, J * CP], bf16, tag="tt")
            t3 = t[:, :].rearrange("p (j c) -> p j c", j=J)[:, :, 0:CA]
            nc.vector.tensor_tensor(t3, bE3[:, :, 0], bE3[:, :, 1],
                                    Alu.add)
            return t

        def t_bcast(t):
            """[p, (j, c-padded)] bf16 -> broadcast AP [p, J, D, CA]."""
            return (t[:, :].rearrange("p (j c) -> p j c", j=J)[:, :, 0:CA]
                    .unsqueeze(2).broadcast_to([128, J, D, CA]))

        for ch in range(NCH):
            # ---- iteration 1 (uniform c = 1/J) ----
            xsum = small.tile([128, CA], bf16, tag="xsum")
            with nc.allow_low_precision("bf16 routing intermediates"):
                nc.vector.tensor_reduce(
                    xsum[:, :],
                    Xcs[ch][:, :].rearrange("p (c s) -> p c s", c=CA), AxX,
                    Alu.add)
            xs1 = small.tile([128, CA], bf16, tag="xsum1")
            nc.scalar.mul(xs1[:, :], xsum[:, :], 1.0 / J)
            xs_b = (xs1[:, :].unsqueeze(1).unsqueeze(1)
                    .broadcast_to([128, J, D, CA]))
            s_ps, s_sb = c_step(ch, xs_b)
            vt = squash(ch, s_ps, s_sb, want_bf16=True)
            u = u_step(ch, vt)
            e_heavy(ch, u, L[ch][:, :])  # L = db1  (b was zero)

            # ---- iteration 2 ----
            ct = softmax(ch)
            t = b_heavy(ch, ct)
            s_ps, s_sb = c_step(ch, t_bcast(t))
            vt = squash(ch, s_ps, s_sb, want_bf16=True)
            u = u_step(ch, vt)
            db = spool.tile([128, J * S], f32, tag="db2")
            e_heavy(ch, u, db[:, :])
            nc.vector.tensor_tensor(L[ch][:, :], L[ch][:, :], db[:, :],
                                    Alu.add)

            # ---- iteration 3 (only v needed) ----
            ct = softmax(ch)
            t = b_heavy(ch, ct)
            s_ps, s_sb = c_step(ch, t_bcast(t))
            vt = squash(ch, s_ps, s_sb, want_bf16=False)
            dmae.dma_start(v_d[ch * B4:(ch + 1) * B4, :],
                                vt[0:128:NS, :])

    if split_waits:
        _split_multi_waits(nc)
    return nc


def _split_multi_waits(nc):
    """Walrus's cayman codegen allows exactly ONE sync wait per TPB
    instruction (NEURON_ISA_TPB_EVENTS has a single wait slot). Tile's
    scheduler attaches several waits to dependency-merge instructions,
    which the native bass encoder handles but the neuronx-cc path rejects
    ("Too many sync wait commands"). Split the extras onto engine-local
    NoOp instructions inserted immediately before the owner so the wait
    semantics are unchanged.
    """
    from concourse import mybir

    for bbname, bbwrap in nc.bb_map.items():
        bb = bbwrap.bb
        insts = bb.instructions
        i = 0
        while i < len(insts):
            ins = insts[i]
            si = getattr(ins, "sync_info", None)
            if si is None or len(si.on_wait or []) <= 1:
                i += 1
                continue
            waits = list(si.on_wait)
            engine = ins.engine
            for w in waits[:-1]:
                nop = mybir.InstNoOp(
                    name=nc.get_next_instruction_name(),
                    engine=engine,
                    bass_nofuse=True,
                    sync_info=mybir.SyncInfo(on_wait=[w], on_update=[]),
                )
                insts.insert(i, nop)
                i += 1
            ins.sync_info = mybir.SyncInfo(on_wait=[waits[-1]],
                                           on_update=si.on_update)
            i += 1


def _get_program(split_waits=True, dve_chunks=8, dma_eng="sync"):
    key = ("nc", split_waits, dve_chunks, dma_eng)
    if key not in _CACHE:
        _CACHE[key] = _build_program(split_waits, dve_chunks, dma_eng)
    return _CACHE[key]


def _host_prep(x, W, bias):
    """Build per-core input maps."""
    bf = np.float16
    x = np.ascontiguousarray(x, dtype=np.float32)
    W = np.ascontiguousarray(W, dtype=np.float32)
    bias = np.ascontiguousarray(bias, dtype=np.float32)
    bs = x.shape[0]

    xga = x.reshape(bs, NS, C_IN, S)
    xa = np.concatenate(
        [xga, np.ones((bs, NS, 1, S), dtype=np.float32)], axis=2)
    # [core, ch, b4, g, c, s]
    x6 = xa.reshape(NCORES, NCH, B4, NS, CA, S)
    xcs = np.ascontiguousarray(x6).reshape(
        NCORES, NCH, 128, CA * S).astype(bf)
    x6sc = x6.transpose(0, 1, 2, 3, 5, 4)      # [.., s, c]
    x6sp = np.concatenate(
        [x6sc, np.zeros(x6sc.shape[:-1] + (CP - CA,), np.float32)], axis=-1)
    xsc = np.ascontiguousarray(x6sp).reshape(
        NCORES, NCH, 128, S * CP).astype(bf)

    Wa = np.concatenate(
        [W.reshape(NS, J, D, C_IN),
         bias.reshape(NS, J, D, 1)], axis=3)            # [g, j, d, c]
    Wap = np.concatenate(
        [Wa, np.zeros(Wa.shape[:-1] + (CP - CA,), np.float32)], axis=-1)
    wc = np.tile(Wap.reshape(NS, J * D * CP), (B4, 1)).astype(bf)
    wu = np.tile(
        Wa.transpose(0, 1, 3, 2).reshape(NS, J * CA * D),
        (B4, 1)).astype(bf)                             # [128, (j,c,d)]
    onesb = np.kron(np.eye(B4, dtype=np.float32),
                    np.ones((NS, NS), dtype=np.float32)).astype(bf)

    in_maps = [
        {"xcs": np.ascontiguousarray(xcs[k]),
         "xsc": np.ascontiguousarray(xsc[k]),
         "wc": wc, "wu": wu, "onesb": onesb}
        for k in range(NCORES)
    ]
    return in_maps


def kernel(x, W, bias, b0):
    from concourse.bass_utils import run_bass_kernel_spmd

    nc = _get_program()
    in_maps = _host_prep(x, W, bias)
    res = run_bass_kernel_spmd(nc, in_maps, list(range(NCORES)))
    out = np.concatenate([res.results[k]["v"] for k in range(NCORES)],
                         axis=0)
    return np.ascontiguousarray(out.reshape(NCORES * BLOC, J, D))



# revision 5
# speedup vs baseline: 1.3139x; 1.3139x over previous
# Trainium2 Bass kernel for nn_CapLayer (CapsNet grouped 1x1 conv + dynamic routing).
#
# Key algebraic restructuring: the huge intermediate pred[b, i=(g,s), (j,d)]
# (188MB for the full batch) is NEVER materialized. Routing is computed in a
# factored form:
#   pred[b,(g,s),(j,d)] = sum_c Wa[g,j,d,c] * xga[b,g,c,s]     (c augmented with
#                                                               a ones channel to
#                                                               absorb the bias)
#   t[b,j,g,c]  = sum_s c[b,j,(g,s)] * xga[b,g,c,s]
#   s[b,j,d]    = sum_{g,c} t[b,j,g,c] * Wa[g,j,d,c]
#   u[b,j,g,c]  = sum_d v[b,j,d] * Wa[g,j,d,c]
#   db[b,j,g,s] = sum_c u[b,j,g,c] * xga[b,g,c,s]
# Iteration 1 collapses (softmax of zeros is uniform): t1 = xsum / J.
# The squash factor f = |s|/(1+|s|^2) is folded into u (u = f * (s @ W))
# instead of materializing v, which takes the scalar squash chain off the
# critical path between the PE matmul and the next product.
#
# Sharding: pure data parallel, 32 samples per core across 8 cores.
# On-chip layout: partition p = (b4, g) with 4 samples x 32 groups = 128
# partitions; 8 chunks cover the 32 local samples. The g-contraction for
# s[b,(j,d)] is done on the TensorEngine with a block-diagonal ones matrix,
# which also replicates s across the g-partitions for free (so u stays
# in the same partition layout).
#
# Schedule: stages are emitted in a wavefront across all 8 chunks (stage-major
# order) so each engine always has independent work from other chunks while
# one chunk traverses the cross-engine chain (DVE product -> PE matmul ->
# Act copy -> DVE tree). Scratch tile pools rotate bufs=4 deep. A tunable
# subset of the big broadcast-products runs on GPSIMD (Pool) to balance
# engine load against DVE.

import sys

import numpy as np

# concourse (Bass/Tile) ships with the container; make sure it's importable
# when the grader runs kernel.py from a bare directory.
for _p in ("/opt/trn_rl_repo", "/root/.axon_site/_ro/trn_rl_repo"):
    if _p not in sys.path:
        sys.path.insert(0, _p)

NS, J, D, C_IN, H, WID, RN = 32, 10, 16, 8, 6, 6, 3
S = H * WID            # 36 spatial positions
CA = C_IN + 1          # 9 channels including the ones channel
CP = 10                # padded channel stride (4B alignment for bf16 rows)
NCORES = 8
BLOC = 32              # samples per core
B4 = 4                 # samples per chunk
NCH = BLOC // B4       # 8 chunks

# Chunk subsets routed to GPSIMD (Pool), spread evenly across the wavefront.
_SPREAD = [0, 4, 2, 6, 1, 5, 3, 7]

_CACHE = {}


def _build_program(split_waits=True, pool_pb=8, pool_pe=4, pool_pc=0,
                   pool_pu=0, bufs=4, dma_eng="sync"):
    from contextlib import ExitStack

    import concourse.bass as bass
    import concourse.tile as tile
    from concourse import mybir

    f32 = mybir.dt.float32
    bf16 = mybir.dt.float16
    Alu = mybir.AluOpType
    Act = mybir.ActivationFunctionType
    AxX = mybir.AxisListType.X

    nc = bass.Bass("TRN2", target_bir_lowering=True, debug=False,
                   num_devices=NCORES)

    xcs_d = nc.dram_tensor("xcs", [NCH, 128, CA * S], bf16,
                           kind="ExternalInput").ap()      # free = (c, s)
    xsc_d = nc.dram_tensor("xsc", [NCH, 128, S * CP], bf16,
                           kind="ExternalInput").ap()      # free = (s, c10)
    wc_d = nc.dram_tensor("wc", [128, J * D * CP], bf16,
                          kind="ExternalInput").ap()       # free = (j, d, c10)
    wu_d = nc.dram_tensor("wu", [128, J * CA * D], bf16,
                          kind="ExternalInput").ap()       # free = (j, c, d)
    ones_d = nc.dram_tensor("onesb", [128, 128], bf16,
                            kind="ExternalInput").ap()     # blockdiag over b4
    v_d = nc.dram_tensor("v", [BLOC, J * D], f32,
                         kind="ExternalOutput").ap()

    sel_pb = set(_SPREAD[:pool_pb])
    sel_pe = set(_SPREAD[:pool_pe])
    sel_pc = set(_SPREAD[:pool_pc])
    sel_pu = set(_SPREAD[:pool_pu])

    dmae = {"gpsimd": nc.gpsimd, "sync": nc.sync}[dma_eng]
    with tile.TileContext(nc) as tc, ExitStack() as ctx:
        consts = ctx.enter_context(tc.tile_pool(name="consts", bufs=1))
        xpool = ctx.enter_context(tc.tile_pool(name="xpool", bufs=1))
        lpool = ctx.enter_context(tc.tile_pool(name="lpool", bufs=1))
        spool = ctx.enter_context(tc.tile_pool(name="scratch", bufs=bufs))
        small = ctx.enter_context(tc.tile_pool(name="small", bufs=bufs))
        vpool = ctx.enter_context(tc.tile_pool(name="vpool", bufs=bufs))
        psum = ctx.enter_context(tc.tile_pool(name="psum", bufs=4,
                                              space="PSUM"))

        wc_t = consts.tile([128, J * D * CP], bf16, tag="wc")
        dmae.dma_start(wc_t[:, :], wc_d[:, :])
        wu_t = consts.tile([128, J * CA * D], bf16, tag="wu")
        dmae.dma_start(wu_t[:, :], wu_d[:, :])
        ones_t = consts.tile([128, 128], bf16, tag="onesb")
        dmae.dma_start(ones_t[:, :], ones_d[:, :])

        # Persistent per-chunk tiles.
        Xcs = []   # xga [p, (c, s)] bf16
        Xsc = []   # xga [p, (s, c)] bf16
        L = []     # routing logits b, layout [p, (j, s)] bf16
        for ch in range(NCH):
            xt = xpool.tile([128, CA * S], bf16, tag=f"Xcs{ch}",
                            name=f"Xcs{ch}")
            dmae.dma_start(xt[:, :], xcs_d[ch, :, :])
            Xcs.append(xt)
            xt2 = xpool.tile([128, S * CP], bf16, tag=f"Xsc{ch}",
                             name=f"Xsc{ch}")
            dmae.dma_start(xt2[:, :], xsc_d[ch, :, :])
            Xsc.append(xt2)
            L.append(lpool.tile([128, J * S], bf16, tag=f"L{ch}",
                                name=f"L{ch}"))

        # Per-chunk in-flight state handed between stages.
        ST = [dict() for _ in range(NCH)]

        def peng(ch, sel):
            return nc.gpsimd if ch in sel else nc.vector

        # ---------------- stage functions ----------------

        def st_xsum(ch):
            xsum = small.tile([128, CA], bf16, tag="xsum", name="xsum")
            with nc.allow_low_precision("bf16 routing intermediates"):
                nc.vector.tensor_reduce(
                    xsum[:, :],
                    Xcs[ch][:, :].rearrange("p (c s) -> p c s", c=CA), AxX,
                    Alu.add)
            ST[ch]["xsum"] = xsum

        def st_xs1(ch):
            xs1 = small.tile([128, CA], bf16, tag="xsum1", name="xs1")
            nc.scalar.mul(xs1[:, :], ST[ch]["xsum"][:, :], 1.0 / J)
            ST[ch]["t"] = None
            ST[ch]["t_b"] = (xs1[:, :].unsqueeze(1).unsqueeze(1)
                             .broadcast_to([128, J, D, CA]))

        def st_pc(ch):
            # prodC[p, (j, d, c)] = t_bcast * Wa
            eng = peng(ch, sel_pc)
            pc = spool.tile([128, J * D * CP], bf16, tag="prodC", name="pc")
            pc4 = (pc[:, :].rearrange("p (j d c) -> p j d c", j=J, d=D)
                   [:, :, :, 0:CA])
            wc4 = (wc_t[:, :].rearrange("p (j d c) -> p j d c", j=J, d=D)
                   [:, :, :, 0:CA])
            eng.tensor_tensor(pc4, ST[ch]["t_b"], wc4, Alu.mult)
            ST[ch]["pc"] = pc

        def st_mm(ch):
            # PE contracts g (partitions, via blockdiag ones) AND c (PSUM
            # accumulation over the 9 channel slices).
            pcz = ST[ch]["pc"][:, :].rearrange("p (a c) -> p a c", c=CP)
            ps = psum.tile([128, J * D], f32, tag="psum_s", name="ps")
            for c in range(CA):
                nc.tensor.matmul(ps[:, :], ones_t[:, :], pcz[:, :, c],
                                 start=(c == 0), stop=(c == CA - 1))
            ST[ch]["ps"] = ps

        def st_scpy(ch):
            # s in bf16 for the u-product, plus s^2 for the squash norm.
            ps = ST[ch]["ps"]
            sb = small.tile([128, J * D], bf16, tag="s_b16", name="sb")
            nc.scalar.copy(sb[:, :], ps[:, :])
            s2 = small.tile([128, J * D], f32, tag="s2", name="s2")
            nc.scalar.activation(s2[:, :], ps[:, :], Act.Square)
            ST[ch]["sb"] = sb
            ST[ch]["s2"] = s2

        def st_scpy3(ch):
            ps = ST[ch]["ps"]
            s_sb = small.tile([128, J * D], f32, tag="s_sb", name="s_sb")
            nc.scalar.copy(s_sb[:, :], ps[:, :])
            s2 = small.tile([128, J * D], f32, tag="s2", name="s2")
            nc.scalar.activation(s2[:, :], ps[:, :], Act.Square)
            ST[ch]["s_sb"] = s_sb
            ST[ch]["s2"] = s2

        def st_n2(ch):
            n2 = small.tile([128, J], f32, tag="n2", name="n2")
            nc.vector.tensor_reduce(
                n2[:, :],
                ST[ch]["s2"][:, :].rearrange("p (j d) -> p j d", j=J), AxX,
                Alu.add)
            ST[ch]["n2"] = n2

        def st_act2(ch):
            n2 = ST[ch]["n2"]
            n2p1 = small.tile([128, J], f32, tag="n2p1", name="n2p1")
            nc.scalar.add(n2p1[:, :], n2[:, :], 1.0)
            nr = small.tile([128, J], f32, tag="nrm", name="nr")
            nc.scalar.activation(nr[:, :], n2[:, :], Act.Sqrt)
            ST[ch]["n2p1"] = n2p1
            ST[ch]["nr"] = nr

        def st_rf(ch):
            r = small.tile([128, J], f32, tag="rcp", name="r")
            nc.vector.reciprocal(r[:, :], ST[ch]["n2p1"][:, :])
            f = small.tile([128, J], f32, tag="fac", name="f")
            nc.vector.tensor_tensor(f[:, :], ST[ch]["nr"][:, :], r[:, :],
                                    Alu.mult)
            ST[ch]["f"] = f

        def st_pu(ch):
            # produ[p, (j, c, d)] = s_bcast * Wa  (squash factor applied
            # later, on the d-reduced output)
            eng = peng(ch, sel_pu)
            pu = spool.tile([128, J * CA * D], bf16, tag="produ", name="pu")
            pu4 = pu[:, :].rearrange("p (j c d) -> p j c d", j=J, c=CA)
            wu4 = wu_t[:, :].rearrange("p (j c d) -> p j c d", j=J, c=CA)
            sbb = (ST[ch]["sb"][:, :].rearrange("p (j d) -> p j d", j=J)
                   .unsqueeze(2).broadcast_to([128, J, CA, D]))
            eng.tensor_tensor(pu4, sbb, wu4, Alu.mult)
            ST[ch]["pu"] = pu

        def st_utree(ch):
            puz = ST[ch]["pu"][:, :].rearrange("p (a d) -> p a d", d=D)
            uA = spool.tile([128, 90 * 8], bf16, tag="tr720", name="uA")
            uA3 = uA[:, :].rearrange("p (a c) -> p a c", c=8)
            nc.vector.tensor_tensor(uA3, puz[:, :, 0:8], puz[:, :, 8:16],
                                    Alu.add)
            uB = spool.tile([128, 90 * 4], bf16, tag="tr360", name="uB")
            uB3 = uB[:, :].rearrange("p (a c) -> p a c", c=4)
            nc.vector.tensor_tensor(uB3, uA3[:, :, 0:4], uA3[:, :, 4:8],
                                    Alu.add)
            uC = spool.tile([128, 90 * 2], bf16, tag="tr180c", name="uC")
            uC3 = uC[:, :].rearrange("p (a c) -> p a c", c=2)
            nc.vector.tensor_tensor(uC3, uB3[:, :, 0:2], uB3[:, :, 2:4],
                                    Alu.add)
            ut = small.tile([128, J * CP], bf16, tag="ut", name="ut")
            ut3 = ut[:, :].rearrange("p (j c) -> p j c", j=J)[:, :, 0:CA]
            nc.vector.tensor_tensor(ut3, uC3[:, :, 0], uC3[:, :, 1],
                                    Alu.add)
            ST[ch]["ut"] = ut

        def st_uscale(ch):
            # u = f * (s @ W): fold the squash factor into u.
            u = small.tile([128, J * CP], bf16, tag="u", name="u")
            u3 = u[:, :].rearrange("p (j c) -> p j c", j=J)[:, :, 0:CA]
            ut3 = (ST[ch]["ut"][:, :].rearrange("p (j c) -> p j c", j=J)
                   [:, :, 0:CA])
            fb = (ST[ch]["f"][:, :].unsqueeze(2)
                  .broadcast_to([128, J, CA]))
            nc.vector.tensor_tensor(u3, ut3, fb, Alu.mult)
            ST[ch]["u"] = u

        def st_pe(ch):
            # prodE[p, (j, s, c)] over the 8 real channels only; the ones
            # channel (u[j,8]) is added directly into L afterwards.
            eng = peng(ch, sel_pe)
            pe = spool.tile([128, J * S * CP], bf16, tag="bigP", name="pe")
            pe4 = (pe[:, :].rearrange("p (j s c) -> p j s c", j=J, s=S)
                   [:, :, :, 0:C_IN])
            ub = (ST[ch]["u"][:, :].rearrange("p (j c) -> p j c", j=J)
                  [:, :, 0:C_IN].unsqueeze(2)
                  .broadcast_to([128, J, S, C_IN]))
            xb = (Xsc[ch][:, :].rearrange("p (s c) -> p s c", s=S)
                  [:, :, 0:C_IN].unsqueeze(1)
                  .broadcast_to([128, J, S, C_IN]))
            eng.tensor_tensor(pe4, ub, xb, Alu.mult)
            ST[ch]["pe"] = pe

        def st_etree(ch):
            pez = ST[ch]["pe"][:, :].rearrange("p (a c) -> p a c", c=CP)
            eA = spool.tile([128, 360 * 4], bf16, tag="tr1440", name="eA")
            eA3 = eA[:, :].rearrange("p (a c) -> p a c", c=4)
            nc.vector.tensor_tensor(eA3, pez[:, :, 0:4], pez[:, :, 4:8],
                                    Alu.add)
            eB = spool.tile([128, 360 * 2], bf16, tag="tr720", name="eB")
            eB3 = eB[:, :].rearrange("p (a c) -> p a c", c=2)
            nc.vector.tensor_tensor(eB3, eA3[:, :, 0:2], eA3[:, :, 2:4],
                                    Alu.add)
            eC = spool.tile([128, J * S], bf16, tag="eC", name="eC")
            nc.vector.tensor_tensor(eC[:, :], eB3[:, :, 0], eB3[:, :, 1],
                                    Alu.add)
            ST[ch]["eC"] = eC

        def _u8b(ch):
            return (ST[ch]["u"][:, :].rearrange("p (j c) -> p j c", j=J)
                    [:, :, 8].unsqueeze(2).broadcast_to([128, J, S]))

        def st_lupd1(ch):
            # L = db1 = eC + u[j,8]  (b was zero)
            L3 = L[ch][:, :].rearrange("p (j s) -> p j s", j=J)
            eC3 = ST[ch]["eC"][:, :].rearrange("p (j s) -> p j s", j=J)
            nc.vector.tensor_tensor(L3, eC3, _u8b(ch), Alu.add)

        def st_lupd2(ch):
            L3 = L[ch][:, :].rearrange("p (j s) -> p j s", j=J)
            eC3 = ST[ch]["eC"][:, :].rearrange("p (j s) -> p j s", j=J)
            nc.vector.tensor_tensor(L3, L3, eC3, Alu.add)
            nc.vector.tensor_tensor(L3, L3, _u8b(ch), Alu.add)

        def st_exp(ch):
            et = spool.tile([128, J * S], bf16, tag="expt", name="et")
            nc.scalar.activation(et[:, :], L[ch][:, :], Act.Exp)
            ST[ch]["et"] = et

        def st_zred(ch):
            z = small.tile([128, S], f32, tag="z", name="z")
            ejs = (ST[ch]["et"][:, :].rearrange("p (j s) -> p j s", j=J)
                   .transpose([0, 2, 1]))
            nc.vector.tensor_reduce(z[:, :], ejs, AxX, Alu.add)
            ST[ch]["z"] = z

        def st_zr(ch):
            zr = small.tile([128, S], bf16, tag="zr", name="zr")
            with nc.allow_low_precision("bf16 softmax normalizer"):
                nc.vector.reciprocal(zr[:, :], ST[ch]["z"][:, :])
            ST[ch]["zr"] = zr

        def st_ct(ch):
            ct = spool.tile([128, J * S], bf16, tag="ct", name="ct")
            zb = ST[ch]["zr"][:, :].unsqueeze(1).broadcast_to([128, J, S])
            nc.vector.tensor_tensor(
                ct[:, :].rearrange("p (j s) -> p j s", j=J),
                ST[ch]["et"][:, :].rearrange("p (j s) -> p j s", j=J), zb,
                Alu.mult)
            ST[ch]["ct"] = ct

        def st_pb(ch):
            eng = peng(ch, sel_pb)
            pb = spool.tile([128, J * CA * S], bf16, tag="bigP", name="pb")
            pb4 = pb[:, :].rearrange("p (j c s) -> p j c s", j=J, c=CA)
            cb = (ST[ch]["ct"][:, :].rearrange("p (j s) -> p j s", j=J)
                  .unsqueeze(2).broadcast_to([128, J, CA, S]))
            xb = (Xcs[ch][:, :].rearrange("p (c s) -> p c s", c=CA)
                  .unsqueeze(1).broadcast_to([128, J, CA, S]))
            eng.tensor_tensor(pb4, cb, xb, Alu.mult)
            ST[ch]["pb"] = pb

        def st_btree(ch):
            pbz = ST[ch]["pb"][:, :].rearrange("p (a s) -> p a s", s=S)
            bA = spool.tile([128, 90 * 16], bf16, tag="tr1440", name="bA")
            bA3 = bA[:, :].rearrange("p (a c) -> p a c", c=16)
            nc.vector.tensor_tensor(bA3, pbz[:, :, 0:16], pbz[:, :, 16:32],
                                    Alu.add)
            bB = spool.tile([128, 90 * 8], bf16, tag="tr720", name="bB")
            bB3 = bB[:, :].rearrange("p (a c) -> p a c", c=8)
            nc.vector.tensor_tensor(bB3, bA3[:, :, 0:8], bA3[:, :, 8:16],
                                    Alu.add)
            bC = spool.tile([128, 90 * 4], bf16, tag="tr360", name="bC")
            bC3 = bC[:, :].rearrange("p (a c) -> p a c", c=4)
            nc.vector.tensor_tensor(bC3, bB3[:, :, 0:4], bB3[:, :, 4:8],
                                    Alu.add)
            # tail s=32..35 pairs
            bT = spool.tile([128, 90 * 2], bf16, tag="tr180a", name="bT")
            bT3 = bT[:, :].rearrange("p (a c) -> p a c", c=2)
            nc.vector.tensor_tensor(bT3, pbz[:, :, 32:34], pbz[:, :, 34:36],
                                    Alu.add)
            bD = spool.tile([128, 90 * 2], bf16, tag="tr180b", name="bD")
            bD3 = bD[:, :].rearrange("p (a c) -> p a c", c=2)
            nc.vector.tensor_tensor(bD3, bC3[:, :, 0:2], bC3[:, :, 2:4],
                                    Alu.add)
            bE = spool.tile([128, 90 * 2], bf16, tag="tr180c", name="bE")
            bE3 = bE[:, :].rearrange("p (a c) -> p a c", c=2)
            nc.vector.tensor_tensor(bE3, bD3[:, :, :], bT3[:, :, :],
                                    Alu.add)
            t = small.tile([128, J * CP], bf16, tag="tt", name="t")
            t3 = t[:, :].rearrange("p (j c) -> p j c", j=J)[:, :, 0:CA]
            nc.vector.tensor_tensor(t3, bE3[:, :, 0], bE3[:, :, 1],
                                    Alu.add)
            ST[ch]["t"] = t
            ST[ch]["t_b"] = (t[:, :].rearrange("p (j c) -> p j c", j=J)
                             [:, :, 0:CA].unsqueeze(2)
                             .broadcast_to([128, J, D, CA]))

        def st_vt(ch):
            # final v = s * f, fp32
            vt = vpool.tile([128, J * D], f32, tag="vtf", name="vt")
            fb = (ST[ch]["f"][:, :].unsqueeze(2)
                  .broadcast_to([128, J, D]))
            nc.vector.tensor_tensor(
                vt[:, :].rearrange("p (j d) -> p j d", j=J),
                ST[ch]["s_sb"][:, :].rearrange("p (j d) -> p j d", j=J), fb,
                Alu.mult)
            ST[ch]["vt"] = vt

        def st_out(ch):
            dmae.dma_start(v_d[ch * B4:(ch + 1) * B4, :],
                           ST[ch]["vt"][0:128:NS, :])

        # ---------------- emission: stage-major wavefront ----------------
        # Chunks are processed in groups of `bufs`: within a group, stages
        # are emitted stage-major (so every engine has independent work from
        # the other chunks of the group), and every tile's consumers are
        # emitted before the next group recycles its buffer.

        def emit(stages):
            for g0 in range(0, NCH, bufs):
                for fn in stages:
                    for ch in range(g0, min(g0 + bufs, NCH)):
                        fn(ch)

        c_sq_u = [st_pc, st_mm, st_scpy, st_n2, st_act2, st_rf, st_pu,
                  st_utree, st_uscale, st_pe, st_etree]
        softmax_b = [st_exp, st_zred, st_zr, st_ct, st_pb, st_btree]

        emit([st_xsum, st_xs1] + c_sq_u + [st_lupd1])
        emit(softmax_b + c_sq_u + [st_lupd2])
        emit(softmax_b + [st_pc, st_mm, st_scpy3, st_n2, st_act2, st_rf,
                          st_vt, st_out])

    if split_waits:
        _split_multi_waits(nc)
    return nc


def _split_multi_waits(nc):
    """Walrus's cayman codegen allows exactly ONE sync wait per TPB
    instruction (NEURON_ISA_TPB_EVENTS has a single wait slot). Tile's
    scheduler attaches several waits to dependency-merge instructions,
    which the native bass encoder handles but the neuronx-cc path rejects
    ("Too many sync wait commands"). Split the extras onto engine-local
    NoOp instructions inserted immediately before the owner so the wait
    semantics are unchanged.
    """
    from concourse import mybir

    for bbname, bbwrap in nc.bb_map.items():
        bb = bbwrap.bb
        insts = bb.instructions
        i = 0
        while i < len(insts):
            ins = insts[i]
            si = getattr(ins, "sync_info", None)
            if si is None or len(si.on_wait or []) <= 1:
                i += 1
                continue
            waits = list(si.on_wait)
            engine = ins.engine
            for w in waits[:-1]:
                nop = mybir.InstNoOp(
                    name=nc.get_next_instruction_name(),
                    engine=engine,
                    bass_nofuse=True,
                    sync_info=mybir.SyncInfo(on_wait=[w], on_update=[]),
                )
                insts.insert(i, nop)
                i += 1
            ins.sync_info = mybir.SyncInfo(on_wait=[waits[-1]],
                                           on_update=si.on_update)
            i += 1


def _get_program(split_waits=True, **kw):
    key = ("nc", split_waits, tuple(sorted(kw.items())))
    if key not in _CACHE:
        _CACHE[key] = _build_program(split_waits, **kw)
    return _CACHE[key]


def _host_prep(x, W, bias):
    """Build per-core input maps."""
    bf = np.float16
    x = np.ascontiguousarray(x, dtype=np.float32)
    W = np.ascontiguousarray(W, dtype=np.float32)
    bias = np.ascontiguousarray(bias, dtype=np.float32)
    bs = x.shape[0]

    xga = x.reshape(bs, NS, C_IN, S)
    xa = np.concatenate(
        [xga, np.ones((bs, NS, 1, S), dtype=np.float32)], axis=2)
    # [core, ch, b4, g, c, s]
    x6 = xa.reshape(NCORES, NCH, B4, NS, CA, S)
    xcs = np.ascontiguousarray(x6).reshape(
        NCORES, NCH, 128, CA * S).astype(bf)
    x6sc = x6.transpose(0, 1, 2, 3, 5, 4)      # [.., s, c]
    x6sp = np.concatenate(
        [x6sc, np.zeros(x6sc.shape[:-1] + (CP - CA,), np.float32)], axis=-1)
    xsc = np.ascontiguousarray(x6sp).reshape(
        NCORES, NCH, 128, S * CP).astype(bf)

    Wa = np.concatenate(
        [W.reshape(NS, J, D, C_IN),
         bias.reshape(NS, J, D, 1)], axis=3)            # [g, j, d, c]
    Wap = np.concatenate(
        [Wa, np.zeros(Wa.shape[:-1] + (CP - CA,), np.float32)], axis=-1)
    wc = np.tile(Wap.reshape(NS, J * D * CP), (B4, 1)).astype(bf)
    wu = np.tile(
        Wa.transpose(0, 1, 3, 2).reshape(NS, J * CA * D),
        (B4, 1)).astype(bf)                             # [128, (j,c,d)]
    onesb = np.kron(np.eye(B4, dtype=np.float32),
                    np.ones((NS, NS), dtype=np.float32)).astype(bf)

    in_maps = [
        {"xcs": np.ascontiguousarray(xcs[k]),
         "xsc": np.ascontiguousarray(xsc[k]),
         "wc": wc, "wu": wu, "onesb": onesb}
        for k in range(NCORES)
    ]
    return in_maps


def kernel(x, W, bias, b0):
    from concourse.bass_utils import run_bass_kernel_spmd

    nc = _get_program()
    in_maps = _host_prep(x, W, bias)
    res = run_bass_kernel_spmd(nc, in_maps, list(range(NCORES)))
    out = np.concatenate([res.results[k]["v"] for k in range(NCORES)],
                         axis=0)
    return np.ascontiguousarray(out.reshape(NCORES * BLOC, J, D))


# revision 9
# speedup vs baseline: 1.6763x; 1.2758x over previous
# Trainium2 Bass kernel for nn_CapLayer (CapsNet grouped 1x1 conv + dynamic routing).
#
# Key algebraic restructuring: the huge intermediate pred[b, i=(g,s), (j,d)]
# (188MB for the full batch) is NEVER materialized. Routing is computed in a
# factored form:
#   pred[b,(g,s),(j,d)] = sum_c Wa[g,j,d,c] * xga[b,g,c,s]     (c augmented with
#                                                               a ones channel to
#                                                               absorb the bias)
#   t[b,j,g,c]  = sum_s c[b,j,(g,s)] * xga[b,g,c,s]
#   s[b,j,d]    = sum_{g,c} t[b,j,g,c] * Wa[g,j,d,c]
#   u[b,j,g,c]  = sum_d v[b,j,d] * Wa[g,j,d,c]
#   db[b,j,g,s] = sum_c u[b,j,g,c] * xga[b,g,c,s]
# Iteration 1 collapses (softmax of zeros is uniform): t1 = xsum / J.
# The squash factor f = |s|/(1+|s|^2) is folded into u (u = f * (s @ W))
# instead of materializing v, which takes the scalar squash chain off the
# critical path between the PE matmul and the next product.
#
# Sharding: pure data parallel, 32 samples per core across 8 cores.
# On-chip layout: partition p = (b4, g) with 4 samples x 32 groups = 128
# partitions; 8 chunks cover the 32 local samples. The g-contraction for
# s[b,(j,d)] is done on the TensorEngine with a block-diagonal ones matrix,
# which also replicates s across the g-partitions for free (so u stays
# in the same partition layout).
#
# Schedule: stages are emitted in a wavefront across all 8 chunks (stage-major
# order) so each engine always has independent work from other chunks while
# one chunk traverses the cross-engine chain (DVE product -> PE matmul ->
# Act copy -> DVE tree). Scratch tile pools rotate bufs=4 deep. A tunable
# subset of the big broadcast-products runs on GPSIMD (Pool) to balance
# engine load against DVE.

import sys

import numpy as np

# concourse (Bass/Tile) ships with the container; make sure it's importable
# when the grader runs kernel.py from a bare directory.
for _p in ("/opt/trn_rl_repo", "/root/.axon_site/_ro/trn_rl_repo"):
    if _p not in sys.path:
        sys.path.insert(0, _p)

NS, J, D, C_IN, H, WID, RN = 32, 10, 16, 8, 6, 6, 3
S = H * WID            # 36 spatial positions
CA = C_IN + 1          # 9 channels including the ones channel
CP = 10                # padded channel stride (4B alignment for bf16 rows)
NCORES = 8
BLOC = 32              # samples per core
B4 = 4                 # samples per chunk
NCH = BLOC // B4       # 8 chunks

# Chunk subsets routed to GPSIMD (Pool), spread evenly across the wavefront.
_SPREAD = [0, 4, 2, 6, 1, 5, 3, 7]

_CACHE = {}


def _build_program(split_waits=True, kb=3, ke=3, kc=2, ku=2,
                   bufs=4, dma_eng="sync"):
    from contextlib import ExitStack

    import concourse.bass as bass
    import concourse.tile as tile
    from concourse import mybir

    f32 = mybir.dt.float32
    bf16 = mybir.dt.float16
    Alu = mybir.AluOpType
    Act = mybir.ActivationFunctionType
    AxX = mybir.AxisListType.X

    nc = bass.Bass("TRN2", target_bir_lowering=True, debug=False,
                   num_devices=NCORES)

    xcs_d = nc.dram_tensor("xcs", [NCH, 128, CA * S], bf16,
                           kind="ExternalInput").ap()      # free = (c, s)
    xsc_d = nc.dram_tensor("xsc", [NCH, 128, S * CP], bf16,
                           kind="ExternalInput").ap()      # free = (s, c10)
    wc_d = nc.dram_tensor("wc", [128, J * D * CP], bf16,
                          kind="ExternalInput").ap()       # free = (j, d, c10)
    wu_d = nc.dram_tensor("wu", [128, J * CA * D], bf16,
                          kind="ExternalInput").ap()       # free = (j, c, d)
    ones_d = nc.dram_tensor("onesb", [128, 128], bf16,
                            kind="ExternalInput").ap()     # blockdiag over b4
    v_d = nc.dram_tensor("v", [BLOC, J * D], f32,
                         kind="ExternalOutput").ap()

    dmae = {"gpsimd": nc.gpsimd, "sync": nc.sync}[dma_eng]
    with tile.TileContext(nc) as tc, ExitStack() as ctx:
        consts = ctx.enter_context(tc.tile_pool(name="consts", bufs=1))
        xpool = ctx.enter_context(tc.tile_pool(name="xpool", bufs=1))
        lpool = ctx.enter_context(tc.tile_pool(name="lpool", bufs=1))
        spool = ctx.enter_context(tc.tile_pool(name="scratch", bufs=bufs))
        small = ctx.enter_context(tc.tile_pool(name="small", bufs=bufs))
        vpool = ctx.enter_context(tc.tile_pool(name="vpool", bufs=bufs))
        psum = ctx.enter_context(tc.tile_pool(name="psum", bufs=4,
                                              space="PSUM"))

        wc_t = consts.tile([128, J * D * CP], bf16, tag="wc")
        dmae.dma_start(wc_t[:, :], wc_d[:, :])
        wu_t = consts.tile([128, J * CA * D], bf16, tag="wu")
        dmae.dma_start(wu_t[:, :], wu_d[:, :])
        ones_t = consts.tile([128, 128], bf16, tag="onesb")
        dmae.dma_start(ones_t[:, :], ones_d[:, :])

        # Persistent per-chunk tiles.
        Xcs = []   # xga [p, (c, s)] bf16
        Xsc = []   # xga [p, (s, c)] bf16
        L = []     # routing logits b, layout [p, (j, s)] bf16
        for ch in range(NCH):
            xt = xpool.tile([128, CA * S], bf16, tag=f"Xcs{ch}",
                            name=f"Xcs{ch}")
            dmae.dma_start(xt[:, :], xcs_d[ch, :, :])
            Xcs.append(xt)
            xt2 = xpool.tile([128, S * CP], bf16, tag=f"Xsc{ch}",
                             name=f"Xsc{ch}")
            dmae.dma_start(xt2[:, :], xsc_d[ch, :, :])
            Xsc.append(xt2)
            L.append(lpool.tile([128, J * S], bf16, tag=f"L{ch}",
                                name=f"L{ch}"))

        # Per-chunk in-flight state handed between stages.
        ST = [dict() for _ in range(NCH)]

        def split_tt(out4, in0, in1, k):
            """Emit a broadcast-product split along the j (outermost free)
            dim: DVE takes j < J-k, Pool takes the last k rows. Both write
            disjoint j-ranges of the same tile."""
            if k < J:
                nc.vector.tensor_tensor(out4[:, 0:J - k], in0[:, 0:J - k],
                                        in1[:, 0:J - k], Alu.mult)
            if k > 0:
                nc.gpsimd.tensor_tensor(out4[:, J - k:J], in0[:, J - k:J],
                                        in1[:, J - k:J], Alu.mult)

        # ---------------- stage functions ----------------

        def st_xsum(ch):
            xsum = small.tile([128, CA], bf16, tag="xsum", name="xsum")
            with nc.allow_low_precision("bf16 routing intermediates"):
                nc.vector.tensor_reduce(
                    xsum[:, :],
                    Xcs[ch][:, :].rearrange("p (c s) -> p c s", c=CA), AxX,
                    Alu.add)
            ST[ch]["xsum"] = xsum

        def st_xs1(ch):
            xs1 = small.tile([128, CA], bf16, tag="xsum1", name="xs1")
            nc.scalar.mul(xs1[:, :], ST[ch]["xsum"][:, :], 1.0 / J)
            ST[ch]["t"] = None
            ST[ch]["t_b"] = (xs1[:, :].unsqueeze(1).unsqueeze(1)
                             .broadcast_to([128, J, D, CA]))

        def st_pc(ch):
            # prodC[p, (j, d, c)] = t_bcast * Wa
            pc = spool.tile([128, J * D * CP], bf16, tag="prodC", name="pc")
            pc4 = (pc[:, :].rearrange("p (j d c) -> p j d c", j=J, d=D)
                   [:, :, :, 0:CA])
            wc4 = (wc_t[:, :].rearrange("p (j d c) -> p j d c", j=J, d=D)
                   [:, :, :, 0:CA])
            split_tt(pc4, ST[ch]["t_b"], wc4, kc)
            ST[ch]["pc"] = pc

        def st_mm(ch):
            # PE contracts g (partitions, via blockdiag ones) AND c (PSUM
            # accumulation over the 9 channel slices).
            pcz = ST[ch]["pc"][:, :].rearrange("p (a c) -> p a c", c=CP)
            ps = psum.tile([128, J * D], f32, tag="psum_s", name="ps")
            for c in range(CA):
                nc.tensor.matmul(ps[:, :], ones_t[:, :], pcz[:, :, c],
                                 start=(c == 0), stop=(c == CA - 1))
            ST[ch]["ps"] = ps

        def st_scpy(ch):
            # s in bf16 for the u-product, plus s^2 for the squash norm.
            ps = ST[ch]["ps"]
            sb = small.tile([128, J * D], bf16, tag="s_b16", name="sb")
            nc.scalar.copy(sb[:, :], ps[:, :])
            s2 = small.tile([128, J * D], f32, tag="s2", name="s2")
            nc.scalar.activation(s2[:, :], ps[:, :], Act.Square)
            ST[ch]["sb"] = sb
            ST[ch]["s2"] = s2

        def st_scpy3(ch):
            ps = ST[ch]["ps"]
            s_sb = small.tile([128, J * D], f32, tag="s_sb", name="s_sb")
            nc.scalar.copy(s_sb[:, :], ps[:, :])
            s2 = small.tile([128, J * D], f32, tag="s2", name="s2")
            nc.scalar.activation(s2[:, :], ps[:, :], Act.Square)
            ST[ch]["s_sb"] = s_sb
            ST[ch]["s2"] = s2

        def st_n2(ch):
            n2 = small.tile([128, J], f32, tag="n2", name="n2")
            nc.vector.tensor_reduce(
                n2[:, :],
                ST[ch]["s2"][:, :].rearrange("p (j d) -> p j d", j=J), AxX,
                Alu.add)
            ST[ch]["n2"] = n2

        def st_act2(ch):
            n2 = ST[ch]["n2"]
            n2p1 = small.tile([128, J], f32, tag="n2p1", name="n2p1")
            nc.scalar.add(n2p1[:, :], n2[:, :], 1.0)
            nr = small.tile([128, J], f32, tag="nrm", name="nr")
            nc.scalar.activation(nr[:, :], n2[:, :], Act.Sqrt)
            ST[ch]["n2p1"] = n2p1
            ST[ch]["nr"] = nr

        def st_rf(ch):
            r = small.tile([128, J], f32, tag="rcp", name="r")
            nc.vector.reciprocal(r[:, :], ST[ch]["n2p1"][:, :])
            f = small.tile([128, J], f32, tag="fac", name="f")
            nc.vector.tensor_tensor(f[:, :], ST[ch]["nr"][:, :], r[:, :],
                                    Alu.mult)
            ST[ch]["f"] = f

        def st_pu(ch):
            # produ[p, (j, c, d)] = s_bcast * Wa  (squash factor applied
            # later, on the d-reduced output)
            pu = spool.tile([128, J * CA * D], bf16, tag="produ", name="pu")
            pu4 = pu[:, :].rearrange("p (j c d) -> p j c d", j=J, c=CA)
            wu4 = wu_t[:, :].rearrange("p (j c d) -> p j c d", j=J, c=CA)
            sbb = (ST[ch]["sb"][:, :].rearrange("p (j d) -> p j d", j=J)
                   .unsqueeze(2).broadcast_to([128, J, CA, D]))
            split_tt(pu4, sbb, wu4, ku)
            ST[ch]["pu"] = pu

        def st_utree(ch):
            puz = ST[ch]["pu"][:, :].rearrange("p (a d) -> p a d", d=D)
            uA = spool.tile([128, 90 * 8], bf16, tag="tr720", name="uA")
            uA3 = uA[:, :].rearrange("p (a c) -> p a c", c=8)
            nc.vector.tensor_tensor(uA3, puz[:, :, 0:8], puz[:, :, 8:16],
                                    Alu.add)
            uB = spool.tile([128, 90 * 4], bf16, tag="tr360", name="uB")
            uB3 = uB[:, :].rearrange("p (a c) -> p a c", c=4)
            nc.vector.tensor_tensor(uB3, uA3[:, :, 0:4], uA3[:, :, 4:8],
                                    Alu.add)
            uC = spool.tile([128, 90 * 2], bf16, tag="tr180c", name="uC")
            uC3 = uC[:, :].rearrange("p (a c) -> p a c", c=2)
            nc.vector.tensor_tensor(uC3, uB3[:, :, 0:2], uB3[:, :, 2:4],
                                    Alu.add)
            ut = small.tile([128, J * CP], bf16, tag="ut", name="ut")
            ut3 = ut[:, :].rearrange("p (j c) -> p j c", j=J)[:, :, 0:CA]
            nc.vector.tensor_tensor(ut3, uC3[:, :, 0], uC3[:, :, 1],
                                    Alu.add)
            ST[ch]["ut"] = ut

        def st_uscale(ch):
            # u = f * (s @ W): fold the squash factor into u.
            u = small.tile([128, J * CP], bf16, tag="u", name="u")
            u3 = u[:, :].rearrange("p (j c) -> p j c", j=J)[:, :, 0:CA]
            ut3 = (ST[ch]["ut"][:, :].rearrange("p (j c) -> p j c", j=J)
                   [:, :, 0:CA])
            fb = (ST[ch]["f"][:, :].unsqueeze(2)
                  .broadcast_to([128, J, CA]))
            nc.vector.tensor_tensor(u3, ut3, fb, Alu.mult)
            ST[ch]["u"] = u

        def st_pe(ch):
            # prodE[p, (j, s, c)] over the 8 real channels only; the ones
            # channel (u[j,8]) is added directly into L afterwards.
            pe = spool.tile([128, J * S * CP], bf16, tag="bigP", name="pe")
            pe4 = (pe[:, :].rearrange("p (j s c) -> p j s c", j=J, s=S)
                   [:, :, :, 0:C_IN])
            ub = (ST[ch]["u"][:, :].rearrange("p (j c) -> p j c", j=J)
                  [:, :, 0:C_IN].unsqueeze(2)
                  .broadcast_to([128, J, S, C_IN]))
            xb = (Xsc[ch][:, :].rearrange("p (s c) -> p s c", s=S)
                  [:, :, 0:C_IN].unsqueeze(1)
                  .broadcast_to([128, J, S, C_IN]))
            split_tt(pe4, ub, xb, ke)
            ST[ch]["pe"] = pe

        def st_etree(ch):
            pez = ST[ch]["pe"][:, :].rearrange("p (a c) -> p a c", c=CP)
            eA = spool.tile([128, 360 * 4], bf16, tag="tr1440", name="eA")
            eA3 = eA[:, :].rearrange("p (a c) -> p a c", c=4)
            nc.vector.tensor_tensor(eA3, pez[:, :, 0:4], pez[:, :, 4:8],
                                    Alu.add)
            eB = spool.tile([128, 360 * 2], bf16, tag="tr720", name="eB")
            eB3 = eB[:, :].rearrange("p (a c) -> p a c", c=2)
            nc.vector.tensor_tensor(eB3, eA3[:, :, 0:2], eA3[:, :, 2:4],
                                    Alu.add)
            eC = spool.tile([128, J * S], bf16, tag="eC", name="eC")
            nc.vector.tensor_tensor(eC[:, :], eB3[:, :, 0], eB3[:, :, 1],
                                    Alu.add)
            ST[ch]["eC"] = eC

        def _u8b(ch):
            return (ST[ch]["u"][:, :].rearrange("p (j c) -> p j c", j=J)
                    [:, :, 8].unsqueeze(2).broadcast_to([128, J, S]))

        def st_lupd1(ch):
            # L = db1 = eC + u[j,8]  (b was zero)
            L3 = L[ch][:, :].rearrange("p (j s) -> p j s", j=J)
            eC3 = ST[ch]["eC"][:, :].rearrange("p (j s) -> p j s", j=J)
            nc.vector.tensor_tensor(L3, eC3, _u8b(ch), Alu.add)

        def st_lupd2(ch):
            L3 = L[ch][:, :].rearrange("p (j s) -> p j s", j=J)
            eC3 = ST[ch]["eC"][:, :].rearrange("p (j s) -> p j s", j=J)
            nc.vector.tensor_tensor(L3, L3, eC3, Alu.add)
            nc.vector.tensor_tensor(L3, L3, _u8b(ch), Alu.add)

        def st_exp(ch):
            et = spool.tile([128, J * S], bf16, tag="expt", name="et")
            nc.scalar.activation(et[:, :], L[ch][:, :], Act.Exp)
            ST[ch]["et"] = et

        def st_zred(ch):
            z = small.tile([128, S], f32, tag="z", name="z")
            ejs = (ST[ch]["et"][:, :].rearrange("p (j s) -> p j s", j=J)
                   .transpose([0, 2, 1]))
            nc.vector.tensor_reduce(z[:, :], ejs, AxX, Alu.add)
            ST[ch]["z"] = z

        def st_zr(ch):
            zr = small.tile([128, S], bf16, tag="zr", name="zr")
            with nc.allow_low_precision("bf16 softmax normalizer"):
                nc.vector.reciprocal(zr[:, :], ST[ch]["z"][:, :])
            ST[ch]["zr"] = zr

        def st_ct(ch):
            ct = spool.tile([128, J * S], bf16, tag="ct", name="ct")
            zb = ST[ch]["zr"][:, :].unsqueeze(1).broadcast_to([128, J, S])
            nc.vector.tensor_tensor(
                ct[:, :].rearrange("p (j s) -> p j s", j=J),
                ST[ch]["et"][:, :].rearrange("p (j s) -> p j s", j=J), zb,
                Alu.mult)
            ST[ch]["ct"] = ct

        def st_pb(ch):
            pb = spool.tile([128, J * CA * S], bf16, tag="bigP", name="pb")
            pb4 = pb[:, :].rearrange("p (j c s) -> p j c s", j=J, c=CA)
            cb = (ST[ch]["ct"][:, :].rearrange("p (j s) -> p j s", j=J)
                  .unsqueeze(2).broadcast_to([128, J, CA, S]))
            xb = (Xcs[ch][:, :].rearrange("p (c s) -> p c s", c=CA)
                  .unsqueeze(1).broadcast_to([128, J, CA, S]))
            split_tt(pb4, cb, xb, kb)
            ST[ch]["pb"] = pb

        def st_btree(ch):
            pbz = ST[ch]["pb"][:, :].rearrange("p (a s) -> p a s", s=S)
            bA = spool.tile([128, 90 * 16], bf16, tag="tr1440", name="bA")
            bA3 = bA[:, :].rearrange("p (a c) -> p a c", c=16)
            nc.vector.tensor_tensor(bA3, pbz[:, :, 0:16], pbz[:, :, 16:32],
                                    Alu.add)
            bB = spool.tile([128, 90 * 8], bf16, tag="tr720", name="bB")
            bB3 = bB[:, :].rearrange("p (a c) -> p a c", c=8)
            nc.vector.tensor_tensor(bB3, bA3[:, :, 0:8], bA3[:, :, 8:16],
                                    Alu.add)
            bC = spool.tile([128, 90 * 4], bf16, tag="tr360", name="bC")
            bC3 = bC[:, :].rearrange("p (a c) -> p a c", c=4)
            nc.vector.tensor_tensor(bC3, bB3[:, :, 0:4], bB3[:, :, 4:8],
                                    Alu.add)
            # tail s=32..35 pairs
            bT = spool.tile([128, 90 * 2], bf16, tag="tr180a", name="bT")
            bT3 = bT[:, :].rearrange("p (a c) -> p a c", c=2)
            nc.vector.tensor_tensor(bT3, pbz[:, :, 32:34], pbz[:, :, 34:36],
                                    Alu.add)
            bD = spool.tile([128, 90 * 2], bf16, tag="tr180b", name="bD")
            bD3 = bD[:, :].rearrange("p (a c) -> p a c", c=2)
            nc.vector.tensor_tensor(bD3, bC3[:, :, 0:2], bC3[:, :, 2:4],
                                    Alu.add)
            bE = spool.tile([128, 90 * 2], bf16, tag="tr180c", name="bE")
            bE3 = bE[:, :].rearrange("p (a c) -> p a c", c=2)
            nc.vector.tensor_tensor(bE3, bD3[:, :, :], bT3[:, :, :],
                                    Alu.add)
            t = small.tile([128, J * CP], bf16, tag="tt", name="t")
            t3 = t[:, :].rearrange("p (j c) -> p j c", j=J)[:, :, 0:CA]
            nc.vector.tensor_tensor(t3, bE3[:, :, 0], bE3[:, :, 1],
                                    Alu.add)
            ST[ch]["t"] = t
            ST[ch]["t_b"] = (t[:, :].rearrange("p (j c) -> p j c", j=J)
                             [:, :, 0:CA].unsqueeze(2)
                             .broadcast_to([128, J, D, CA]))

        def st_vt(ch):
            # final v = s * f, fp32
            vt = vpool.tile([128, J * D], f32, tag="vtf", name="vt")
            fb = (ST[ch]["f"][:, :].unsqueeze(2)
                  .broadcast_to([128, J, D]))
            nc.vector.tensor_tensor(
                vt[:, :].rearrange("p (j d) -> p j d", j=J),
                ST[ch]["s_sb"][:, :].rearrange("p (j d) -> p j d", j=J), fb,
                Alu.mult)
            ST[ch]["vt"] = vt

        def st_out(ch):
            dmae.dma_start(v_d[ch * B4:(ch + 1) * B4, :],
                           ST[ch]["vt"][0:128:NS, :])

        # ---------------- emission: stage-major wavefront ----------------
        # Chunks are processed in groups of `bufs`: within a group, stages
        # are emitted stage-major (so every engine has independent work from
        # the other chunks of the group), and every tile's consumers are
        # emitted before the next group recycles its buffer.

        def emit(stages):
            for g0 in range(0, NCH, bufs):
                for fn in stages:
                    for ch in range(g0, min(g0 + bufs, NCH)):
                        fn(ch)

        c_sq_u = [st_pc, st_mm, st_scpy, st_n2, st_act2, st_rf, st_pu,
                  st_utree, st_uscale, st_pe, st_etree]
        softmax_b = [st_exp, st_zred, st_zr, st_ct, st_pb, st_btree]

        emit([st_xsum, st_xs1] + c_sq_u + [st_lupd1])
        emit(softmax_b + c_sq_u + [st_lupd2])
        emit(softmax_b + [st_pc, st_mm, st_scpy3, st_n2, st_act2, st_rf,
                          st_vt, st_out])

    if split_waits:
        _split_multi_waits(nc)
    return nc


def _split_multi_waits(nc):
    """Walrus's cayman codegen allows exactly ONE sync wait per TPB
    instruction (NEURON_ISA_TPB_EVENTS has a single wait slot). Tile's
    scheduler attaches several waits to dependency-merge instructions,
    which the native bass encoder handles but the neuronx-cc path rejects
    ("Too many sync wait commands"). Split the extras onto engine-local
    NoOp instructions inserted immediately before the owner so the wait
    semantics are unchanged.
    """
    from concourse import mybir

    for bbname, bbwrap in nc.bb_map.items():
        bb = bbwrap.bb
        insts = bb.instructions
        i = 0
        while i < len(insts):
            ins = insts[i]
            si = getattr(ins, "sync_info", None)
            if si is None or len(si.on_wait or []) <= 1:
                i += 1
                continue
            waits = list(si.on_wait)
            engine = ins.engine
            for w in waits[:-1]:
                nop = mybir.InstNoOp(
                    name=nc.get_next_instruction_name(),
                    engine=engine,
                    bass_nofuse=True,
                    sync_info=mybir.SyncInfo(on_wait=[w], on_update=[]),
                )
                insts.insert(i, nop)
                i += 1
            ins.sync_info = mybir.SyncInfo(on_wait=[waits[-1]],
                                           on_update=si.on_update)
            i += 1


def _get_program(split_waits=True, **kw):
    key = ("nc", split_waits, tuple(sorted(kw.items())))
    if key not in _CACHE:
        _CACHE[key] = _build_program(split_waits, **kw)
    return _CACHE[key]


def _host_prep(x, W, bias):
    """Build per-core input maps."""
    bf = np.float16
    x = np.ascontiguousarray(x, dtype=np.float32)
    W = np.ascontiguousarray(W, dtype=np.float32)
    bias = np.ascontiguousarray(bias, dtype=np.float32)
    bs = x.shape[0]

    xga = x.reshape(bs, NS, C_IN, S)
    xa = np.concatenate(
        [xga, np.ones((bs, NS, 1, S), dtype=np.float32)], axis=2)
    # [core, ch, b4, g, c, s]
    x6 = xa.reshape(NCORES, NCH, B4, NS, CA, S)
    xcs = np.ascontiguousarray(x6).reshape(
        NCORES, NCH, 128, CA * S).astype(bf)
    x6sc = x6.transpose(0, 1, 2, 3, 5, 4)      # [.., s, c]
    x6sp = np.concatenate(
        [x6sc, np.zeros(x6sc.shape[:-1] + (CP - CA,), np.float32)], axis=-1)
    xsc = np.ascontiguousarray(x6sp).reshape(
        NCORES, NCH, 128, S * CP).astype(bf)

    Wa = np.concatenate(
        [W.reshape(NS, J, D, C_IN),
         bias.reshape(NS, J, D, 1)], axis=3)            # [g, j, d, c]
    Wap = np.concatenate(
        [Wa, np.zeros(Wa.shape[:-1] + (CP - CA,), np.float32)], axis=-1)
    wc = np.tile(Wap.reshape(NS, J * D * CP), (B4, 1)).astype(bf)
    wu = np.tile(
        Wa.transpose(0, 1, 3, 2).reshape(NS, J * CA * D),
        (B4, 1)).astype(bf)                             # [128, (j,c,d)]
    onesb = np.kron(np.eye(B4, dtype=np.float32),
                    np.ones((NS, NS), dtype=np.float32)).astype(bf)

    in_maps = [
        {"xcs": np.ascontiguousarray(xcs[k]),
         "xsc": np.ascontiguousarray(xsc[k]),
         "wc": wc, "wu": wu, "onesb": onesb}
        for k in range(NCORES)
    ]
    return in_maps


def kernel(x, W, bias, b0):
    from concourse.bass_utils import run_bass_kernel_spmd

    nc = _get_program()
    in_maps = _host_prep(x, W, bias)
    res = run_bass_kernel_spmd(nc, in_maps, list(range(NCORES)))
    out = np.concatenate([res.results[k]["v"] for k in range(NCORES)],
                         axis=0)
    return np.ascontiguousarray(out.reshape(NCORES * BLOC, J, D))


# revision 10
# speedup vs baseline: 1.9241x; 1.1479x over previous
# Trainium2 Bass kernel for nn_CapLayer (CapsNet grouped 1x1 conv + dynamic routing).
#
# Key algebraic restructuring: the huge intermediate pred[b, i=(g,s), (j,d)]
# (188MB for the full batch) is NEVER materialized. Routing is computed in a
# factored form:
#   pred[b,(g,s),(j,d)] = sum_c Wa[g,j,d,c] * xga[b,g,c,s]     (c augmented with
#                                                               a ones channel to
#                                                               absorb the bias)
#   t[b,j,g,c]  = sum_s c[b,j,(g,s)] * xga[b,g,c,s]
#   s[b,j,d]    = sum_{g,c} t[b,j,g,c] * Wa[g,j,d,c]
#   u[b,j,g,c]  = sum_d v[b,j,d] * Wa[g,j,d,c]
#   db[b,j,g,s] = sum_c u[b,j,g,c] * xga[b,g,c,s]
# Iteration 1 collapses (softmax of zeros is uniform): t1 = xsum / J.
# The squash factor f = |s|/(1+|s|^2) is folded into u (u = f * (s @ W))
# instead of materializing v, which takes the scalar squash chain off the
# critical path between the PE matmul and the next product.
#
# Sharding: pure data parallel, 32 samples per core across 8 cores.
# On-chip layout: partition p = (b4, g) with 4 samples x 32 groups = 128
# partitions; 8 chunks cover the 32 local samples.
#
# Engine placement:
#  - The two W-contractions ride the TensorEngine: the g+c contraction for
#    s[b,(j,d)] uses a block-diagonal ones matrix (which also replicates s
#    across the g-partitions for free), and the c-contraction for db uses
#    an identity matrix with PSUM accumulation over the 9 channel slices —
#    this replaces a whole DVE add-tree and absorbs the bias channel.
#  - Every big broadcast-product is split along j between DVE (bf16 2x mode)
#    and GPSIMD/Pool so both engines chew each product concurrently.
#  - The Activation engine does psum evacuation (including writing db
#    directly into the logits), exp, square, sqrt.
#
# Schedule: stages are emitted in a wavefront across chunk groups of `bufs`
# (stage-major order) so each engine always has independent work from other
# chunks while one chunk traverses the cross-engine chain.

import sys

import numpy as np

# concourse (Bass/Tile) ships with the container; make sure it's importable
# when the grader runs kernel.py from a bare directory.
for _p in ("/opt/trn_rl_repo", "/root/.axon_site/_ro/trn_rl_repo"):
    if _p not in sys.path:
        sys.path.insert(0, _p)

NS, J, D, C_IN, H, WID, RN = 32, 10, 16, 8, 6, 6, 3
S = H * WID            # 36 spatial positions
CA = C_IN + 1          # 9 channels including the ones channel
CP = 10                # padded channel stride (4B alignment for bf16 rows)
NCORES = 8
BLOC = 32              # samples per core
B4 = 4                 # samples per chunk
NCH = BLOC // B4       # 8 chunks

_CACHE = {}


def _build_program(split_waits=True, kb=4, ke=4, kc=3, ku=3,
                   bufs=4, dma_eng="sync"):
    from contextlib import ExitStack

    import concourse.bass as bass
    import concourse.tile as tile
    from concourse import mybir

    f32 = mybir.dt.float32
    bf16 = mybir.dt.float16
    Alu = mybir.AluOpType
    Act = mybir.ActivationFunctionType
    AxX = mybir.AxisListType.X

    nc = bass.Bass("TRN2", target_bir_lowering=True, debug=False,
                   num_devices=NCORES)

    xcs_d = nc.dram_tensor("xcs", [128, NCH * CA * S], bf16,
                           kind="ExternalInput").ap()      # free = (ch, c, s)
    xsc_d = nc.dram_tensor("xsc", [128, NCH * S * CP], bf16,
                           kind="ExternalInput").ap()      # free = (ch, s, c10)
    wc_d = nc.dram_tensor("wc", [128, J * D * CP], bf16,
                          kind="ExternalInput").ap()       # free = (j, d, c10)
    wu_d = nc.dram_tensor("wu", [128, J * CA * D], bf16,
                          kind="ExternalInput").ap()       # free = (j, c, d)
    ones_d = nc.dram_tensor("onesb", [128, 128], bf16,
                            kind="ExternalInput").ap()     # blockdiag over b4
    eye_d = nc.dram_tensor("eye", [128, 128], bf16,
                           kind="ExternalInput").ap()      # identity
    v_d = nc.dram_tensor("v", [BLOC, J * D], f32,
                         kind="ExternalOutput").ap()

    dmae = {"gpsimd": nc.gpsimd, "sync": nc.sync}[dma_eng]
    with tile.TileContext(nc) as tc, ExitStack() as ctx:
        consts = ctx.enter_context(tc.tile_pool(name="consts", bufs=1))
        xpool = ctx.enter_context(tc.tile_pool(name="xpool", bufs=1))
        lpool = ctx.enter_context(tc.tile_pool(name="lpool", bufs=1))
        spool = ctx.enter_context(tc.tile_pool(name="scratch", bufs=bufs))
        small = ctx.enter_context(tc.tile_pool(name="small", bufs=bufs))
        vpool = ctx.enter_context(tc.tile_pool(name="vpool", bufs=bufs))
        psum = ctx.enter_context(tc.tile_pool(name="psum", bufs=4,
                                              space="PSUM"))

        # x first (the first compute stage needs it), consts after.
        xall = xpool.tile([128, NCH * CA * S], bf16, tag="xall", name="xall")
        dmae.dma_start(xall[:, :], xcs_d[:, :])
        wc_t = consts.tile([128, J * D * CP], bf16, tag="wc")
        dmae.dma_start(wc_t[:, :], wc_d[:, :])
        ones_t = consts.tile([128, 128], bf16, tag="onesb")
        dmae.dma_start(ones_t[:, :], ones_d[:, :])
        xall2 = xpool.tile([128, NCH * S * CP], bf16, tag="xall2",
                           name="xall2")
        dmae.dma_start(xall2[:, :], xsc_d[:, :])
        wu_t = consts.tile([128, J * CA * D], bf16, tag="wu")
        dmae.dma_start(wu_t[:, :], wu_d[:, :])
        eye_t = consts.tile([128, 128], bf16, tag="eye")
        dmae.dma_start(eye_t[:, :], eye_d[:, :])

        Xcs = [xall[:, ch * CA * S:(ch + 1) * CA * S] for ch in range(NCH)]
        Xsc = [xall2[:, ch * S * CP:(ch + 1) * S * CP] for ch in range(NCH)]
        L = []     # routing logits b, layout [p, (j, s)] bf16
        for ch in range(NCH):
            L.append(lpool.tile([128, J * S], bf16, tag=f"L{ch}",
                                name=f"L{ch}"))

        # Per-chunk in-flight state handed between stages.
        ST = [dict() for _ in range(NCH)]

        def split_tt(out4, in0, in1, k):
            """Emit a broadcast-product split along the j (outermost free)
            dim: DVE takes j < J-k, Pool takes the last k rows. Both write
            disjoint j-ranges of the same tile."""
            if k < J:
                nc.vector.tensor_tensor(out4[:, 0:J - k], in0[:, 0:J - k],
                                        in1[:, 0:J - k], Alu.mult)
            if k > 0:
                nc.gpsimd.tensor_tensor(out4[:, J - k:J], in0[:, J - k:J],
                                        in1[:, J - k:J], Alu.mult)

        # ---------------- stage functions ----------------

        def st_xsum(ch):
            xsum = small.tile([128, CA], bf16, tag="xsum", name="xsum")
            with nc.allow_low_precision("bf16 routing intermediates"):
                nc.vector.tensor_reduce(
                    xsum[:, :],
                    Xcs[ch].rearrange("p (c s) -> p c s", c=CA), AxX,
                    Alu.add)
            ST[ch]["xsum"] = xsum

        def st_xs1(ch):
            xs1 = small.tile([128, CA], bf16, tag="xsum1", name="xs1")
            nc.scalar.mul(xs1[:, :], ST[ch]["xsum"][:, :], 1.0 / J)
            ST[ch]["t_b"] = (xs1[:, :].unsqueeze(1).unsqueeze(1)
                             .broadcast_to([128, J, D, CA]))

        def st_pc(ch):
            # prodC[p, (j, d, c)] = t_bcast * Wa
            pc = spool.tile([128, J * D * CP], bf16, tag="prodC", name="pc")
            pc4 = (pc[:, :].rearrange("p (j d c) -> p j d c", j=J, d=D)
                   [:, :, :, 0:CA])
            wc4 = (wc_t[:, :].rearrange("p (j d c) -> p j d c", j=J, d=D)
                   [:, :, :, 0:CA])
            split_tt(pc4, ST[ch]["t_b"], wc4, kc)
            ST[ch]["pc"] = pc

        def st_mm(ch):
            # PE contracts g (partitions, via blockdiag ones) AND c (PSUM
            # accumulation over the 9 channel slices).
            pcz = ST[ch]["pc"][:, :].rearrange("p (a c) -> p a c", c=CP)
            ps = psum.tile([128, J * D], f32, tag="psum_s", name="ps")
            for c in range(CA):
                nc.tensor.matmul(ps[:, :], ones_t[:, :], pcz[:, :, c],
                                 start=(c == 0), stop=(c == CA - 1))
            ST[ch]["ps"] = ps

        def st_scpy(ch):
            # s in bf16 for the u-product, plus s^2 for the squash norm.
            ps = ST[ch]["ps"]
            sb = small.tile([128, J * D], bf16, tag="s_b16", name="sb")
            nc.scalar.copy(sb[:, :], ps[:, :])
            s2 = small.tile([128, J * D], f32, tag="s2", name="s2")
            nc.scalar.activation(s2[:, :], ps[:, :], Act.Square)
            ST[ch]["sb"] = sb
            ST[ch]["s2"] = s2

        def st_scpy3(ch):
            ps = ST[ch]["ps"]
            s_sb = small.tile([128, J * D], f32, tag="s_sb", name="s_sb")
            nc.scalar.copy(s_sb[:, :], ps[:, :])
            s2 = small.tile([128, J * D], f32, tag="s2", name="s2")
            nc.scalar.activation(s2[:, :], ps[:, :], Act.Square)
            ST[ch]["s_sb"] = s_sb
            ST[ch]["s2"] = s2

        def st_n2(ch):
            n2 = small.tile([128, J], f32, tag="n2", name="n2")
            nc.vector.tensor_reduce(
                n2[:, :],
                ST[ch]["s2"][:, :].rearrange("p (j d) -> p j d", j=J), AxX,
                Alu.add)
            ST[ch]["n2"] = n2

        def st_nr(ch):
            nr = small.tile([128, J], f32, tag="nrm", name="nr")
            nc.scalar.activation(nr[:, :], ST[ch]["n2"][:, :], Act.Sqrt)
            ST[ch]["nr"] = nr

        def st_rf(ch):
            n2p1 = small.tile([128, J], f32, tag="n2p1", name="n2p1")
            nc.vector.tensor_scalar_add(n2p1[:, :], ST[ch]["n2"][:, :], 1.0)
            r = small.tile([128, J], f32, tag="rcp", name="r")
            nc.vector.reciprocal(r[:, :], n2p1[:, :])
            f = small.tile([128, J], f32, tag="fac", name="f")
            nc.vector.tensor_tensor(f[:, :], ST[ch]["nr"][:, :], r[:, :],
                                    Alu.mult)
            ST[ch]["f"] = f

        def st_pu(ch):
            # produ[p, (j, c, d)] = s_bcast * Wa  (squash factor applied
            # later, on the d-reduced output)
            pu = spool.tile([128, J * CA * D], bf16, tag="produ", name="pu")
            pu4 = pu[:, :].rearrange("p (j c d) -> p j c d", j=J, c=CA)
            wu4 = wu_t[:, :].rearrange("p (j c d) -> p j c d", j=J, c=CA)
            sbb = (ST[ch]["sb"][:, :].rearrange("p (j d) -> p j d", j=J)
                   .unsqueeze(2).broadcast_to([128, J, CA, D]))
            split_tt(pu4, sbb, wu4, ku)
            ST[ch]["pu"] = pu

        def st_utree(ch):
            puz = ST[ch]["pu"][:, :].rearrange("p (a d) -> p a d", d=D)
            uA = spool.tile([128, 90 * 8], bf16, tag="tr720", name="uA")
            uA3 = uA[:, :].rearrange("p (a c) -> p a c", c=8)
            nc.vector.tensor_tensor(uA3, puz[:, :, 0:8], puz[:, :, 8:16],
                                    Alu.add)
            uB = spool.tile([128, 90 * 4], bf16, tag="tr360", name="uB")
            uB3 = uB[:, :].rearrange("p (a c) -> p a c", c=4)
            nc.vector.tensor_tensor(uB3, uA3[:, :, 0:4], uA3[:, :, 4:8],
                                    Alu.add)
            uC = spool.tile([128, 90 * 2], bf16, tag="tr180c", name="uC")
            uC3 = uC[:, :].rearrange("p (a c) -> p a c", c=2)
            nc.vector.tensor_tensor(uC3, uB3[:, :, 0:2], uB3[:, :, 2:4],
                                    Alu.add)
            ut = small.tile([128, J * CP], bf16, tag="ut", name="ut")
            ut3 = ut[:, :].rearrange("p (j c) -> p j c", j=J)[:, :, 0:CA]
            nc.vector.tensor_tensor(ut3, uC3[:, :, 0], uC3[:, :, 1],
                                    Alu.add)
            ST[ch]["ut"] = ut

        def st_uscale(ch):
            # u = f * (s @ W): fold the squash factor into u.
            u = small.tile([128, J * CP], bf16, tag="u", name="u")
            u3 = u[:, :].rearrange("p (j c) -> p j c", j=J)[:, :, 0:CA]
            ut3 = (ST[ch]["ut"][:, :].rearrange("p (j c) -> p j c", j=J)
                   [:, :, 0:CA])
            fb = (ST[ch]["f"][:, :].unsqueeze(2)
                  .broadcast_to([128, J, CA]))
            nc.vector.tensor_tensor(u3, ut3, fb, Alu.mult)
            ST[ch]["u"] = u

        def st_pe(ch):
            # prodE[p, (j, s, c)] over all 9 channels (the ones channel
            # carries u[j,8], summed into db by the PE c-contraction).
            pe = spool.tile([128, J * S * CP], bf16, tag="bigP", name="pe")
            pe4 = (pe[:, :].rearrange("p (j s c) -> p j s c", j=J, s=S)
                   [:, :, :, 0:CA])
            ub = (ST[ch]["u"][:, :].rearrange("p (j c) -> p j c", j=J)
                  [:, :, 0:CA].unsqueeze(2)
                  .broadcast_to([128, J, S, CA]))
            xb = (Xsc[ch].rearrange("p (s c) -> p s c", s=S)
                  [:, :, 0:CA].unsqueeze(1)
                  .broadcast_to([128, J, S, CA]))
            split_tt(pe4, ub, xb, ke)
            ST[ch]["pe"] = pe

        def st_emm(ch):
            # db[p, (j, s)] = sum_c prodE: identity matmul with PSUM
            # accumulation over the 9 channel slices (partition-preserving).
            pez = ST[ch]["pe"][:, :].rearrange("p (a c) -> p a c", c=CP)
            pse = psum.tile([128, J * S], f32, tag="psum_e", name="pse")
            for c in range(CA):
                nc.tensor.matmul(pse[:, :], eye_t[:, :], pez[:, :, c],
                                 start=(c == 0), stop=(c == CA - 1))
            ST[ch]["pse"] = pse

        def st_lcp1(ch):
            # iteration 1: L = db1 (b was zero) — straight psum evacuation.
            nc.scalar.copy(L[ch][:, :], ST[ch]["pse"][:, :])

        def st_lcp2(ch):
            db = small.tile([128, J * S], bf16, tag="db", name="db")
            nc.scalar.copy(db[:, :], ST[ch]["pse"][:, :])
            ST[ch]["db"] = db

        def st_ladd(ch):
            nc.vector.tensor_tensor(L[ch][:, :], L[ch][:, :],
                                    ST[ch]["db"][:, :], Alu.add)

        def st_exp(ch):
            et = spool.tile([128, J * S], bf16, tag="expt", name="et")
            nc.scalar.activation(et[:, :], L[ch][:, :], Act.Exp)
            ST[ch]["et"] = et

        def st_zred(ch):
            z = small.tile([128, S], f32, tag="z", name="z")
            ejs = (ST[ch]["et"][:, :].rearrange("p (j s) -> p j s", j=J)
                   .transpose([0, 2, 1]))
            nc.vector.tensor_reduce(z[:, :], ejs, AxX, Alu.add)
            ST[ch]["z"] = z

        def st_zr(ch):
            zr = small.tile([128, S], bf16, tag="zr", name="zr")
            with nc.allow_low_precision("bf16 softmax normalizer"):
                nc.vector.reciprocal(zr[:, :], ST[ch]["z"][:, :])
            ST[ch]["zr"] = zr

        def st_ct(ch):
            ct = spool.tile([128, J * S], bf16, tag="ct", name="ct")
            zb = ST[ch]["zr"][:, :].unsqueeze(1).broadcast_to([128, J, S])
            nc.vector.tensor_tensor(
                ct[:, :].rearrange("p (j s) -> p j s", j=J),
                ST[ch]["et"][:, :].rearrange("p (j s) -> p j s", j=J), zb,
                Alu.mult)
            ST[ch]["ct"] = ct

        def st_pb(ch):
            pb = spool.tile([128, J * CA * S], bf16, tag="bigP", name="pb")
            pb4 = pb[:, :].rearrange("p (j c s) -> p j c s", j=J, c=CA)
            cb = (ST[ch]["ct"][:, :].rearrange("p (j s) -> p j s", j=J)
                  .unsqueeze(2).broadcast_to([128, J, CA, S]))
            xb = (Xcs[ch].rearrange("p (c s) -> p c s", c=CA)
                  .unsqueeze(1).broadcast_to([128, J, CA, S]))
            split_tt(pb4, cb, xb, kb)
            ST[ch]["pb"] = pb

        def st_btree(ch):
            pbz = ST[ch]["pb"][:, :].rearrange("p (a s) -> p a s", s=S)
            bA = spool.tile([128, 90 * 16], bf16, tag="tr1440", name="bA")
            bA3 = bA[:, :].rearrange("p (a c) -> p a c", c=16)
            nc.vector.tensor_tensor(bA3, pbz[:, :, 0:16], pbz[:, :, 16:32],
                                    Alu.add)
            bB = spool.tile([128, 90 * 8], bf16, tag="tr720", name="bB")
            bB3 = bB[:, :].rearrange("p (a c) -> p a c", c=8)
            nc.vector.tensor_tensor(bB3, bA3[:, :, 0:8], bA3[:, :, 8:16],
                                    Alu.add)
            bC = spool.tile([128, 90 * 4], bf16, tag="tr360", name="bC")
            bC3 = bC[:, :].rearrange("p (a c) -> p a c", c=4)
            nc.vector.tensor_tensor(bC3, bB3[:, :, 0:4], bB3[:, :, 4:8],
                                    Alu.add)
            # tail s=32..35 pairs
            bT = spool.tile([128, 90 * 2], bf16, tag="tr180a", name="bT")
            bT3 = bT[:, :].rearrange("p (a c) -> p a c", c=2)
            nc.vector.tensor_tensor(bT3, pbz[:, :, 32:34], pbz[:, :, 34:36],
                                    Alu.add)
            bD = spool.tile([128, 90 * 2], bf16, tag="tr180b", name="bD")
            bD3 = bD[:, :].rearrange("p (a c) -> p a c", c=2)
            nc.vector.tensor_tensor(bD3, bC3[:, :, 0:2], bC3[:, :, 2:4],
                                    Alu.add)
            bE = spool.tile([128, 90 * 2], bf16, tag="tr180c", name="bE")
            bE3 = bE[:, :].rearrange("p (a c) -> p a c", c=2)
            nc.vector.tensor_tensor(bE3, bD3[:, :, :], bT3[:, :, :],
                                    Alu.add)
            t = small.tile([128, J * CP], bf16, tag="tt", name="t")
            t3 = t[:, :].rearrange("p (j c) -> p j c", j=J)[:, :, 0:CA]
            nc.vector.tensor_tensor(t3, bE3[:, :, 0], bE3[:, :, 1],
                                    Alu.add)
            ST[ch]["t_b"] = (t[:, :].rearrange("p (j c) -> p j c", j=J)
                             [:, :, 0:CA].unsqueeze(2)
                             .broadcast_to([128, J, D, CA]))

        def st_vt(ch):
            # final v = s * f, fp32
            vt = vpool.tile([128, J * D], f32, tag="vtf", name="vt")
            fb = (ST[ch]["f"][:, :].unsqueeze(2)
                  .broadcast_to([128, J, D]))
            nc.vector.tensor_tensor(
                vt[:, :].rearrange("p (j d) -> p j d", j=J),
                ST[ch]["s_sb"][:, :].rearrange("p (j d) -> p j d", j=J), fb,
                Alu.mult)
            ST[ch]["vt"] = vt

        def st_out(ch):
            dmae.dma_start(v_d[ch * B4:(ch + 1) * B4, :],
                           ST[ch]["vt"][0:128:NS, :])

        # ---------------- emission: stage-major wavefront ----------------
        # Chunks are processed in groups of `bufs`: within a group, stages
        # are emitted stage-major (so every engine has independent work from
        # the other chunks of the group), and every tile's consumers are
        # emitted before the next group recycles its buffer.

        def emit(stages):
            for g0 in range(0, NCH, bufs):
                for fn in stages:
                    for ch in range(g0, min(g0 + bufs, NCH)):
                        fn(ch)

        c_sq_u = [st_pc, st_mm, st_scpy, st_n2, st_nr, st_rf, st_pu,
                  st_utree, st_uscale, st_pe, st_emm]
        softmax_b = [st_exp, st_zred, st_zr, st_ct, st_pb, st_btree]

        emit([st_xsum, st_xs1] + c_sq_u + [st_lcp1])
        emit(softmax_b + c_sq_u + [st_lcp2, st_ladd])
        emit(softmax_b + [st_pc, st_mm, st_scpy3, st_n2, st_nr, st_rf,
                          st_vt, st_out])

    if split_waits:
        _split_multi_waits(nc)
    return nc


def _split_multi_waits(nc):
    """Walrus's cayman codegen allows exactly ONE sync wait per TPB
    instruction (NEURON_ISA_TPB_EVENTS has a single wait slot). Tile's
    scheduler attaches several waits to dependency-merge instructions,
    which the native bass encoder handles but the neuronx-cc path rejects
    ("Too many sync wait commands"). Split the extras onto engine-local
    NoOp instructions inserted immediately before the owner so the wait
    semantics are unchanged.
    """
    from concourse import mybir

    for bbname, bbwrap in nc.bb_map.items():
        bb = bbwrap.bb
        insts = bb.instructions
        i = 0
        while i < len(insts):
            ins = insts[i]
            si = getattr(ins, "sync_info", None)
            if si is None or len(si.on_wait or []) <= 1:
                i += 1
                continue
            waits = list(si.on_wait)
            engine = ins.engine
            for w in waits[:-1]:
                nop = mybir.InstNoOp(
                    name=nc.get_next_instruction_name(),
                    engine=engine,
                    bass_nofuse=True,
                    sync_info=mybir.SyncInfo(on_wait=[w], on_update=[]),
                )
                insts.insert(i, nop)
                i += 1
            ins.sync_info = mybir.SyncInfo(on_wait=[waits[-1]],
                                           on_update=si.on_update)
            i += 1


def _get_program(split_waits=True, **kw):
    key = ("nc", split_waits, tuple(sorted(kw.items())))
    if key not in _CACHE:
        _CACHE[key] = _build_program(split_waits, **kw)
    return _CACHE[key]


def _host_prep(x, W, bias):
    """Build per-core input maps."""
    bf = np.float16
    x = np.ascontiguousarray(x, dtype=np.float32)
    W = np.ascontiguousarray(W, dtype=np.float32)
    bias = np.ascontiguousarray(bias, dtype=np.float32)
    bs = x.shape[0]

    xga = x.reshape(bs, NS, C_IN, S)
    xa = np.concatenate(
        [xga, np.ones((bs, NS, 1, S), dtype=np.float32)], axis=2)
    # [core, ch, b4, g, c, s] -> partition-major [core, b4, g, ch, c, s]
    x6 = xa.reshape(NCORES, NCH, B4, NS, CA, S)
    x6p = x6.transpose(0, 2, 3, 1, 4, 5)
    xcs = np.ascontiguousarray(x6p).reshape(
        NCORES, 128, NCH * CA * S).astype(bf)
    x6sc = x6p.transpose(0, 1, 2, 3, 5, 4)    # [.., ch, s, c]
    x6sp = np.concatenate(
        [x6sc, np.zeros(x6sc.shape[:-1] + (CP - CA,), np.float32)], axis=-1)
    xsc = np.ascontiguousarray(x6sp).reshape(
        NCORES, 128, NCH * S * CP).astype(bf)

    Wa = np.concatenate(
        [W.reshape(NS, J, D, C_IN),
         bias.reshape(NS, J, D, 1)], axis=3)            # [g, j, d, c]
    Wap = np.concatenate(
        [Wa, np.zeros(Wa.shape[:-1] + (CP - CA,), np.float32)], axis=-1)
    wc = np.tile(Wap.reshape(NS, J * D * CP), (B4, 1)).astype(bf)
    wu = np.tile(
        Wa.transpose(0, 1, 3, 2).reshape(NS, J * CA * D),
        (B4, 1)).astype(bf)                             # [128, (j,c,d)]
    onesb = np.kron(np.eye(B4, dtype=np.float32),
                    np.ones((NS, NS), dtype=np.float32)).astype(bf)
    eye = np.eye(128, dtype=np.float32).astype(bf)

    in_maps = [
        {"xcs": np.ascontiguousarray(xcs[k]),
         "xsc": np.ascontiguousarray(xsc[k]),
         "wc": wc, "wu": wu, "onesb": onesb, "eye": eye}
        for k in range(NCORES)
    ]
    return in_maps


def kernel(x, W, bias, b0):
    from concourse.bass_utils import run_bass_kernel_spmd

    nc = _get_program()
    in_maps = _host_prep(x, W, bias)
    res = run_bass_kernel_spmd(nc, in_maps, list(range(NCORES)))
    out = np.concatenate([res.results[k]["v"] for k in range(NCORES)],
                         axis=0)
    return np.ascontiguousarray(out.reshape(NCORES * BLOC, J, D))


# revision 17
# speedup vs baseline: 2.1564x; 1.1207x over previous
# Trainium2 Bass kernel for nn_CapLayer (CapsNet grouped 1x1 conv + dynamic routing).
#
# Key algebraic restructuring: the huge intermediate pred[b, i=(g,s), (j,d)]
# (188MB for the full batch) is NEVER materialized. Routing is computed in a
# factored form:
#   pred[b,(g,s),(j,d)] = sum_c Wa[g,j,d,c] * xga[b,g,c,s]     (c augmented with
#                                                               a ones channel to
#                                                               absorb the bias)
#   t[b,j,g,c]  = sum_s c[b,j,(g,s)] * xga[b,g,c,s]
#   s[b,j,d]    = sum_{g,c} t[b,j,g,c] * Wa[g,j,d,c]
#   u[b,j,g,c]  = sum_d v[b,j,d] * Wa[g,j,d,c]
#   db[b,j,g,s] = sum_c u[b,j,g,c] * xga[b,g,c,s]
# Iteration 1 collapses (softmax of zeros is uniform): t1 = xsum / J.
# The squash factor f = |s|/(1+|s|^2) is folded into u (u = f * (s @ W))
# instead of materializing v, which takes the scalar squash chain off the
# critical path between the PE matmul and the next product.
#
# Sharding: pure data parallel, 32 samples per core across 8 cores.
# On-chip layout: partition p = (b4, g) with 4 samples x 32 groups = 128
# partitions; 8 chunks cover the 32 local samples.
#
# Engine placement:
#  - The two W-contractions ride the TensorEngine: the g+c contraction for
#    s[b,(j,d)] uses a block-diagonal ones matrix (which also replicates s
#    across the g-partitions for free), and the c-contraction for db uses
#    an identity matrix with PSUM accumulation over the 9 channel slices —
#    this replaces a whole DVE add-tree and absorbs the bias channel.
#  - Every big broadcast-product is split along j between DVE (bf16 2x mode)
#    and GPSIMD/Pool so both engines chew each product concurrently.
#  - The Activation engine does psum evacuation (including writing db
#    directly into the logits), exp, square, sqrt.
#
# Schedule: stages are emitted in a wavefront across chunk groups of `bufs`
# (stage-major order) so each engine always has independent work from other
# chunks while one chunk traverses the cross-engine chain.

import sys

import numpy as np

# concourse (Bass/Tile) ships with the container; make sure it's importable
# when the grader runs kernel.py from a bare directory.
for _p in ("/opt/trn_rl_repo", "/root/.axon_site/_ro/trn_rl_repo"):
    if _p not in sys.path:
        sys.path.insert(0, _p)

NS, J, D, C_IN, H, WID, RN = 32, 10, 16, 8, 6, 6, 3
S = H * WID            # 36 spatial positions
CA = C_IN + 1          # 9 channels including the ones channel
CP = 10                # padded channel stride (4B alignment for bf16 rows)
NCORES = 8
BLOC = 32              # samples per core
B4 = 4                 # samples per chunk
NCH = BLOC // B4       # 8 chunks

_CACHE = {}


def _build_program(split_waits=True, kb=(4, 4), ke=(3, 3), kc=(3, 4, 6),
                   ku=(4, 4), bufs=6, grp=4, dma_eng="sync"):
    kb = (kb, kb) if isinstance(kb, int) else kb
    ke = (ke, ke) if isinstance(ke, int) else ke
    kc = (kc, kc, kc) if isinstance(kc, int) else kc
    ku = (ku, ku) if isinstance(ku, int) else ku
    from contextlib import ExitStack

    import concourse.bass as bass
    import concourse.tile as tile
    from concourse import mybir

    f32 = mybir.dt.float32
    bf16 = mybir.dt.float16
    Alu = mybir.AluOpType
    Act = mybir.ActivationFunctionType
    AxX = mybir.AxisListType.X

    nc = bass.Bass("TRN2", target_bir_lowering=True, debug=False,
                   num_devices=NCORES)

    xcs_d = nc.dram_tensor("xcs", [128, NCH * CA * S], bf16,
                           kind="ExternalInput").ap()      # free = (ch, c, s)
    xsc_d = nc.dram_tensor("xsc", [128, NCH * S * CP], bf16,
                           kind="ExternalInput").ap()      # free = (ch, s, c10)
    wc_d = nc.dram_tensor("wc", [128, J * D * CP], bf16,
                          kind="ExternalInput").ap()       # free = (j, d, c10)
    wu_d = nc.dram_tensor("wu", [128, J * CA * D], bf16,
                          kind="ExternalInput").ap()       # free = (j, c, d)
    ones_d = nc.dram_tensor("onesb", [128, 128], bf16,
                            kind="ExternalInput").ap()     # blockdiag over b4
    eye_d = nc.dram_tensor("eye", [128, 128], bf16,
                           kind="ExternalInput").ap()      # identity
    v_d = nc.dram_tensor("v", [BLOC, J * D], f32,
                         kind="ExternalOutput").ap()

    dmae = {"gpsimd": nc.gpsimd, "sync": nc.sync}[dma_eng]
    with tile.TileContext(nc) as tc, ExitStack() as ctx:
        consts = ctx.enter_context(tc.tile_pool(name="consts", bufs=1))
        xpool = ctx.enter_context(tc.tile_pool(name="xpool", bufs=1))
        lpool = ctx.enter_context(tc.tile_pool(name="lpool", bufs=1))
        spool = ctx.enter_context(tc.tile_pool(name="scratch", bufs=bufs))
        small = ctx.enter_context(tc.tile_pool(name="small", bufs=bufs))
        vpool = ctx.enter_context(tc.tile_pool(name="vpool", bufs=bufs))
        psum = ctx.enter_context(tc.tile_pool(name="psum", bufs=4,
                                              space="PSUM"))

        # x first (the first compute stage needs it), consts after.
        xall = xpool.tile([128, NCH * CA * S], bf16, tag="xall", name="xall")
        CSZ = CA * S
        dmae.dma_start(xall[:, 0:CSZ], xcs_d[:, 0:CSZ])
        HX = 4 * CSZ
        dmae.dma_start(xall[:, CSZ:HX], xcs_d[:, CSZ:HX])
        wc_t = consts.tile([128, J * D * CP], bf16, tag="wc")
        dmae.dma_start(wc_t[:, :], wc_d[:, :])
        ones_t = consts.tile([128, 128], bf16, tag="onesb")
        dmae.dma_start(ones_t[:, :], ones_d[:, :])
        dmae.dma_start(xall[:, HX:], xcs_d[:, HX:])
        xall2 = xpool.tile([128, NCH * S * CP], bf16, tag="xall2",
                           name="xall2")
        dmae.dma_start(xall2[:, :], xsc_d[:, :])
        wu_t = consts.tile([128, J * CA * D], bf16, tag="wu")
        dmae.dma_start(wu_t[:, :], wu_d[:, :])
        eye_t = consts.tile([128, 128], bf16, tag="eye")
        dmae.dma_start(eye_t[:, :], eye_d[:, :])

        Xcs = [xall[:, ch * CA * S:(ch + 1) * CA * S] for ch in range(NCH)]
        Xsc = [xall2[:, ch * S * CP:(ch + 1) * S * CP] for ch in range(NCH)]
        L = []     # routing logits b, layout [p, (j, s)] bf16
        for ch in range(NCH):
            L.append(lpool.tile([128, J * S], bf16, tag=f"L{ch}",
                                name=f"L{ch}"))

        # Per-chunk in-flight state handed between stages.
        ST = [dict() for _ in range(NCH)]
        IT = [0]   # current iteration index (0-based), set at emission

        def split_tt(out4, in0, in1, k):
            """Emit a broadcast-product split along the j (outermost free)
            dim: DVE takes j < J-k, Pool takes the last k rows. Both write
            disjoint j-ranges of the same tile."""
            if k < J:
                nc.vector.tensor_tensor(out4[:, 0:J - k], in0[:, 0:J - k],
                                        in1[:, 0:J - k], Alu.mult)
            if k > 0:
                nc.gpsimd.tensor_tensor(out4[:, J - k:J], in0[:, J - k:J],
                                        in1[:, J - k:J], Alu.mult)

        # ---------------- stage functions ----------------

        def st_xsum(ch):
            xsum = small.tile([128, CA], bf16, tag="xsum", name="xsum")
            with nc.allow_low_precision("bf16 routing intermediates"):
                nc.vector.tensor_reduce(
                    xsum[:, :],
                    Xcs[ch].rearrange("p (c s) -> p c s", c=CA), AxX,
                    Alu.add)
            ST[ch]["xsum"] = xsum

        def st_xs1(ch):
            xs1 = small.tile([128, CA], bf16, tag="xsum1", name="xs1")
            with nc.allow_low_precision("bf16 routing intermediates"):
                nc.vector.tensor_scalar_mul(xs1[:, :], ST[ch]["xsum"][:, :],
                                            1.0 / J)
            ST[ch]["t_b"] = (xs1[:, :].unsqueeze(1).unsqueeze(1)
                             .broadcast_to([128, J, D, CA]))

        def st_pc(ch):
            # prodC[p, (j, d, c)] = t_bcast * Wa
            pc = spool.tile([128, J * D * CP], bf16, tag="prodC", name="pc")
            pc4 = (pc[:, :].rearrange("p (j d c) -> p j d c", j=J, d=D)
                   [:, :, :, 0:CA])
            wc4 = (wc_t[:, :].rearrange("p (j d c) -> p j d c", j=J, d=D)
                   [:, :, :, 0:CA])
            split_tt(pc4, ST[ch]["t_b"], wc4, kc[IT[0]])
            ST[ch]["pc"] = pc

        def st_mm(ch):
            # PE contracts g (partitions, via blockdiag ones) AND c (PSUM
            # accumulation over the 9 channel slices).
            pcz = ST[ch]["pc"][:, :].rearrange("p (a c) -> p a c", c=CP)
            ps = psum.tile([128, J * D], f32, tag="psum_s", name="ps")
            for c in range(CA):
                nc.tensor.matmul(ps[:, :], ones_t[:, :], pcz[:, :, c],
                                 start=(c == 0), stop=(c == CA - 1))
            ST[ch]["ps"] = ps

        def st_scpy(ch):
            # s in bf16 for the u-product, plus s^2 for the squash norm.
            ps = ST[ch]["ps"]
            sb = small.tile([128, J * D], bf16, tag="s_b16", name="sb")
            nc.scalar.copy(sb[:, :], ps[:, :])
            s2 = small.tile([128, J * D], f32, tag="s2", name="s2")
            nc.scalar.activation(s2[:, :], ps[:, :], Act.Square)
            ST[ch]["sb"] = sb
            ST[ch]["s2"] = s2

        def st_scpy3(ch):
            ps = ST[ch]["ps"]
            s_sb = small.tile([128, J * D], f32, tag="s_sb", name="s_sb")
            nc.scalar.copy(s_sb[:, :], ps[:, :])
            s2 = small.tile([128, J * D], f32, tag="s2", name="s2")
            nc.scalar.activation(s2[:, :], ps[:, :], Act.Square)
            ST[ch]["s_sb"] = s_sb
            ST[ch]["s2"] = s2

        def st_n2(ch):
            n2 = small.tile([128, J], f32, tag="n2", name="n2")
            nc.vector.tensor_reduce(
                n2[:, :],
                ST[ch]["s2"][:, :].rearrange("p (j d) -> p j d", j=J), AxX,
                Alu.add)
            ST[ch]["n2"] = n2

        def st_nr(ch):
            nr = small.tile([128, J], f32, tag="nrm", name="nr")
            nc.scalar.activation(nr[:, :], ST[ch]["n2"][:, :], Act.Sqrt)
            ST[ch]["nr"] = nr

        def st_rf(ch):
            n2p1 = small.tile([128, J], f32, tag="n2p1", name="n2p1")
            nc.vector.tensor_scalar_add(n2p1[:, :], ST[ch]["n2"][:, :], 1.0)
            r = small.tile([128, J], f32, tag="rcp", name="r")
            nc.vector.reciprocal(r[:, :], n2p1[:, :])
            f = small.tile([128, J], f32, tag="fac", name="f")
            nc.vector.tensor_tensor(f[:, :], ST[ch]["nr"][:, :], r[:, :],
                                    Alu.mult)
            ST[ch]["f"] = f

        def st_pu(ch):
            # produ[p, (j, c, d)] = s_bcast * Wa  (squash factor applied
            # later, on the d-reduced output)
            pu = spool.tile([128, J * CA * D], bf16, tag="produ", name="pu")
            pu4 = pu[:, :].rearrange("p (j c d) -> p j c d", j=J, c=CA)
            wu4 = wu_t[:, :].rearrange("p (j c d) -> p j c d", j=J, c=CA)
            sbb = (ST[ch]["sb"][:, :].rearrange("p (j d) -> p j d", j=J)
                   .unsqueeze(2).broadcast_to([128, J, CA, D]))
            split_tt(pu4, sbb, wu4, ku[IT[0]])
            ST[ch]["pu"] = pu

        def st_utree(ch):
            puz = ST[ch]["pu"][:, :].rearrange("p (a d) -> p a d", d=D)
            uA = spool.tile([128, 90 * 8], bf16, tag="tr720", name="uA")
            uA3 = uA[:, :].rearrange("p (a c) -> p a c", c=8)
            # first level split at the product's j engine boundary so the
            # DVE-produced rows reduce without waiting for the Pool half
            mu = (J - ku[IT[0]]) * CA
            nc.vector.tensor_tensor(uA3[:, 0:mu], puz[:, 0:mu, 0:8],
                                    puz[:, 0:mu, 8:16], Alu.add)
            if mu < J * CA:
                nc.vector.tensor_tensor(uA3[:, mu:], puz[:, mu:, 0:8],
                                        puz[:, mu:, 8:16], Alu.add)
            uB = spool.tile([128, 90 * 4], bf16, tag="tr360", name="uB")
            uB3 = uB[:, :].rearrange("p (a c) -> p a c", c=4)
            nc.vector.tensor_tensor(uB3, uA3[:, :, 0:4], uA3[:, :, 4:8],
                                    Alu.add)
            uC = spool.tile([128, 90 * 2], bf16, tag="tr180c", name="uC")
            uC3 = uC[:, :].rearrange("p (a c) -> p a c", c=2)
            nc.vector.tensor_tensor(uC3, uB3[:, :, 0:2], uB3[:, :, 2:4],
                                    Alu.add)
            ut = small.tile([128, J * CP], bf16, tag="ut", name="ut")
            ut3 = ut[:, :].rearrange("p (j c) -> p j c", j=J)[:, :, 0:CA]
            nc.vector.tensor_tensor(ut3, uC3[:, :, 0], uC3[:, :, 1],
                                    Alu.add)
            ST[ch]["ut"] = ut

        def st_uscale(ch):
            # u = f * (s @ W): fold the squash factor into u.
            u = small.tile([128, J * CP], bf16, tag="u", name="u")
            u3 = u[:, :].rearrange("p (j c) -> p j c", j=J)[:, :, 0:CA]
            ut3 = (ST[ch]["ut"][:, :].rearrange("p (j c) -> p j c", j=J)
                   [:, :, 0:CA])
            fb = (ST[ch]["f"][:, :].unsqueeze(2)
                  .broadcast_to([128, J, CA]))
            nc.vector.tensor_tensor(u3, ut3, fb, Alu.mult)
            ST[ch]["u"] = u

        def st_pe(ch):
            # prodE[p, (j, s, c)] over all 9 channels (the ones channel
            # carries u[j,8], summed into db by the PE c-contraction).
            pe = spool.tile([128, J * S * CP], bf16, tag="bigP", name="pe")
            pe4 = (pe[:, :].rearrange("p (j s c) -> p j s c", j=J, s=S)
                   [:, :, :, 0:CA])
            ub = (ST[ch]["u"][:, :].rearrange("p (j c) -> p j c", j=J)
                  [:, :, 0:CA].unsqueeze(2)
                  .broadcast_to([128, J, S, CA]))
            xb = (Xsc[ch].rearrange("p (s c) -> p s c", s=S)
                  [:, :, 0:CA].unsqueeze(1)
                  .broadcast_to([128, J, S, CA]))
            split_tt(pe4, ub, xb, ke[IT[0]])
            ST[ch]["pe"] = pe

        def st_emm(ch):
            # db[p, (j, s)] = sum_c prodE: identity matmul with PSUM
            # accumulation over the 9 channel slices (partition-preserving).
            pez = ST[ch]["pe"][:, :].rearrange("p (a c) -> p a c", c=CP)
            pse = psum.tile([128, J * S], f32, tag="psum_e", name="pse")
            for c in range(CA):
                nc.tensor.matmul(pse[:, :], eye_t[:, :], pez[:, :, c],
                                 start=(c == 0), stop=(c == CA - 1))
            ST[ch]["pse"] = pse

        def st_lcp1(ch):
            # iteration 1: L = db1 (b was zero) — straight psum evacuation.
            nc.scalar.copy(L[ch][:, :], ST[ch]["pse"][:, :])

        def st_lcp2(ch):
            db = small.tile([128, J * S], bf16, tag="db", name="db")
            nc.scalar.copy(db[:, :], ST[ch]["pse"][:, :])
            ST[ch]["db"] = db

        def st_ladd(ch):
            nc.vector.tensor_tensor(L[ch][:, :], L[ch][:, :],
                                    ST[ch]["db"][:, :], Alu.add)

        def st_exp(ch):
            et = spool.tile([128, J * S], bf16, tag="expt", name="et")
            nc.scalar.activation(et[:, :], L[ch][:, :], Act.Exp)
            ST[ch]["et"] = et

        def st_zred(ch):
            z = small.tile([128, S], f32, tag="z", name="z")
            ejs = (ST[ch]["et"][:, :].rearrange("p (j s) -> p j s", j=J)
                   .transpose([0, 2, 1]))
            nc.vector.tensor_reduce(z[:, :], ejs, AxX, Alu.add)
            ST[ch]["z"] = z

        def st_zr(ch):
            zr = small.tile([128, S], bf16, tag="zr", name="zr")
            with nc.allow_low_precision("bf16 softmax normalizer"):
                nc.vector.reciprocal(zr[:, :], ST[ch]["z"][:, :])
            ST[ch]["zr"] = zr

        def st_ct(ch):
            ct = spool.tile([128, J * S], bf16, tag="ct", name="ct")
            zb = ST[ch]["zr"][:, :].unsqueeze(1).broadcast_to([128, J, S])
            nc.vector.tensor_tensor(
                ct[:, :].rearrange("p (j s) -> p j s", j=J),
                ST[ch]["et"][:, :].rearrange("p (j s) -> p j s", j=J), zb,
                Alu.mult)
            ST[ch]["ct"] = ct

        def st_pb(ch):
            pb = spool.tile([128, J * CA * S], bf16, tag="bigP", name="pb")
            pb4 = pb[:, :].rearrange("p (j c s) -> p j c s", j=J, c=CA)
            cb = (ST[ch]["ct"][:, :].rearrange("p (j s) -> p j s", j=J)
                  .unsqueeze(2).broadcast_to([128, J, CA, S]))
            xb = (Xcs[ch].rearrange("p (c s) -> p c s", c=CA)
                  .unsqueeze(1).broadcast_to([128, J, CA, S]))
            split_tt(pb4, cb, xb, kb[IT[0] - 1])
            ST[ch]["pb"] = pb

        def st_btree(ch):
            pbz = ST[ch]["pb"][:, :].rearrange("p (a s) -> p a s", s=S)
            bA = spool.tile([128, 90 * 16], bf16, tag="tr1440", name="bA")
            bA3 = bA[:, :].rearrange("p (a c) -> p a c", c=16)
            mb = (J - kb[IT[0] - 1]) * CA
            nc.vector.tensor_tensor(bA3[:, 0:mb], pbz[:, 0:mb, 0:16],
                                    pbz[:, 0:mb, 16:32], Alu.add)
            if mb < J * CA:
                nc.vector.tensor_tensor(bA3[:, mb:], pbz[:, mb:, 0:16],
                                        pbz[:, mb:, 16:32], Alu.add)
            bB = spool.tile([128, 90 * 8], bf16, tag="tr720", name="bB")
            bB3 = bB[:, :].rearrange("p (a c) -> p a c", c=8)
            nc.vector.tensor_tensor(bB3, bA3[:, :, 0:8], bA3[:, :, 8:16],
                                    Alu.add)
            # tail s=32..35 pairs
            bT = spool.tile([128, 90 * 2], bf16, tag="tr180a", name="bT")
            bT3 = bT[:, :].rearrange("p (a c) -> p a c", c=2)
            nc.vector.tensor_tensor(bT3, pbz[:, :, 32:34], pbz[:, :, 34:36],
                                    Alu.add)
            ST[ch]["bB3"] = bB3
            ST[ch]["bT3"] = bT3

        def st_tmm(ch):
            # remaining s-reduction (8 bB slices + 2 tail slices) on PE via
            # identity-matmul PSUM accumulation.
            pst = psum.tile([128, J * CA], f32, tag="psum_e", name="pst")
            bB3, bT3 = ST[ch]["bB3"], ST[ch]["bT3"]
            for k in range(8):
                nc.tensor.matmul(pst[:, :], eye_t[:, :], bB3[:, :, k],
                                 start=(k == 0), stop=False)
            nc.tensor.matmul(pst[:, :], eye_t[:, :], bT3[:, :, 0],
                             start=False, stop=False)
            nc.tensor.matmul(pst[:, :], eye_t[:, :], bT3[:, :, 1],
                             start=False, stop=True)
            ST[ch]["pst"] = pst

        def st_tcp(ch):
            t = small.tile([128, J * CP], bf16, tag="tt", name="t")
            t3 = t[:, :].rearrange("p (j c) -> p j c", j=J)[:, :, 0:CA]
            nc.scalar.copy(
                t3, ST[ch]["pst"][:, :].rearrange("p (j c) -> p j c", j=J))
            ST[ch]["t_b"] = (t[:, :].rearrange("p (j c) -> p j c", j=J)
                             [:, :, 0:CA].unsqueeze(2)
                             .broadcast_to([128, J, D, CA]))

        def st_vt(ch):
            # final v = s * f, fp32
            vt = vpool.tile([128, J * D], f32, tag="vtf", name="vt")
            fb = (ST[ch]["f"][:, :].unsqueeze(2)
                  .broadcast_to([128, J, D]))
            nc.vector.tensor_tensor(
                vt[:, :].rearrange("p (j d) -> p j d", j=J),
                ST[ch]["s_sb"][:, :].rearrange("p (j d) -> p j d", j=J), fb,
                Alu.mult)
            ST[ch]["vt"] = vt

        def st_out(ch):
            dmae.dma_start(v_d[ch * B4:(ch + 1) * B4, :],
                           ST[ch]["vt"][0:128:NS, :])

        # ---------------- emission: stage-major wavefront ----------------
        # Chunks are processed in groups of `bufs`: within a group, stages
        # are emitted stage-major (so every engine has independent work from
        # the other chunks of the group), and every tile's consumers are
        # emitted before the next group recycles its buffer.

        def emit(stages):
            for g0 in range(0, NCH, grp):
                for fn in stages:
                    for ch in range(g0, min(g0 + grp, NCH)):
                        fn(ch)

        c_sq_u = [st_pc, st_mm, st_scpy, st_n2, st_nr, st_rf, st_pu,
                  st_utree, st_uscale, st_pe, st_emm]
        softmax_b = [st_exp, st_zred, st_zr, st_ct, st_pb, st_btree, st_tmm,
                     st_tcp]

        IT[0] = 0
        emit([st_xsum, st_xs1] + c_sq_u)
        IT[0] = 1
        emit([st_lcp1] + softmax_b + c_sq_u)
        IT[0] = 2
        emit([st_lcp2, st_ladd] + softmax_b +
             [st_pc, st_mm, st_scpy3, st_n2, st_nr, st_rf, st_vt, st_out])

    if split_waits:
        _split_multi_waits(nc)
    return nc


def _split_multi_waits(nc):
    """Walrus's cayman codegen allows exactly ONE sync wait per TPB
    instruction (NEURON_ISA_TPB_EVENTS has a single wait slot). Tile's
    scheduler attaches several waits to dependency-merge instructions,
    which the native bass encoder handles but the neuronx-cc path rejects
    ("Too many sync wait commands"). Split the extras onto engine-local
    NoOp instructions inserted immediately before the owner so the wait
    semantics are unchanged.
    """
    from concourse import mybir

    for bbname, bbwrap in nc.bb_map.items():
        bb = bbwrap.bb
        insts = bb.instructions
        i = 0
        while i < len(insts):
            ins = insts[i]
            si = getattr(ins, "sync_info", None)
            if si is None or len(si.on_wait or []) <= 1:
                i += 1
                continue
            waits = list(si.on_wait)
            engine = ins.engine
            for w in waits[:-1]:
                nop = mybir.InstNoOp(
                    name=nc.get_next_instruction_name(),
                    engine=engine,
                    bass_nofuse=True,
                    sync_info=mybir.SyncInfo(on_wait=[w], on_update=[]),
                )
                insts.insert(i, nop)
                i += 1
            ins.sync_info = mybir.SyncInfo(on_wait=[waits[-1]],
                                           on_update=si.on_update)
            i += 1


def _get_program(split_waits=True, **kw):
    key = ("nc", split_waits, tuple(sorted(kw.items())))
    if key not in _CACHE:
        _CACHE[key] = _build_program(split_waits, **kw)
    return _CACHE[key]


def _host_prep(x, W, bias):
    """Build per-core input maps."""
    bf = np.float16
    x = np.ascontiguousarray(x, dtype=np.float32)
    W = np.ascontiguousarray(W, dtype=np.float32)
    bias = np.ascontiguousarray(bias, dtype=np.float32)
    bs = x.shape[0]

    xga = x.reshape(bs, NS, C_IN, S)
    xa = np.concatenate(
        [xga, np.ones((bs, NS, 1, S), dtype=np.float32)], axis=2)
    # [core, ch, b4, g, c, s] -> partition-major [core, b4, g, ch, c, s]
    x6 = xa.reshape(NCORES, NCH, B4, NS, CA, S)
    x6p = x6.transpose(0, 2, 3, 1, 4, 5)
    xcs = np.ascontiguousarray(x6p).reshape(
        NCORES, 128, NCH * CA * S).astype(bf)
    x6sc = x6p.transpose(0, 1, 2, 3, 5, 4)    # [.., ch, s, c]
    x6sp = np.concatenate(
        [x6sc, np.zeros(x6sc.shape[:-1] + (CP - CA,), np.float32)], axis=-1)
    xsc = np.ascontiguousarray(x6sp).reshape(
        NCORES, 128, NCH * S * CP).astype(bf)

    Wa = np.concatenate(
        [W.reshape(NS, J, D, C_IN),
         bias.reshape(NS, J, D, 1)], axis=3)            # [g, j, d, c]
    Wap = np.concatenate(
        [Wa, np.zeros(Wa.shape[:-1] + (CP - CA,), np.float32)], axis=-1)
    wc = np.tile(Wap.reshape(NS, J * D * CP), (B4, 1)).astype(bf)
    wu = np.tile(
        Wa.transpose(0, 1, 3, 2).reshape(NS, J * CA * D),
        (B4, 1)).astype(bf)                             # [128, (j,c,d)]
    onesb = np.kron(np.eye(B4, dtype=np.float32),
                    np.ones((NS, NS), dtype=np.float32)).astype(bf)
    eye = np.eye(128, dtype=np.float32).astype(bf)

    in_maps = [
        {"xcs": np.ascontiguousarray(xcs[k]),
         "xsc": np.ascontiguousarray(xsc[k]),
         "wc": wc, "wu": wu, "onesb": onesb, "eye": eye}
        for k in range(NCORES)
    ]
    return in_maps


def kernel(x, W, bias, b0):
    from concourse.bass_utils import run_bass_kernel_spmd

    nc = _get_program()
    in_maps = _host_prep(x, W, bias)
    res = run_bass_kernel_spmd(nc, in_maps, list(range(NCORES)))
    out = np.concatenate([res.results[k]["v"] for k in range(NCORES)],
                         axis=0)
    return np.ascontiguousarray(out.reshape(NCORES * BLOC, J, D))


# revision 20
# speedup vs baseline: 2.4105x; 1.1178x over previous
# Trainium2 Bass kernel for nn_CapLayer (CapsNet grouped 1x1 conv + dynamic routing).
#
# Key algebraic restructuring: the huge intermediate pred[b, i=(g,s), (j,d)]
# (188MB for the full batch) is NEVER materialized. Routing is computed in a
# factored form:
#   pred[b,(g,s),(j,d)] = sum_c Wa[g,j,d,c] * xga[b,g,c,s]     (c augmented with
#                                                               a ones channel to
#                                                               absorb the bias)
#   t[b,j,g,c]  = sum_s c[b,j,(g,s)] * xga[b,g,c,s]
#   s[b,j,d]    = sum_{g,c} t[b,j,g,c] * Wa[g,j,d,c]
#   u[b,j,g,c]  = sum_d v[b,j,d] * Wa[g,j,d,c]
#   db[b,j,g,s] = sum_c u[b,j,g,c] * xga[b,g,c,s]
# Iteration 1 collapses (softmax of zeros is uniform): t1 = xsum / J.
# The squash factor f = |s|/(1+|s|^2) is folded into u (u = f * (s @ W))
# instead of materializing v, which takes the scalar squash chain off the
# critical path between the PE matmul and the next product.
#
# Sharding: pure data parallel, 32 samples per core across 8 cores.
# On-chip layout: partition p = (b4, g) with 4 samples x 32 groups = 128
# partitions; 8 chunks cover the 32 local samples.
#
# Engine placement:
#  - The two W-contractions ride the TensorEngine: the g+c contraction for
#    s[b,(j,d)] uses a block-diagonal ones matrix (which also replicates s
#    across the g-partitions for free), and the c-contraction for db uses
#    an identity matrix with PSUM accumulation over the 9 channel slices —
#    this replaces a whole DVE add-tree and absorbs the bias channel.
#  - Every big broadcast-product is split along j between DVE (bf16 2x mode)
#    and GPSIMD/Pool so both engines chew each product concurrently.
#  - The Activation engine does psum evacuation (including writing db
#    directly into the logits), exp, square, sqrt.
#
# Schedule: stages are emitted in a wavefront across chunk groups of `bufs`
# (stage-major order) so each engine always has independent work from other
# chunks while one chunk traverses the cross-engine chain.

import sys

import numpy as np

# concourse (Bass/Tile) ships with the container; make sure it's importable
# when the grader runs kernel.py from a bare directory.
for _p in ("/opt/trn_rl_repo", "/root/.axon_site/_ro/trn_rl_repo"):
    if _p not in sys.path:
        sys.path.insert(0, _p)

NS, J, D, C_IN, H, WID, RN = 32, 10, 16, 8, 6, 6, 3
S = H * WID            # 36 spatial positions
CA = C_IN + 1          # 9 channels including the ones channel
CP = 10                # padded channel stride (4B alignment for bf16 rows)
NCORES = 8
BLOC = 32              # samples per core
B4 = 4                 # samples per chunk
NCH = BLOC // B4       # 8 chunks

_CACHE = {}


def _build_program(split_waits=True, kb=(3, 3), ke=(2, 3), kc=(3, 4, 4),
                   ku=(4, 4), bufs=6, grp=4, dma_eng="sync"):
    kb = (kb, kb) if isinstance(kb, int) else kb
    ke = (ke, ke) if isinstance(ke, int) else ke
    kc = (kc, kc, kc) if isinstance(kc, int) else kc
    ku = (ku, ku) if isinstance(ku, int) else ku
    from contextlib import ExitStack

    import concourse.bass as bass
    import concourse.tile as tile
    from concourse import mybir

    f32 = mybir.dt.float32
    bf16 = mybir.dt.float16
    Alu = mybir.AluOpType
    Act = mybir.ActivationFunctionType
    AxX = mybir.AxisListType.X

    nc = bass.Bass("TRN2", target_bir_lowering=True, debug=False,
                   num_devices=NCORES)

    xcs_d = nc.dram_tensor("xcs", [128, NCH * CA * S], bf16,
                           kind="ExternalInput").ap()      # free = (ch, c, s)
    xsc_d = nc.dram_tensor("xsc", [128, NCH * S * CP], bf16,
                           kind="ExternalInput").ap()      # free = (ch, s, c10)
    wc_d = nc.dram_tensor("wc", [128, J * D * CP], bf16,
                          kind="ExternalInput").ap()       # free = (j, d, c10)
    wu_d = nc.dram_tensor("wu", [128, J * CA * D], bf16,
                          kind="ExternalInput").ap()       # free = (j, c, d)
    ones_d = nc.dram_tensor("onesb", [128, 128], bf16,
                            kind="ExternalInput").ap()     # blockdiag over b4
    eye_d = nc.dram_tensor("eye", [128, 128], bf16,
                           kind="ExternalInput").ap()      # identity
    v_d = nc.dram_tensor("v", [BLOC, J * D], f32,
                         kind="ExternalOutput").ap()

    dmae = {"gpsimd": nc.gpsimd, "sync": nc.sync}[dma_eng]
    with tile.TileContext(nc) as tc, ExitStack() as ctx:
        consts = ctx.enter_context(tc.tile_pool(name="consts", bufs=1))
        xpool = ctx.enter_context(tc.tile_pool(name="xpool", bufs=1))
        lpool = ctx.enter_context(tc.tile_pool(name="lpool", bufs=1))
        spool = ctx.enter_context(tc.tile_pool(name="scratch", bufs=bufs))
        small = ctx.enter_context(tc.tile_pool(name="small", bufs=bufs))
        vpool = ctx.enter_context(tc.tile_pool(name="vpool", bufs=bufs))
        psum = ctx.enter_context(tc.tile_pool(name="psum", bufs=4,
                                              space="PSUM"))

        # x first (the first compute stage needs it), consts after.
        xall = xpool.tile([128, NCH * CA * S], bf16, tag="xall", name="xall")
        CSZ = CA * S
        dmae.dma_start(xall[:, 0:CSZ], xcs_d[:, 0:CSZ])
        HX = 4 * CSZ
        dmae.dma_start(xall[:, CSZ:HX], xcs_d[:, CSZ:HX])
        wc_t = consts.tile([128, J * D * CP], bf16, tag="wc")
        dmae.dma_start(wc_t[:, :], wc_d[:, :])
        ones_t = consts.tile([128, 128], bf16, tag="onesb")
        dmae.dma_start(ones_t[:, :], ones_d[:, :])
        dmae.dma_start(xall[:, HX:], xcs_d[:, HX:])
        xall2 = xpool.tile([128, NCH * S * CP], bf16, tag="xall2",
                           name="xall2")
        dmae.dma_start(xall2[:, :], xsc_d[:, :])
        wu_t = consts.tile([128, J * CA * D], bf16, tag="wu")
        dmae.dma_start(wu_t[:, :], wu_d[:, :])
        eye_t = consts.tile([128, 128], bf16, tag="eye")
        dmae.dma_start(eye_t[:, :], eye_d[:, :])

        Xcs = [xall[:, ch * CA * S:(ch + 1) * CA * S] for ch in range(NCH)]
        Xsc = [xall2[:, ch * S * CP:(ch + 1) * S * CP] for ch in range(NCH)]
        L = []     # routing logits b, layout [p, (j, s)] bf16
        for ch in range(NCH):
            L.append(lpool.tile([128, J * S], bf16, tag=f"L{ch}",
                                name=f"L{ch}"))

        # Per-chunk in-flight state handed between stages.
        ST = [dict() for _ in range(NCH)]
        IT = [0]   # current iteration index (0-based), set at emission

        def split_tt(out4, in0, in1, k):
            """Emit a broadcast-product split along the j (outermost free)
            dim: DVE takes j < J-k, Pool takes the last k rows. Both write
            disjoint j-ranges of the same tile."""
            if k < J:
                nc.vector.tensor_tensor(out4[:, 0:J - k], in0[:, 0:J - k],
                                        in1[:, 0:J - k], Alu.mult)
            if k > 0:
                nc.gpsimd.tensor_tensor(out4[:, J - k:J], in0[:, J - k:J],
                                        in1[:, J - k:J], Alu.mult)

        # ---------------- stage functions ----------------

        def st_xsum(ch):
            xsum = small.tile([128, CA], bf16, tag="xsum", name="xsum")
            with nc.allow_low_precision("bf16 routing intermediates"):
                nc.vector.tensor_reduce(
                    xsum[:, :],
                    Xcs[ch].rearrange("p (c s) -> p c s", c=CA), AxX,
                    Alu.add)
            ST[ch]["xsum"] = xsum

        def st_xs1(ch):
            xs1 = small.tile([128, CA], bf16, tag="xsum1", name="xs1")
            with nc.allow_low_precision("bf16 routing intermediates"):
                nc.vector.tensor_scalar_mul(xs1[:, :], ST[ch]["xsum"][:, :],
                                            1.0 / J)
            ST[ch]["t_b"] = (xs1[:, :].unsqueeze(1).unsqueeze(1)
                             .broadcast_to([128, J, D, CA]))

        def st_pc(ch):
            # prodC[p, (j, d, c)] = t_bcast * Wa
            pc = spool.tile([128, J * D * CP], bf16, tag="prodC", name="pc")
            pc4 = (pc[:, :].rearrange("p (j d c) -> p j d c", j=J, d=D)
                   [:, :, :, 0:CA])
            wc4 = (wc_t[:, :].rearrange("p (j d c) -> p j d c", j=J, d=D)
                   [:, :, :, 0:CA])
            split_tt(pc4, ST[ch]["t_b"], wc4, kc[IT[0]])
            ST[ch]["pc"] = pc

        def st_mm(ch):
            # PE contracts g (partitions, via blockdiag ones) AND c (PSUM
            # accumulation over the 9 channel slices).
            pcz = ST[ch]["pc"][:, :].rearrange("p (a c) -> p a c", c=CP)
            ps = psum.tile([128, J * D], f32, tag="psum_s", name="ps")
            for c in range(CA):
                nc.tensor.matmul(ps[:, :], ones_t[:, :], pcz[:, :, c],
                                 start=(c == 0), stop=(c == CA - 1))
            ST[ch]["ps"] = ps

        def st_scpy(ch):
            # s in bf16 for the u-product, plus s^2 for the squash norm.
            ps = ST[ch]["ps"]
            sb = small.tile([128, J * D], bf16, tag="s_b16", name="sb")
            nc.scalar.copy(sb[:, :], ps[:, :])
            s2 = small.tile([128, J * D], f32, tag="s2", name="s2")
            nc.scalar.activation(s2[:, :], ps[:, :], Act.Square)
            ST[ch]["sb"] = sb
            ST[ch]["s2"] = s2

        def st_scpy3(ch):
            ps = ST[ch]["ps"]
            s_sb = small.tile([128, J * D], f32, tag="s_sb", name="s_sb")
            nc.scalar.copy(s_sb[:, :], ps[:, :])
            s2 = small.tile([128, J * D], f32, tag="s2", name="s2")
            nc.scalar.activation(s2[:, :], ps[:, :], Act.Square)
            ST[ch]["s_sb"] = s_sb
            ST[ch]["s2"] = s2

        def st_n2(ch):
            n2 = small.tile([128, J], f32, tag="n2", name="n2")
            nc.vector.tensor_reduce(
                n2[:, :],
                ST[ch]["s2"][:, :].rearrange("p (j d) -> p j d", j=J), AxX,
                Alu.add)
            ST[ch]["n2"] = n2

        def st_nr(ch):
            nr = small.tile([128, J], f32, tag="nrm", name="nr")
            nc.scalar.activation(nr[:, :], ST[ch]["n2"][:, :], Act.Sqrt)
            ST[ch]["nr"] = nr

        def st_rf(ch):
            n2p1 = small.tile([128, J], f32, tag="n2p1", name="n2p1")
            nc.vector.tensor_scalar_add(n2p1[:, :], ST[ch]["n2"][:, :], 1.0)
            r = small.tile([128, J], f32, tag="rcp", name="r")
            nc.vector.reciprocal(r[:, :], n2p1[:, :])
            f = small.tile([128, J], f32, tag="fac", name="f")
            nc.vector.tensor_tensor(f[:, :], ST[ch]["nr"][:, :], r[:, :],
                                    Alu.mult)
            ST[ch]["f"] = f

        def st_pu(ch):
            # produ[p, (j, c, d)] = s_bcast * Wa  (squash factor applied
            # later, on the d-reduced output)
            pu = spool.tile([128, J * CA * D], bf16, tag="produ", name="pu")
            pu4 = pu[:, :].rearrange("p (j c d) -> p j c d", j=J, c=CA)
            wu4 = wu_t[:, :].rearrange("p (j c d) -> p j c d", j=J, c=CA)
            sbb = (ST[ch]["sb"][:, :].rearrange("p (j d) -> p j d", j=J)
                   .unsqueeze(2).broadcast_to([128, J, CA, D]))
            split_tt(pu4, sbb, wu4, ku[IT[0]])
            ST[ch]["pu"] = pu

        def st_utree(ch):
            puz = ST[ch]["pu"][:, :].rearrange("p (a d) -> p a d", d=D)
            uA = spool.tile([128, 90 * 8], bf16, tag="tr720", name="uA")
            uA3 = uA[:, :].rearrange("p (a c) -> p a c", c=8)
            # first level split at the product's j engine boundary so the
            # DVE-produced rows reduce without waiting for the Pool half
            mu = (J - ku[IT[0]]) * CA
            nc.vector.tensor_tensor(uA3[:, 0:mu], puz[:, 0:mu, 0:8],
                                    puz[:, 0:mu, 8:16], Alu.add)
            if mu < J * CA:
                nc.vector.tensor_tensor(uA3[:, mu:], puz[:, mu:, 0:8],
                                        puz[:, mu:, 8:16], Alu.add)
            ST[ch]["uA3"] = uA3

        def st_umm(ch):
            # remaining d-reduction (8 uA slices) on PE via identity-matmul
            # PSUM accumulation.
            psu = psum.tile([128, J * CA], f32, tag="psum_e", name="psu")
            uA3 = ST[ch]["uA3"]
            for k in range(8):
                nc.tensor.matmul(psu[:, :], eye_t[:, :], uA3[:, :, k],
                                 start=(k == 0), stop=(k == 7))
            ST[ch]["psu"] = psu

        def st_ucp(ch):
            ut = small.tile([128, J * CP], bf16, tag="ut", name="ut")
            ut3 = ut[:, :].rearrange("p (j c) -> p j c", j=J)[:, :, 0:CA]
            nc.scalar.copy(
                ut3, ST[ch]["psu"][:, :].rearrange("p (j c) -> p j c", j=J))
            ST[ch]["ut"] = ut

        def st_uscale(ch):
            # u = f * (s @ W): fold the squash factor into u.
            u = small.tile([128, J * CP], bf16, tag="u", name="u")
            u3 = u[:, :].rearrange("p (j c) -> p j c", j=J)[:, :, 0:CA]
            ut3 = (ST[ch]["ut"][:, :].rearrange("p (j c) -> p j c", j=J)
                   [:, :, 0:CA])
            fb = (ST[ch]["f"][:, :].unsqueeze(2)
                  .broadcast_to([128, J, CA]))
            nc.vector.tensor_tensor(u3, ut3, fb, Alu.mult)
            ST[ch]["u"] = u

        def st_pe(ch):
            # prodE[p, (j, s, c)] over all 9 channels (the ones channel
            # carries u[j,8], summed into db by the PE c-contraction).
            pe = spool.tile([128, J * S * CP], bf16, tag="bigP", name="pe")
            pe4 = (pe[:, :].rearrange("p (j s c) -> p j s c", j=J, s=S)
                   [:, :, :, 0:CA])
            ub = (ST[ch]["u"][:, :].rearrange("p (j c) -> p j c", j=J)
                  [:, :, 0:CA].unsqueeze(2)
                  .broadcast_to([128, J, S, CA]))
            xb = (Xsc[ch].rearrange("p (s c) -> p s c", s=S)
                  [:, :, 0:CA].unsqueeze(1)
                  .broadcast_to([128, J, S, CA]))
            split_tt(pe4, ub, xb, ke[IT[0]])
            ST[ch]["pe"] = pe

        def st_emm(ch):
            # db[p, (j, s)] = sum_c prodE: identity matmul with PSUM
            # accumulation over the 9 channel slices (partition-preserving).
            pez = ST[ch]["pe"][:, :].rearrange("p (a c) -> p a c", c=CP)
            pse = psum.tile([128, J * S], f32, tag="psum_e", name="pse")
            for c in range(CA):
                nc.tensor.matmul(pse[:, :], eye_t[:, :], pez[:, :, c],
                                 start=(c == 0), stop=(c == CA - 1))
            ST[ch]["pse"] = pse

        def st_lcp1(ch):
            # iteration 1: L = db1 (b was zero) — straight psum evacuation.
            nc.scalar.copy(L[ch][:, :], ST[ch]["pse"][:, :])

        def st_lcp2(ch):
            db = small.tile([128, J * S], bf16, tag="db", name="db")
            nc.scalar.copy(db[:, :], ST[ch]["pse"][:, :])
            ST[ch]["db"] = db

        def st_ladd(ch):
            nc.vector.tensor_tensor(L[ch][:, :], L[ch][:, :],
                                    ST[ch]["db"][:, :], Alu.add)

        def st_exp(ch):
            et = spool.tile([128, J * S], bf16, tag="expt", name="et")
            nc.scalar.activation(et[:, :], L[ch][:, :], Act.Exp)
            ST[ch]["et"] = et

        def st_zred(ch):
            z = small.tile([128, S], f32, tag="z", name="z")
            ejs = (ST[ch]["et"][:, :].rearrange("p (j s) -> p j s", j=J)
                   .transpose([0, 2, 1]))
            nc.vector.tensor_reduce(z[:, :], ejs, AxX, Alu.add)
            ST[ch]["z"] = z

        def st_zr(ch):
            zr = small.tile([128, S], bf16, tag="zr", name="zr")
            with nc.allow_low_precision("bf16 softmax normalizer"):
                nc.vector.reciprocal(zr[:, :], ST[ch]["z"][:, :])
            ST[ch]["zr"] = zr

        def st_ct(ch):
            ct = spool.tile([128, J * S], bf16, tag="ct", name="ct")
            zb = ST[ch]["zr"][:, :].unsqueeze(1).broadcast_to([128, J, S])
            nc.vector.tensor_tensor(
                ct[:, :].rearrange("p (j s) -> p j s", j=J),
                ST[ch]["et"][:, :].rearrange("p (j s) -> p j s", j=J), zb,
                Alu.mult)
            ST[ch]["ct"] = ct

        def st_pb(ch):
            pb = spool.tile([128, J * CA * S], bf16, tag="bigP", name="pb")
            pb4 = pb[:, :].rearrange("p (j c s) -> p j c s", j=J, c=CA)
            cb = (ST[ch]["ct"][:, :].rearrange("p (j s) -> p j s", j=J)
                  .unsqueeze(2).broadcast_to([128, J, CA, S]))
            xb = (Xcs[ch].rearrange("p (c s) -> p c s", c=CA)
                  .unsqueeze(1).broadcast_to([128, J, CA, S]))
            split_tt(pb4, cb, xb, kb[IT[0] - 1])
            ST[ch]["pb"] = pb

        def st_btree(ch):
            pbz = ST[ch]["pb"][:, :].rearrange("p (a s) -> p a s", s=S)
            bA = spool.tile([128, 90 * 16], bf16, tag="tr1440", name="bA")
            bA3 = bA[:, :].rearrange("p (a c) -> p a c", c=16)
            mb = (J - kb[IT[0] - 1]) * CA
            nc.vector.tensor_tensor(bA3[:, 0:mb], pbz[:, 0:mb, 0:16],
                                    pbz[:, 0:mb, 16:32], Alu.add)
            if mb < J * CA:
                nc.vector.tensor_tensor(bA3[:, mb:], pbz[:, mb:, 0:16],
                                        pbz[:, mb:, 16:32], Alu.add)
            bB = spool.tile([128, 90 * 8], bf16, tag="tr720", name="bB")
            bB3 = bB[:, :].rearrange("p (a c) -> p a c", c=8)
            nc.vector.tensor_tensor(bB3, bA3[:, :, 0:8], bA3[:, :, 8:16],
                                    Alu.add)
            # tail s=32..35 pairs
            bT = spool.tile([128, 90 * 2], bf16, tag="tr180a", name="bT")
            bT3 = bT[:, :].rearrange("p (a c) -> p a c", c=2)
            nc.vector.tensor_tensor(bT3, pbz[:, :, 32:34], pbz[:, :, 34:36],
                                    Alu.add)
            ST[ch]["bB3"] = bB3
            ST[ch]["bT3"] = bT3

        def st_tmm(ch):
            # remaining s-reduction (8 bB slices + 2 tail slices) on PE via
            # identity-matmul PSUM accumulation.
            pst = psum.tile([128, J * CA], f32, tag="psum_e", name="pst")
            bB3, bT3 = ST[ch]["bB3"], ST[ch]["bT3"]
            for k in range(8):
                nc.tensor.matmul(pst[:, :], eye_t[:, :], bB3[:, :, k],
                                 start=(k == 0), stop=False)
            nc.tensor.matmul(pst[:, :], eye_t[:, :], bT3[:, :, 0],
                             start=False, stop=False)
            nc.tensor.matmul(pst[:, :], eye_t[:, :], bT3[:, :, 1],
                             start=False, stop=True)
            ST[ch]["pst"] = pst

        def st_tcp(ch):
            t = small.tile([128, J * CP], bf16, tag="tt", name="t")
            t3 = t[:, :].rearrange("p (j c) -> p j c", j=J)[:, :, 0:CA]
            nc.scalar.copy(
                t3, ST[ch]["pst"][:, :].rearrange("p (j c) -> p j c", j=J))
            ST[ch]["t_b"] = (t[:, :].rearrange("p (j c) -> p j c", j=J)
                             [:, :, 0:CA].unsqueeze(2)
                             .broadcast_to([128, J, D, CA]))

        def st_vt(ch):
            # final v = s * f, fp32
            vt = vpool.tile([128, J * D], f32, tag="vtf", name="vt")
            fb = (ST[ch]["f"][:, :].unsqueeze(2)
                  .broadcast_to([128, J, D]))
            nc.vector.tensor_tensor(
                vt[:, :].rearrange("p (j d) -> p j d", j=J),
                ST[ch]["s_sb"][:, :].rearrange("p (j d) -> p j d", j=J), fb,
                Alu.mult)
            ST[ch]["vt"] = vt

        def st_out(ch):
            dmae.dma_start(v_d[ch * B4:(ch + 1) * B4, :],
                           ST[ch]["vt"][0:128:NS, :])

        # ---------------- emission: stage-major wavefront ----------------
        # Chunks are processed in groups of `bufs`: within a group, stages
        # are emitted stage-major (so every engine has independent work from
        # the other chunks of the group), and every tile's consumers are
        # emitted before the next group recycles its buffer.

        def emit(stages):
            for g0 in range(0, NCH, grp):
                for fn in stages:
                    for ch in range(g0, min(g0 + grp, NCH)):
                        fn(ch)

        c_sq_u = [st_pc, st_mm, st_scpy, st_n2, st_nr, st_rf, st_pu,
                  st_utree, st_umm, st_ucp, st_uscale, st_pe, st_emm]
        softmax_b = [st_exp, st_zred, st_zr, st_ct, st_pb, st_btree, st_tmm,
                     st_tcp]

        IT[0] = 0
        emit([st_xsum, st_xs1] + c_sq_u)
        IT[0] = 1
        emit([st_lcp1] + softmax_b + c_sq_u)
        IT[0] = 2
        emit([st_lcp2, st_ladd] + softmax_b +
             [st_pc, st_mm, st_scpy3, st_n2, st_nr, st_rf, st_vt, st_out])

    if split_waits:
        _split_multi_waits(nc)
    return nc


def _split_multi_waits(nc):
    """Walrus's cayman codegen allows exactly ONE sync wait per TPB
    instruction (NEURON_ISA_TPB_EVENTS has a single wait slot). Tile's
    scheduler attaches several waits to dependency-merge instructions,
    which the native bass encoder handles but the neuronx-cc path rejects
    ("Too many sync wait commands"). Split the extras onto engine-local
    NoOp instructions inserted immediately before the owner so the wait
    semantics are unchanged.
    """
    from concourse import mybir

    for bbname, bbwrap in nc.bb_map.items():
        bb = bbwrap.bb
        insts = bb.instructions
        i = 0
        while i < len(insts):
            ins = insts[i]
            si = getattr(ins, "sync_info", None)
            if si is None or len(si.on_wait or []) <= 1:
                i += 1
                continue
            waits = list(si.on_wait)
            engine = ins.engine
            for w in waits[:-1]:
                nop = mybir.InstNoOp(
                    name=nc.get_next_instruction_name(),
                    engine=engine,
                    bass_nofuse=True,
                    sync_info=mybir.SyncInfo(on_wait=[w], on_update=[]),
                )
                insts.insert(i, nop)
                i += 1
            ins.sync_info = mybir.SyncInfo(on_wait=[waits[-1]],
                                           on_update=si.on_update)
            i += 1


def _get_program(split_waits=True, **kw):
    key = ("nc", split_waits, tuple(sorted(kw.items())))
    if key not in _CACHE:
        _CACHE[key] = _build_program(split_waits, **kw)
    return _CACHE[key]


def _host_prep(x, W, bias):
    """Build per-core input maps."""
    bf = np.float16
    x = np.ascontiguousarray(x, dtype=np.float32)
    W = np.ascontiguousarray(W, dtype=np.float32)
    bias = np.ascontiguousarray(bias, dtype=np.float32)
    bs = x.shape[0]

    xga = x.reshape(bs, NS, C_IN, S)
    xa = np.concatenate(
        [xga, np.ones((bs, NS, 1, S), dtype=np.float32)], axis=2)
    # [core, ch, b4, g, c, s] -> partition-major [core, b4, g, ch, c, s]
    x6 = xa.reshape(NCORES, NCH, B4, NS, CA, S)
    x6p = x6.transpose(0, 2, 3, 1, 4, 5)
    xcs = np.ascontiguousarray(x6p).reshape(
        NCORES, 128, NCH * CA * S).astype(bf)
    x6sc = x6p.transpose(0, 1, 2, 3, 5, 4)    # [.., ch, s, c]
    x6sp = np.concatenate(
        [x6sc, np.zeros(x6sc.shape[:-1] + (CP - CA,), np.float32)], axis=-1)
    xsc = np.ascontiguousarray(x6sp).reshape(
        NCORES, 128, NCH * S * CP).astype(bf)

    Wa = np.concatenate(
        [W.reshape(NS, J, D, C_IN),
         bias.reshape(NS, J, D, 1)], axis=3)            # [g, j, d, c]
    Wap = np.concatenate(
        [Wa, np.zeros(Wa.shape[:-1] + (CP - CA,), np.float32)], axis=-1)
    wc = np.tile(Wap.reshape(NS, J * D * CP), (B4, 1)).astype(bf)
    wu = np.tile(
        Wa.transpose(0, 1, 3, 2).reshape(NS, J * CA * D),
        (B4, 1)).astype(bf)                             # [128, (j,c,d)]
    onesb = np.kron(np.eye(B4, dtype=np.float32),
                    np.ones((NS, NS), dtype=np.float32)).astype(bf)
    eye = np.eye(128, dtype=np.float32).astype(bf)

    in_maps = [
        {"xcs": np.ascontiguousarray(xcs[k]),
         "xsc": np.ascontiguousarray(xsc[k]),
         "wc": wc, "wu": wu, "onesb": onesb, "eye": eye}
        for k in range(NCORES)
    ]
    return in_maps


def kernel(x, W, bias, b0):
    from concourse.bass_utils import run_bass_kernel_spmd

    nc = _get_program()
    in_maps = _host_prep(x, W, bias)
    res = run_bass_kernel_spmd(nc, in_maps, list(range(NCORES)))
    out = np.concatenate([res.results[k]["v"] for k in range(NCORES)],
                         axis=0)
    return np.ascontiguousarray(out.reshape(NCORES * BLOC, J, D))


# revision 22
# speedup vs baseline: 2.6082x; 1.0820x over previous
# Trainium2 Bass kernel for nn_CapLayer (CapsNet grouped 1x1 conv + dynamic routing).
#
# Key algebraic restructuring: the huge intermediate pred[b, i=(g,s), (j,d)]
# (188MB for the full batch) is NEVER materialized. Routing is computed in a
# factored form:
#   pred[b,(g,s),(j,d)] = sum_c Wa[g,j,d,c] * xga[b,g,c,s]     (c augmented with
#                                                               a ones channel to
#                                                               absorb the bias)
#   t[b,j,g,c]  = sum_s c[b,j,(g,s)] * xga[b,g,c,s]
#   s[b,j,d]    = sum_{g,c} t[b,j,g,c] * Wa[g,j,d,c]
#   u[b,j,g,c]  = sum_d v[b,j,d] * Wa[g,j,d,c]
#   db[b,j,g,s] = sum_c u[b,j,g,c] * xga[b,g,c,s]
# Iteration 1 collapses (softmax of zeros is uniform): t1 = xsum / J.
# The squash factor f = |s|/(1+|s|^2) is folded into u (u = f * (s @ W))
# instead of materializing v, which takes the scalar squash chain off the
# critical path between the PE matmul and the next product.
#
# Sharding: pure data parallel, 32 samples per core across 8 cores.
# On-chip layout: partition p = (b4, g) with 4 samples x 32 groups = 128
# partitions; 8 chunks cover the 32 local samples.
#
# Engine placement:
#  - The two W-contractions ride the TensorEngine: the g+c contraction for
#    s[b,(j,d)] uses a block-diagonal ones matrix (which also replicates s
#    across the g-partitions for free), and the c-contraction for db uses
#    an identity matrix with PSUM accumulation over the 9 channel slices —
#    this replaces a whole DVE add-tree and absorbs the bias channel.
#  - Every big broadcast-product is split along j between DVE (bf16 2x mode)
#    and GPSIMD/Pool so both engines chew each product concurrently.
#  - The Activation engine does psum evacuation (including writing db
#    directly into the logits), exp, square, sqrt.
#
# Schedule: stages are emitted in a wavefront across chunk groups of `bufs`
# (stage-major order) so each engine always has independent work from other
# chunks while one chunk traverses the cross-engine chain.

import sys

import numpy as np

# concourse (Bass/Tile) ships with the container; make sure it's importable
# when the grader runs kernel.py from a bare directory.
for _p in ("/opt/trn_rl_repo", "/root/.axon_site/_ro/trn_rl_repo"):
    if _p not in sys.path:
        sys.path.insert(0, _p)

NS, J, D, C_IN, H, WID, RN = 32, 10, 16, 8, 6, 6, 3
S = H * WID            # 36 spatial positions
CA = C_IN + 1          # 9 channels including the ones channel
CP = 10                # padded channel stride (4B alignment for bf16 rows)
NCORES = 8
BLOC = 32              # samples per core
B4 = 4                 # samples per chunk
NCH = BLOC // B4       # 8 chunks

_CACHE = {}


def _build_program(split_waits=True, kb=(3, 3), ke=(2, 3), kc=(3, 3, 4),
                   ku=(3, 3), bufs=6, grp=4, dma_eng="sync"):
    kb = (kb, kb) if isinstance(kb, int) else kb
    ke = (ke, ke) if isinstance(ke, int) else ke
    kc = (kc, kc, kc) if isinstance(kc, int) else kc
    ku = (ku, ku) if isinstance(ku, int) else ku
    from contextlib import ExitStack

    import concourse.bass as bass
    import concourse.tile as tile
    from concourse import mybir

    f32 = mybir.dt.float32
    bf16 = mybir.dt.float16
    Alu = mybir.AluOpType
    Act = mybir.ActivationFunctionType
    AxX = mybir.AxisListType.X

    nc = bass.Bass("TRN2", target_bir_lowering=True, debug=False,
                   num_devices=NCORES)

    xcs_d = nc.dram_tensor("xcs", [128, NCH * CA * S], bf16,
                           kind="ExternalInput").ap()      # free = (ch, c, s)
    xsc_d = nc.dram_tensor("xsc", [128, NCH * S * CP], bf16,
                           kind="ExternalInput").ap()      # free = (ch, s, c10)
    wc_d = nc.dram_tensor("wc", [128, J * D * CP], bf16,
                          kind="ExternalInput").ap()       # free = (j, d, c10)
    wu_d = nc.dram_tensor("wu", [128, J * CA * D], bf16,
                          kind="ExternalInput").ap()       # free = (j, c, d)
    ones_d = nc.dram_tensor("onesb", [128, 128], bf16,
                            kind="ExternalInput").ap()     # blockdiag over b4
    eye_d = nc.dram_tensor("eye", [128, 128], bf16,
                           kind="ExternalInput").ap()      # identity
    v_d = nc.dram_tensor("v", [BLOC, J * D], f32,
                         kind="ExternalOutput").ap()

    dmae = {"gpsimd": nc.gpsimd, "sync": nc.sync}[dma_eng]
    with tile.TileContext(nc) as tc, ExitStack() as ctx:
        consts = ctx.enter_context(tc.tile_pool(name="consts", bufs=1))
        xpool = ctx.enter_context(tc.tile_pool(name="xpool", bufs=1))
        lpool = ctx.enter_context(tc.tile_pool(name="lpool", bufs=1))
        spool = ctx.enter_context(tc.tile_pool(name="scratch", bufs=bufs))
        small = ctx.enter_context(tc.tile_pool(name="small", bufs=bufs))
        vpool = ctx.enter_context(tc.tile_pool(name="vpool", bufs=bufs))
        psum = ctx.enter_context(tc.tile_pool(name="psum", bufs=4,
                                              space="PSUM"))

        # x first (the first compute stage needs it), consts after.
        xall = xpool.tile([128, NCH * CA * S], bf16, tag="xall", name="xall")
        CSZ = CA * S
        dmae.dma_start(xall[:, 0:CSZ], xcs_d[:, 0:CSZ])
        HX = 4 * CSZ
        dmae.dma_start(xall[:, CSZ:HX], xcs_d[:, CSZ:HX])
        wc_t = consts.tile([128, J * D * CP], bf16, tag="wc")
        dmae.dma_start(wc_t[:, :], wc_d[:, :])
        ones_t = consts.tile([128, 128], bf16, tag="onesb")
        dmae.dma_start(ones_t[:, :], ones_d[:, :])
        dmae.dma_start(xall[:, HX:], xcs_d[:, HX:])
        xall2 = xpool.tile([128, NCH * S * CP], bf16, tag="xall2",
                           name="xall2")
        dmae.dma_start(xall2[:, :], xsc_d[:, :])
        wu_t = consts.tile([128, J * CA * D], bf16, tag="wu")
        dmae.dma_start(wu_t[:, :], wu_d[:, :])
        eye_t = consts.tile([128, 128], bf16, tag="eye")
        dmae.dma_start(eye_t[:, :], eye_d[:, :])

        Xcs = [xall[:, ch * CA * S:(ch + 1) * CA * S] for ch in range(NCH)]
        Xsc = [xall2[:, ch * S * CP:(ch + 1) * S * CP] for ch in range(NCH)]
        L = []     # routing logits b, layout [p, (j, s)] bf16
        for ch in range(NCH):
            L.append(lpool.tile([128, J * S], bf16, tag=f"L{ch}",
                                name=f"L{ch}"))

        # Per-chunk in-flight state handed between stages.
        ST = [dict() for _ in range(NCH)]
        IT = [0]   # current iteration index (0-based), set at emission

        def split_tt(out4, in0, in1, k):
            """Emit a broadcast-product split along the j (outermost free)
            dim: DVE takes j < J-k, Pool takes the last k rows. Both write
            disjoint j-ranges of the same tile."""
            if k < J:
                nc.vector.tensor_tensor(out4[:, 0:J - k], in0[:, 0:J - k],
                                        in1[:, 0:J - k], Alu.mult)
            if k > 0:
                nc.gpsimd.tensor_tensor(out4[:, J - k:J], in0[:, J - k:J],
                                        in1[:, J - k:J], Alu.mult)

        # ---------------- stage functions ----------------

        def st_xsum(ch):
            xsum = small.tile([128, CA], bf16, tag="xsum", name="xsum")
            with nc.allow_low_precision("bf16 routing intermediates"):
                nc.vector.tensor_reduce(
                    xsum[:, :],
                    Xcs[ch].rearrange("p (c s) -> p c s", c=CA), AxX,
                    Alu.add)
            ST[ch]["xsum"] = xsum

        def st_xs1(ch):
            xs1 = small.tile([128, CA], bf16, tag="xsum1", name="xs1")
            with nc.allow_low_precision("bf16 routing intermediates"):
                nc.vector.tensor_scalar_mul(xs1[:, :], ST[ch]["xsum"][:, :],
                                            1.0 / J)
            ST[ch]["t_b"] = (xs1[:, :].unsqueeze(1).unsqueeze(1)
                             .broadcast_to([128, J, D, CA]))

        def st_pc(ch):
            # prodC[p, (j, d, c)] = t_bcast * Wa
            pc = spool.tile([128, J * D * CP], bf16, tag="prodC", name="pc")
            pc4 = (pc[:, :].rearrange("p (j d c) -> p j d c", j=J, d=D)
                   [:, :, :, 0:CA])
            wc4 = (wc_t[:, :].rearrange("p (j d c) -> p j d c", j=J, d=D)
                   [:, :, :, 0:CA])
            split_tt(pc4, ST[ch]["t_b"], wc4, kc[IT[0]])
            ST[ch]["pc"] = pc

        def st_mm(ch):
            # PE contracts g (partitions, via blockdiag ones) AND c (PSUM
            # accumulation over the 9 channel slices).
            pcz = ST[ch]["pc"][:, :].rearrange("p (a c) -> p a c", c=CP)
            ps = psum.tile([128, J * D], f32, tag="psum_s", name="ps")
            for c in range(CA):
                nc.tensor.matmul(ps[:, :], ones_t[:, :], pcz[:, :, c],
                                 start=(c == 0), stop=(c == CA - 1))
            ST[ch]["ps"] = ps

        def st_scpy(ch):
            # s in bf16 for the u-product, plus s^2 for the squash norm.
            ps = ST[ch]["ps"]
            sb = small.tile([128, J * D], bf16, tag="s_b16", name="sb")
            nc.scalar.copy(sb[:, :], ps[:, :])
            s2 = small.tile([128, J * D], f32, tag="s2", name="s2")
            nc.scalar.activation(s2[:, :], ps[:, :], Act.Square)
            ST[ch]["sb"] = sb
            ST[ch]["s2"] = s2

        def st_scpy3(ch):
            ps = ST[ch]["ps"]
            s_sb = small.tile([128, J * D], f32, tag="s_sb", name="s_sb")
            nc.scalar.copy(s_sb[:, :], ps[:, :])
            s2 = small.tile([128, J * D], f32, tag="s2", name="s2")
            nc.scalar.activation(s2[:, :], ps[:, :], Act.Square)
            ST[ch]["s_sb"] = s_sb
            ST[ch]["s2"] = s2

        def st_n2(ch):
            n2 = small.tile([128, J], f32, tag="n2", name="n2")
            nc.vector.tensor_reduce(
                n2[:, :],
                ST[ch]["s2"][:, :].rearrange("p (j d) -> p j d", j=J), AxX,
                Alu.add)
            ST[ch]["n2"] = n2

        def st_nr(ch):
            nr = small.tile([128, J], f32, tag="nrm", name="nr")
            nc.scalar.activation(nr[:, :], ST[ch]["n2"][:, :], Act.Sqrt)
            ST[ch]["nr"] = nr

        def st_rf(ch):
            n2p1 = small.tile([128, J], f32, tag="n2p1", name="n2p1")
            nc.vector.tensor_scalar_add(n2p1[:, :], ST[ch]["n2"][:, :], 1.0)
            r = small.tile([128, J], f32, tag="rcp", name="r")
            nc.vector.reciprocal(r[:, :], n2p1[:, :])
            f = small.tile([128, J], f32, tag="fac", name="f")
            nc.vector.tensor_tensor(f[:, :], ST[ch]["nr"][:, :], r[:, :],
                                    Alu.mult)
            ST[ch]["f"] = f

        def st_pu(ch):
            # produ[p, (j, c, d)] = s_bcast * Wa  (squash factor applied
            # later, on the d-reduced output)
            pu = spool.tile([128, J * CA * D], bf16, tag="produ", name="pu")
            pu4 = pu[:, :].rearrange("p (j c d) -> p j c d", j=J, c=CA)
            wu4 = wu_t[:, :].rearrange("p (j c d) -> p j c d", j=J, c=CA)
            sbb = (ST[ch]["sb"][:, :].rearrange("p (j d) -> p j d", j=J)
                   .unsqueeze(2).broadcast_to([128, J, CA, D]))
            split_tt(pu4, sbb, wu4, ku[IT[0]])
            ST[ch]["pu"] = pu

        def st_umm(ch):
            # d-reduction (16 slices) on PE via identity-matmul PSUM
            # accumulation, straight off the product.
            puz = ST[ch]["pu"][:, :].rearrange("p (a d) -> p a d", d=D)
            psu = psum.tile([128, J * CA], f32, tag="psum_e", name="psu")
            for k in range(D):
                nc.tensor.matmul(psu[:, :], eye_t[:, :], puz[:, :, k],
                                 start=(k == 0), stop=(k == D - 1))
            ST[ch]["psu"] = psu

        def st_ucp(ch):
            ut = small.tile([128, J * CP], bf16, tag="ut", name="ut")
            ut3 = ut[:, :].rearrange("p (j c) -> p j c", j=J)[:, :, 0:CA]
            nc.scalar.copy(
                ut3, ST[ch]["psu"][:, :].rearrange("p (j c) -> p j c", j=J))
            ST[ch]["ut"] = ut

        def st_uscale(ch):
            # u = f * (s @ W): fold the squash factor into u.
            u = small.tile([128, J * CP], bf16, tag="u", name="u")
            u3 = u[:, :].rearrange("p (j c) -> p j c", j=J)[:, :, 0:CA]
            ut3 = (ST[ch]["ut"][:, :].rearrange("p (j c) -> p j c", j=J)
                   [:, :, 0:CA])
            fb = (ST[ch]["f"][:, :].unsqueeze(2)
                  .broadcast_to([128, J, CA]))
            nc.vector.tensor_tensor(u3, ut3, fb, Alu.mult)
            ST[ch]["u"] = u

        def st_pe(ch):
            # prodE[p, (j, s, c)] over all 9 channels (the ones channel
            # carries u[j,8], summed into db by the PE c-contraction).
            pe = spool.tile([128, J * S * CP], bf16, tag="bigP", name="pe")
            pe4 = (pe[:, :].rearrange("p (j s c) -> p j s c", j=J, s=S)
                   [:, :, :, 0:CA])
            ub = (ST[ch]["u"][:, :].rearrange("p (j c) -> p j c", j=J)
                  [:, :, 0:CA].unsqueeze(2)
                  .broadcast_to([128, J, S, CA]))
            xb = (Xsc[ch].rearrange("p (s c) -> p s c", s=S)
                  [:, :, 0:CA].unsqueeze(1)
                  .broadcast_to([128, J, S, CA]))
            split_tt(pe4, ub, xb, ke[IT[0]])
            ST[ch]["pe"] = pe

        def st_emm(ch):
            # db[p, (j, s)] = sum_c prodE: identity matmul with PSUM
            # accumulation over the 9 channel slices (partition-preserving).
            pez = ST[ch]["pe"][:, :].rearrange("p (a c) -> p a c", c=CP)
            pse = psum.tile([128, J * S], f32, tag="psum_e", name="pse")
            for c in range(CA):
                nc.tensor.matmul(pse[:, :], eye_t[:, :], pez[:, :, c],
                                 start=(c == 0), stop=(c == CA - 1))
            ST[ch]["pse"] = pse

        def st_lcp1(ch):
            # iteration 1: L = db1 (b was zero) — straight psum evacuation.
            nc.scalar.copy(L[ch][:, :], ST[ch]["pse"][:, :])

        def st_lcp2(ch):
            db = small.tile([128, J * S], bf16, tag="db", name="db")
            nc.scalar.copy(db[:, :], ST[ch]["pse"][:, :])
            ST[ch]["db"] = db

        def st_ladd(ch):
            nc.vector.tensor_tensor(L[ch][:, :], L[ch][:, :],
                                    ST[ch]["db"][:, :], Alu.add)

        def st_exp(ch):
            et = spool.tile([128, J * S], bf16, tag="expt", name="et")
            nc.scalar.activation(et[:, :], L[ch][:, :], Act.Exp)
            ST[ch]["et"] = et

        def st_zred(ch):
            z = small.tile([128, S], f32, tag="z", name="z")
            ejs = (ST[ch]["et"][:, :].rearrange("p (j s) -> p j s", j=J)
                   .transpose([0, 2, 1]))
            nc.vector.tensor_reduce(z[:, :], ejs, AxX, Alu.add)
            ST[ch]["z"] = z

        def st_zr(ch):
            zr = small.tile([128, S], bf16, tag="zr", name="zr")
            with nc.allow_low_precision("bf16 softmax normalizer"):
                nc.vector.reciprocal(zr[:, :], ST[ch]["z"][:, :])
            ST[ch]["zr"] = zr

        def st_ct(ch):
            ct = spool.tile([128, J * S], bf16, tag="ct", name="ct")
            zb = ST[ch]["zr"][:, :].unsqueeze(1).broadcast_to([128, J, S])
            nc.vector.tensor_tensor(
                ct[:, :].rearrange("p (j s) -> p j s", j=J),
                ST[ch]["et"][:, :].rearrange("p (j s) -> p j s", j=J), zb,
                Alu.mult)
            ST[ch]["ct"] = ct

        def st_pb(ch):
            pb = spool.tile([128, J * CA * S], bf16, tag="bigP", name="pb")
            pb4 = pb[:, :].rearrange("p (j c s) -> p j c s", j=J, c=CA)
            cb = (ST[ch]["ct"][:, :].rearrange("p (j s) -> p j s", j=J)
                  .unsqueeze(2).broadcast_to([128, J, CA, S]))
            xb = (Xcs[ch].rearrange("p (c s) -> p c s", c=CA)
                  .unsqueeze(1).broadcast_to([128, J, CA, S]))
            split_tt(pb4, cb, xb, kb[IT[0] - 1])
            ST[ch]["pb"] = pb

        def st_btree(ch):
            pbz = ST[ch]["pb"][:, :].rearrange("p (a s) -> p a s", s=S)
            bA = spool.tile([128, 90 * 16], bf16, tag="tr1440", name="bA")
            bA3 = bA[:, :].rearrange("p (a c) -> p a c", c=16)
            mb = (J - kb[IT[0] - 1]) * CA
            nc.vector.tensor_tensor(bA3[:, 0:mb], pbz[:, 0:mb, 0:16],
                                    pbz[:, 0:mb, 16:32], Alu.add)
            if mb < J * CA:
                nc.vector.tensor_tensor(bA3[:, mb:], pbz[:, mb:, 0:16],
                                        pbz[:, mb:, 16:32], Alu.add)
            # tail s=32..35 pairs
            bT = spool.tile([128, 90 * 2], bf16, tag="tr180a", name="bT")
            bT3 = bT[:, :].rearrange("p (a c) -> p a c", c=2)
            nc.vector.tensor_tensor(bT3, pbz[:, :, 32:34], pbz[:, :, 34:36],
                                    Alu.add)
            ST[ch]["bA3"] = bA3
            ST[ch]["bT3"] = bT3

        def st_tmm(ch):
            # remaining s-reduction (16 bA slices + 2 tail slices) on PE via
            # identity-matmul PSUM accumulation.
            pst = psum.tile([128, J * CA], f32, tag="psum_e", name="pst")
            bA3, bT3 = ST[ch]["bA3"], ST[ch]["bT3"]
            for k in range(16):
                nc.tensor.matmul(pst[:, :], eye_t[:, :], bA3[:, :, k],
                                 start=(k == 0), stop=False)
            nc.tensor.matmul(pst[:, :], eye_t[:, :], bT3[:, :, 0],
                             start=False, stop=False)
            nc.tensor.matmul(pst[:, :], eye_t[:, :], bT3[:, :, 1],
                             start=False, stop=True)
            ST[ch]["pst"] = pst

        def st_tcp(ch):
            t = small.tile([128, J * CP], bf16, tag="tt", name="t")
            t3 = t[:, :].rearrange("p (j c) -> p j c", j=J)[:, :, 0:CA]
            nc.scalar.copy(
                t3, ST[ch]["pst"][:, :].rearrange("p (j c) -> p j c", j=J))
            ST[ch]["t_b"] = (t[:, :].rearrange("p (j c) -> p j c", j=J)
                             [:, :, 0:CA].unsqueeze(2)
                             .broadcast_to([128, J, D, CA]))

        def st_vt(ch):
            # final v = s * f, fp32
            vt = vpool.tile([128, J * D], f32, tag="vtf", name="vt")
            fb = (ST[ch]["f"][:, :].unsqueeze(2)
                  .broadcast_to([128, J, D]))
            nc.vector.tensor_tensor(
                vt[:, :].rearrange("p (j d) -> p j d", j=J),
                ST[ch]["s_sb"][:, :].rearrange("p (j d) -> p j d", j=J), fb,
                Alu.mult)
            ST[ch]["vt"] = vt

        def st_out(ch):
            dmae.dma_start(v_d[ch * B4:(ch + 1) * B4, :],
                           ST[ch]["vt"][0:128:NS, :])

        # ---------------- emission: stage-major wavefront ----------------
        # Chunks are processed in groups of `bufs`: within a group, stages
        # are emitted stage-major (so every engine has independent work from
        # the other chunks of the group), and every tile's consumers are
        # emitted before the next group recycles its buffer.

        def emit(stages):
            for g0 in range(0, NCH, grp):
                for fn in stages:
                    for ch in range(g0, min(g0 + grp, NCH)):
                        fn(ch)

        c_sq_u = [st_pc, st_mm, st_scpy, st_n2, st_nr, st_rf, st_pu,
                  st_umm, st_ucp, st_uscale, st_pe, st_emm]
        softmax_b = [st_exp, st_zred, st_zr, st_ct, st_pb, st_btree, st_tmm,
                     st_tcp]

        IT[0] = 0
        emit([st_xsum, st_xs1] + c_sq_u)
        IT[0] = 1
        emit([st_lcp1] + softmax_b + c_sq_u)
        IT[0] = 2
        emit([st_lcp2, st_ladd] + softmax_b +
             [st_pc, st_mm, st_scpy3, st_n2, st_nr, st_rf, st_vt, st_out])

    if split_waits:
        _split_multi_waits(nc)
    return nc


def _split_multi_waits(nc):
    """Walrus's cayman codegen allows exactly ONE sync wait per TPB
    instruction (NEURON_ISA_TPB_EVENTS has a single wait slot). Tile's
    scheduler attaches several waits to dependency-merge instructions,
    which the native bass encoder handles but the neuronx-cc path rejects
    ("Too many sync wait commands"). Split the extras onto engine-local
    NoOp instructions inserted immediately before the owner so the wait
    semantics are unchanged.
    """
    from concourse import mybir

    for bbname, bbwrap in nc.bb_map.items():
        bb = bbwrap.bb
        insts = bb.instructions
        i = 0
        while i < len(insts):
            ins = insts[i]
            si = getattr(ins, "sync_info", None)
            if si is None or len(si.on_wait or []) <= 1:
                i += 1
                continue
            waits = list(si.on_wait)
            engine = ins.engine
            for w in waits[:-1]:
                nop = mybir.InstNoOp(
                    name=nc.get_next_instruction_name(),
                    engine=engine,
                    bass_nofuse=True,
                    sync_info=mybir.SyncInfo(on_wait=[w], on_update=[]),
                )
                insts.insert(i, nop)
                i += 1
            ins.sync_info = mybir.SyncInfo(on_wait=[waits[-1]],
                                           on_update=si.on_update)
            i += 1


def _get_program(split_waits=True, **kw):
    key = ("nc", split_waits, tuple(sorted(kw.items())))
    if key not in _CACHE:
        _CACHE[key] = _build_program(split_waits, **kw)
    return _CACHE[key]


def _host_prep(x, W, bias):
    """Build per-core input maps."""
    bf = np.float16
    x = np.ascontiguousarray(x, dtype=np.float32)
    W = np.ascontiguousarray(W, dtype=np.float32)
    bias = np.ascontiguousarray(bias, dtype=np.float32)
    bs = x.shape[0]

    xga = x.reshape(bs, NS, C_IN, S)
    xa = np.concatenate(
        [xga, np.ones((bs, NS, 1, S), dtype=np.float32)], axis=2)
    # [core, ch, b4, g, c, s] -> partition-major [core, b4, g, ch, c, s]
    x6 = xa.reshape(NCORES, NCH, B4, NS, CA, S)
    x6p = x6.transpose(0, 2, 3, 1, 4, 5)
    xcs = np.ascontiguousarray(x6p).reshape(
        NCORES, 128, NCH * CA * S).astype(bf)
    x6sc = x6p.transpose(0, 1, 2, 3, 5, 4)    # [.., ch, s, c]
    x6sp = np.concatenate(
        [x6sc, np.zeros(x6sc.shape[:-1] + (CP - CA,), np.float32)], axis=-1)
    xsc = np.ascontiguousarray(x6sp).reshape(
        NCORES, 128, NCH * S * CP).astype(bf)

    Wa = np.concatenate(
        [W.reshape(NS, J, D, C_IN),
         bias.reshape(NS, J, D, 1)], axis=3)            # [g, j, d, c]
    Wap = np.concatenate(
        [Wa, np.zeros(Wa.shape[:-1] + (CP - CA,), np.float32)], axis=-1)
    wc = np.tile(Wap.reshape(NS, J * D * CP), (B4, 1)).astype(bf)
    wu = np.tile(
        Wa.transpose(0, 1, 3, 2).reshape(NS, J * CA * D),
        (B4, 1)).astype(bf)                             # [128, (j,c,d)]
    onesb = np.kron(np.eye(B4, dtype=np.float32),
                    np.ones((NS, NS), dtype=np.float32)).astype(bf)
    eye = np.eye(128, dtype=np.float32).astype(bf)

    in_maps = [
        {"xcs": np.ascontiguousarray(xcs[k]),
         "xsc": np.ascontiguousarray(xsc[k]),
         "wc": wc, "wu": wu, "onesb": onesb, "eye": eye}
        for k in range(NCORES)
    ]
    return in_maps


def kernel(x, W, bias, b0):
    from concourse.bass_utils import run_bass_kernel_spmd

    nc = _get_program()
    in_maps = _host_prep(x, W, bias)
    res = run_bass_kernel_spmd(nc, in_maps, list(range(NCORES)))
    out = np.concatenate([res.results[k]["v"] for k in range(NCORES)],
                         axis=0)
    return np.ascontiguousarray(out.reshape(NCORES * BLOC, J, D))


# revision 24
# speedup vs baseline: 2.9327x; 1.1244x over previous
# Trainium2 Bass kernel for nn_CapLayer (CapsNet grouped 1x1 conv + dynamic routing).
#
# Key algebraic restructuring: the huge intermediate pred[b, i=(g,s), (j,d)]
# (188MB for the full batch) is NEVER materialized. Routing is computed in a
# factored form:
#   pred[b,(g,s),(j,d)] = sum_c Wa[g,j,d,c] * xga[b,g,c,s]     (c augmented with
#                                                               a ones channel to
#                                                               absorb the bias)
#   t[b,j,g,c]  = sum_s c[b,j,(g,s)] * xga[b,g,c,s]
#   s[b,j,d]    = sum_{g,c} t[b,j,g,c] * Wa[g,j,d,c]
#   u[b,j,g,c]  = sum_d v[b,j,d] * Wa[g,j,d,c]
#   db[b,j,g,s] = sum_c u[b,j,g,c] * xga[b,g,c,s]
# Iteration 1 collapses (softmax of zeros is uniform): t1 = xsum / J.
# The squash factor f = |s|/(1+|s|^2) is folded into u (u = f * (s @ W))
# instead of materializing v, which takes the scalar squash chain off the
# critical path between the PE matmul and the next product.
#
# Sharding: pure data parallel, 32 samples per core across 8 cores.
# On-chip layout: partition p = (b4, g) with 4 samples x 32 groups = 128
# partitions; 8 chunks cover the 32 local samples.
#
# Engine placement:
#  - The two W-contractions ride the TensorEngine: the g+c contraction for
#    s[b,(j,d)] uses a block-diagonal ones matrix (which also replicates s
#    across the g-partitions for free), and the c-contraction for db uses
#    an identity matrix with PSUM accumulation over the 9 channel slices —
#    this replaces a whole DVE add-tree and absorbs the bias channel.
#  - Every big broadcast-product is split along j between DVE (bf16 2x mode)
#    and GPSIMD/Pool so both engines chew each product concurrently.
#  - The Activation engine does psum evacuation (including writing db
#    directly into the logits), exp, square, sqrt.
#
# Schedule: stages are emitted in a wavefront across chunk groups of `bufs`
# (stage-major order) so each engine always has independent work from other
# chunks while one chunk traverses the cross-engine chain.

import sys

import numpy as np

# concourse (Bass/Tile) ships with the container; make sure it's importable
# when the grader runs kernel.py from a bare directory.
for _p in ("/opt/trn_rl_repo", "/root/.axon_site/_ro/trn_rl_repo"):
    if _p not in sys.path:
        sys.path.insert(0, _p)

NS, J, D, C_IN, H, WID, RN = 32, 10, 16, 8, 6, 6, 3
S = H * WID            # 36 spatial positions
CA = C_IN + 1          # 9 channels including the ones channel
CP = 10                # padded channel stride (4B alignment for bf16 rows)
NCORES = 8
BLOC = 32              # samples per core
B4 = 4                 # samples per chunk
NCH = BLOC // B4       # 8 chunks

_CACHE = {}


def _build_program(split_waits=True, kb=(2, 2), ke=(2, 2), kc=(3, 4, 4),
                   ku=(3, 3), bufs=6, grp=4, dma_eng="sync"):
    kb = (kb, kb) if isinstance(kb, int) else kb
    ke = (ke, ke) if isinstance(ke, int) else ke
    kc = (kc, kc, kc) if isinstance(kc, int) else kc
    ku = (ku, ku) if isinstance(ku, int) else ku
    from contextlib import ExitStack

    import concourse.bass as bass
    import concourse.tile as tile
    from concourse import mybir

    f32 = mybir.dt.float32
    bf16 = mybir.dt.float16
    Alu = mybir.AluOpType
    Act = mybir.ActivationFunctionType
    AxX = mybir.AxisListType.X

    nc = bass.Bass("TRN2", target_bir_lowering=True, debug=False,
                   num_devices=NCORES)

    xcs_d = nc.dram_tensor("xcs", [128, NCH * CA * S], bf16,
                           kind="ExternalInput").ap()      # free = (ch, c, s)
    xsc_d = nc.dram_tensor("xsc", [128, NCH * S * CP], bf16,
                           kind="ExternalInput").ap()      # free = (ch, s, c10)
    wc_d = nc.dram_tensor("wc", [128, J * D * CP], bf16,
                          kind="ExternalInput").ap()       # free = (j, d, c10)
    wu_d = nc.dram_tensor("wu", [128, J * CA * D], bf16,
                          kind="ExternalInput").ap()       # free = (j, c, d)
    ones_d = nc.dram_tensor("onesb", [128, 128], bf16,
                            kind="ExternalInput").ap()     # blockdiag over b4
    eye_d = nc.dram_tensor("eye", [128, 128], bf16,
                           kind="ExternalInput").ap()      # identity
    v_d = nc.dram_tensor("v", [BLOC, J * D], f32,
                         kind="ExternalOutput").ap()

    dmae = {"gpsimd": nc.gpsimd, "sync": nc.sync}[dma_eng]
    with tile.TileContext(nc) as tc, ExitStack() as ctx:
        consts = ctx.enter_context(tc.tile_pool(name="consts", bufs=1))
        xpool = ctx.enter_context(tc.tile_pool(name="xpool", bufs=1))
        lpool = ctx.enter_context(tc.tile_pool(name="lpool", bufs=1))
        spool = ctx.enter_context(tc.tile_pool(name="scratch", bufs=bufs))
        small = ctx.enter_context(tc.tile_pool(name="small", bufs=bufs))
        vpool = ctx.enter_context(tc.tile_pool(name="vpool", bufs=bufs))
        psum = ctx.enter_context(tc.tile_pool(name="psum", bufs=4,
                                              space="PSUM"))

        # x first (the first compute stage needs it), consts after.
        xall = xpool.tile([128, NCH * CA * S], bf16, tag="xall", name="xall")
        CSZ = CA * S
        dmae.dma_start(xall[:, 0:CSZ], xcs_d[:, 0:CSZ])
        HX = 4 * CSZ
        dmae.dma_start(xall[:, CSZ:HX], xcs_d[:, CSZ:HX])
        wc_t = consts.tile([128, J * D * CP], bf16, tag="wc")
        dmae.dma_start(wc_t[:, :], wc_d[:, :])
        ones_t = consts.tile([128, 128], bf16, tag="onesb")
        dmae.dma_start(ones_t[:, :], ones_d[:, :])
        dmae.dma_start(xall[:, HX:], xcs_d[:, HX:])
        xall2 = xpool.tile([128, NCH * S * CP], bf16, tag="xall2",
                           name="xall2")
        dmae.dma_start(xall2[:, :], xsc_d[:, :])
        wu_t = consts.tile([128, J * CA * D], bf16, tag="wu")
        dmae.dma_start(wu_t[:, :], wu_d[:, :])
        eye_t = consts.tile([128, 128], bf16, tag="eye")
        dmae.dma_start(eye_t[:, :], eye_d[:, :])

        Xcs = [xall[:, ch * CA * S:(ch + 1) * CA * S] for ch in range(NCH)]
        Xsc = [xall2[:, ch * S * CP:(ch + 1) * S * CP] for ch in range(NCH)]
        L = []     # routing logits b, layout [p, (j, s)] bf16
        for ch in range(NCH):
            L.append(lpool.tile([128, J * S], bf16, tag=f"L{ch}",
                                name=f"L{ch}"))

        # Per-chunk in-flight state handed between stages.
        ST = [dict() for _ in range(NCH)]
        IT = [0]   # current iteration index (0-based), set at emission

        def split_tt(out4, in0, in1, k):
            """Emit a broadcast-product split along the j (outermost free)
            dim: DVE takes j < J-k, Pool takes the last k rows. Both write
            disjoint j-ranges of the same tile."""
            if k < J:
                nc.vector.tensor_tensor(out4[:, 0:J - k], in0[:, 0:J - k],
                                        in1[:, 0:J - k], Alu.mult)
            if k > 0:
                nc.gpsimd.tensor_tensor(out4[:, J - k:J], in0[:, J - k:J],
                                        in1[:, J - k:J], Alu.mult)

        # ---------------- stage functions ----------------

        def st_xsum(ch):
            xsum = small.tile([128, CA], bf16, tag="xsum", name="xsum")
            with nc.allow_low_precision("bf16 routing intermediates"):
                nc.vector.tensor_reduce(
                    xsum[:, :],
                    Xcs[ch].rearrange("p (c s) -> p c s", c=CA), AxX,
                    Alu.add)
            ST[ch]["xsum"] = xsum

        def st_xs1(ch):
            xs1 = small.tile([128, CA], bf16, tag="xsum1", name="xs1")
            with nc.allow_low_precision("bf16 routing intermediates"):
                nc.vector.tensor_scalar_mul(xs1[:, :], ST[ch]["xsum"][:, :],
                                            1.0 / J)
            ST[ch]["t_b"] = (xs1[:, :].unsqueeze(1).unsqueeze(1)
                             .broadcast_to([128, J, D, CA]))

        def st_pc(ch):
            # prodC[p, (j, d, c)] = t_bcast * Wa
            pc = spool.tile([128, J * D * CP], bf16, tag="prodC", name="pc")
            pc4 = (pc[:, :].rearrange("p (j d c) -> p j d c", j=J, d=D)
                   [:, :, :, 0:CA])
            wc4 = (wc_t[:, :].rearrange("p (j d c) -> p j d c", j=J, d=D)
                   [:, :, :, 0:CA])
            split_tt(pc4, ST[ch]["t_b"], wc4, kc[IT[0]])
            ST[ch]["pc"] = pc

        def st_mm(ch):
            # PE contracts g (partitions, via blockdiag ones) AND c (PSUM
            # accumulation over the 9 channel slices).
            pcz = ST[ch]["pc"][:, :].rearrange("p (a c) -> p a c", c=CP)
            ps = psum.tile([128, J * D], f32, tag="psum_s", name="ps")
            for c in range(CA):
                nc.tensor.matmul(ps[:, :], ones_t[:, :], pcz[:, :, c],
                                 start=(c == 0), stop=(c == CA - 1))
            ST[ch]["ps"] = ps

        def st_scpy(ch):
            # s in bf16 for the u-product, plus s^2 for the squash norm.
            ps = ST[ch]["ps"]
            sb = small.tile([128, J * D], bf16, tag="s_b16", name="sb")
            nc.scalar.copy(sb[:, :], ps[:, :])
            s2 = small.tile([128, J * D], f32, tag="s2", name="s2")
            nc.scalar.activation(s2[:, :], ps[:, :], Act.Square)
            ST[ch]["sb"] = sb
            ST[ch]["s2"] = s2

        def st_scpy3(ch):
            ps = ST[ch]["ps"]
            s_sb = small.tile([128, J * D], f32, tag="s_sb", name="s_sb")
            nc.scalar.copy(s_sb[:, :], ps[:, :])
            s2 = small.tile([128, J * D], f32, tag="s2", name="s2")
            nc.scalar.activation(s2[:, :], ps[:, :], Act.Square)
            ST[ch]["s_sb"] = s_sb
            ST[ch]["s2"] = s2

        def st_n2(ch):
            n2 = small.tile([128, J], f32, tag="n2", name="n2")
            nc.vector.tensor_reduce(
                n2[:, :],
                ST[ch]["s2"][:, :].rearrange("p (j d) -> p j d", j=J), AxX,
                Alu.add)
            ST[ch]["n2"] = n2

        def st_nr(ch):
            nr = small.tile([128, J], f32, tag="nrm", name="nr")
            nc.scalar.activation(nr[:, :], ST[ch]["n2"][:, :], Act.Sqrt)
            ST[ch]["nr"] = nr

        def st_rf(ch):
            n2p1 = small.tile([128, J], f32, tag="n2p1", name="n2p1")
            nc.vector.tensor_scalar_add(n2p1[:, :], ST[ch]["n2"][:, :], 1.0)
            r = small.tile([128, J], f32, tag="rcp", name="r")
            nc.vector.reciprocal(r[:, :], n2p1[:, :])
            f = small.tile([128, J], f32, tag="fac", name="f")
            nc.vector.tensor_tensor(f[:, :], ST[ch]["nr"][:, :], r[:, :],
                                    Alu.mult)
            ST[ch]["f"] = f

        def st_pu(ch):
            # produ[p, (j, c, d)] = s_bcast * Wa  (squash factor applied
            # later, on the d-reduced output)
            pu = spool.tile([128, J * CA * D], bf16, tag="produ", name="pu")
            pu4 = pu[:, :].rearrange("p (j c d) -> p j c d", j=J, c=CA)
            wu4 = wu_t[:, :].rearrange("p (j c d) -> p j c d", j=J, c=CA)
            sbb = (ST[ch]["sb"][:, :].rearrange("p (j d) -> p j d", j=J)
                   .unsqueeze(2).broadcast_to([128, J, CA, D]))
            split_tt(pu4, sbb, wu4, ku[IT[0]])
            ST[ch]["pu"] = pu

        def st_umm(ch):
            # d-reduction (16 slices) on PE via identity-matmul PSUM
            # accumulation, straight off the product.
            puz = ST[ch]["pu"][:, :].rearrange("p (a d) -> p a d", d=D)
            psu = psum.tile([128, J * CA], f32, tag="psum_e", name="psu")
            for k in range(D):
                nc.tensor.matmul(psu[:, :], eye_t[:, :], puz[:, :, k],
                                 start=(k == 0), stop=(k == D - 1))
            ST[ch]["psu"] = psu

        def st_ucp(ch):
            ut = small.tile([128, J * CP], bf16, tag="ut", name="ut")
            ut3 = ut[:, :].rearrange("p (j c) -> p j c", j=J)[:, :, 0:CA]
            nc.scalar.copy(
                ut3, ST[ch]["psu"][:, :].rearrange("p (j c) -> p j c", j=J))
            ST[ch]["ut"] = ut

        def st_uscale(ch):
            # u = f * (s @ W): fold the squash factor into u.
            u = small.tile([128, J * CP], bf16, tag="u", name="u")
            u3 = u[:, :].rearrange("p (j c) -> p j c", j=J)[:, :, 0:CA]
            ut3 = (ST[ch]["ut"][:, :].rearrange("p (j c) -> p j c", j=J)
                   [:, :, 0:CA])
            fb = (ST[ch]["f"][:, :].unsqueeze(2)
                  .broadcast_to([128, J, CA]))
            nc.vector.tensor_tensor(u3, ut3, fb, Alu.mult)
            ST[ch]["u"] = u

        def st_pe(ch):
            # prodE[p, (j, s, c)] over all 9 channels (the ones channel
            # carries u[j,8], summed into db by the PE c-contraction).
            pe = spool.tile([128, J * S * CP], bf16, tag="bigP", name="pe")
            pe4 = (pe[:, :].rearrange("p (j s c) -> p j s c", j=J, s=S)
                   [:, :, :, 0:CA])
            ub = (ST[ch]["u"][:, :].rearrange("p (j c) -> p j c", j=J)
                  [:, :, 0:CA].unsqueeze(2)
                  .broadcast_to([128, J, S, CA]))
            xb = (Xsc[ch].rearrange("p (s c) -> p s c", s=S)
                  [:, :, 0:CA].unsqueeze(1)
                  .broadcast_to([128, J, S, CA]))
            split_tt(pe4, ub, xb, ke[IT[0]])
            ST[ch]["pe"] = pe

        def st_emm(ch):
            # db[p, (j, s)] = sum_c prodE: identity matmul with PSUM
            # accumulation over the 9 channel slices (partition-preserving).
            pez = ST[ch]["pe"][:, :].rearrange("p (a c) -> p a c", c=CP)
            pse = psum.tile([128, J * S], f32, tag="psum_e", name="pse")
            for c in range(CA):
                nc.tensor.matmul(pse[:, :], eye_t[:, :], pez[:, :, c],
                                 start=(c == 0), stop=(c == CA - 1))
            ST[ch]["pse"] = pse

        def st_lcp1(ch):
            # iteration 1: L = db1 (b was zero) — straight psum evacuation.
            nc.scalar.copy(L[ch][:, :], ST[ch]["pse"][:, :])

        def st_lcp2(ch):
            db = small.tile([128, J * S], bf16, tag="db", name="db")
            nc.scalar.copy(db[:, :], ST[ch]["pse"][:, :])
            ST[ch]["db"] = db

        def st_ladd(ch):
            nc.vector.tensor_tensor(L[ch][:, :], L[ch][:, :],
                                    ST[ch]["db"][:, :], Alu.add)

        def st_exp(ch):
            et = spool.tile([128, J * S], bf16, tag="expt", name="et")
            nc.scalar.activation(et[:, :], L[ch][:, :], Act.Exp)
            ST[ch]["et"] = et

        def st_zred(ch):
            z = small.tile([128, S], f32, tag="z", name="z")
            ejs = (ST[ch]["et"][:, :].rearrange("p (j s) -> p j s", j=J)
                   .transpose([0, 2, 1]))
            nc.vector.tensor_reduce(z[:, :], ejs, AxX, Alu.add)
            ST[ch]["z"] = z

        def st_zr(ch):
            zr = small.tile([128, S], bf16, tag="zr", name="zr")
            with nc.allow_low_precision("bf16 softmax normalizer"):
                nc.vector.reciprocal(zr[:, :], ST[ch]["z"][:, :])
            ST[ch]["zr"] = zr

        def st_ct(ch):
            ct = spool.tile([128, J * S], bf16, tag="ct", name="ct")
            zb = ST[ch]["zr"][:, :].unsqueeze(1).broadcast_to([128, J, S])
            nc.vector.tensor_tensor(
                ct[:, :].rearrange("p (j s) -> p j s", j=J),
                ST[ch]["et"][:, :].rearrange("p (j s) -> p j s", j=J), zb,
                Alu.mult)
            ST[ch]["ct"] = ct

        def st_pb(ch):
            pb = spool.tile([128, J * CA * S], bf16, tag="bigP", name="pb")
            pb4 = pb[:, :].rearrange("p (j c s) -> p j c s", j=J, c=CA)
            cb = (ST[ch]["ct"][:, :].rearrange("p (j s) -> p j s", j=J)
                  .unsqueeze(2).broadcast_to([128, J, CA, S]))
            xb = (Xcs[ch].rearrange("p (c s) -> p c s", c=CA)
                  .unsqueeze(1).broadcast_to([128, J, CA, S]))
            split_tt(pb4, cb, xb, kb[IT[0] - 1])
            ST[ch]["pb"] = pb

        def st_tmm(ch):
            # s-reduction (all 36 slices) on PE via identity-matmul PSUM
            # accumulation, straight off the product.
            pbz = ST[ch]["pb"][:, :].rearrange("p (a s) -> p a s", s=S)
            pst = psum.tile([128, J * CA], f32, tag="psum_e", name="pst")
            for k in range(S):
                nc.tensor.matmul(pst[:, :], eye_t[:, :], pbz[:, :, k],
                                 start=(k == 0), stop=(k == S - 1))
            ST[ch]["pst"] = pst

        def st_tcp(ch):
            t = small.tile([128, J * CP], bf16, tag="tt", name="t")
            t3 = t[:, :].rearrange("p (j c) -> p j c", j=J)[:, :, 0:CA]
            nc.scalar.copy(
                t3, ST[ch]["pst"][:, :].rearrange("p (j c) -> p j c", j=J))
            ST[ch]["t_b"] = (t[:, :].rearrange("p (j c) -> p j c", j=J)
                             [:, :, 0:CA].unsqueeze(2)
                             .broadcast_to([128, J, D, CA]))

        def st_vt(ch):
            # final v = s * f, fp32
            vt = vpool.tile([128, J * D], f32, tag="vtf", name="vt")
            fb = (ST[ch]["f"][:, :].unsqueeze(2)
                  .broadcast_to([128, J, D]))
            nc.vector.tensor_tensor(
                vt[:, :].rearrange("p (j d) -> p j d", j=J),
                ST[ch]["s_sb"][:, :].rearrange("p (j d) -> p j d", j=J), fb,
                Alu.mult)
            ST[ch]["vt"] = vt

        def st_out(ch):
            dmae.dma_start(v_d[ch * B4:(ch + 1) * B4, :],
                           ST[ch]["vt"][0:128:NS, :])

        # ---------------- emission: stage-major wavefront ----------------
        # Chunks are processed in groups of `bufs`: within a group, stages
        # are emitted stage-major (so every engine has independent work from
        # the other chunks of the group), and every tile's consumers are
        # emitted before the next group recycles its buffer.

        def emit(stages):
            for g0 in range(0, NCH, grp):
                for fn in stages:
                    for ch in range(g0, min(g0 + grp, NCH)):
                        fn(ch)

        c_sq_u = [st_pc, st_mm, st_scpy, st_n2, st_nr, st_rf, st_pu,
                  st_umm, st_ucp, st_uscale, st_pe, st_emm]
        softmax_b = [st_exp, st_zred, st_zr, st_ct, st_pb, st_tmm, st_tcp]

        IT[0] = 0
        emit([st_xsum, st_xs1] + c_sq_u)
        IT[0] = 1
        emit([st_lcp1] + softmax_b + c_sq_u)
        IT[0] = 2
        emit([st_lcp2, st_ladd] + softmax_b +
             [st_pc, st_mm, st_scpy3, st_n2, st_nr, st_rf, st_vt, st_out])

    if split_waits:
        _split_multi_waits(nc)
    return nc


def _split_multi_waits(nc):
    """Walrus's cayman codegen allows exactly ONE sync wait per TPB
    instruction (NEURON_ISA_TPB_EVENTS has a single wait slot). Tile's
    scheduler attaches several waits to dependency-merge instructions,
    which the native bass encoder handles but the neuronx-cc path rejects
    ("Too many sync wait commands"). Split the extras onto engine-local
    NoOp instructions inserted immediately before the owner so the wait
    semantics are unchanged.
    """
    from concourse import mybir

    for bbname, bbwrap in nc.bb_map.items():
        bb = bbwrap.bb
        insts = bb.instructions
        i = 0
        while i < len(insts):
            ins = insts[i]
            si = getattr(ins, "sync_info", None)
            if si is None or len(si.on_wait or []) <= 1:
                i += 1
                continue
            waits = list(si.on_wait)
            engine = ins.engine
            for w in waits[:-1]:
                nop = mybir.InstNoOp(
                    name=nc.get_next_instruction_name(),
                    engine=engine,
                    bass_nofuse=True,
                    sync_info=mybir.SyncInfo(on_wait=[w], on_update=[]),
                )
                insts.insert(i, nop)
                i += 1
            ins.sync_info = mybir.SyncInfo(on_wait=[waits[-1]],
                                           on_update=si.on_update)
            i += 1


def _get_program(split_waits=True, **kw):
    key = ("nc", split_waits, tuple(sorted(kw.items())))
    if key not in _CACHE:
        _CACHE[key] = _build_program(split_waits, **kw)
    return _CACHE[key]


def _host_prep(x, W, bias):
    """Build per-core input maps."""
    bf = np.float16
    x = np.ascontiguousarray(x, dtype=np.float32)
    W = np.ascontiguousarray(W, dtype=np.float32)
    bias = np.ascontiguousarray(bias, dtype=np.float32)
    bs = x.shape[0]

    xga = x.reshape(bs, NS, C_IN, S)
    xa = np.concatenate(
        [xga, np.ones((bs, NS, 1, S), dtype=np.float32)], axis=2)
    # [core, ch, b4, g, c, s] -> partition-major [core, b4, g, ch, c, s]
    x6 = xa.reshape(NCORES, NCH, B4, NS, CA, S)
    x6p = x6.transpose(0, 2, 3, 1, 4, 5)
    xcs = np.ascontiguousarray(x6p).reshape(
        NCORES, 128, NCH * CA * S).astype(bf)
    x6sc = x6p.transpose(0, 1, 2, 3, 5, 4)    # [.., ch, s, c]
    x6sp = np.concatenate(
        [x6sc, np.zeros(x6sc.shape[:-1] + (CP - CA,), np.float32)], axis=-1)
    xsc = np.ascontiguousarray(x6sp).reshape(
        NCORES, 128, NCH * S * CP).astype(bf)

    Wa = np.concatenate(
        [W.reshape(NS, J, D, C_IN),
         bias.reshape(NS, J, D, 1)], axis=3)            # [g, j, d, c]
    Wap = np.concatenate(
        [Wa, np.zeros(Wa.shape[:-1] + (CP - CA,), np.float32)], axis=-1)
    wc = np.tile(Wap.reshape(NS, J * D * CP), (B4, 1)).astype(bf)
    wu = np.tile(
        Wa.transpose(0, 1, 3, 2).reshape(NS, J * CA * D),
        (B4, 1)).astype(bf)                             # [128, (j,c,d)]
    onesb = np.kron(np.eye(B4, dtype=np.float32),
                    np.ones((NS, NS), dtype=np.float32)).astype(bf)
    eye = np.eye(128, dtype=np.float32).astype(bf)

    in_maps = [
        {"xcs": np.ascontiguousarray(xcs[k]),
         "xsc": np.ascontiguousarray(xsc[k]),
         "wc": wc, "wu": wu, "onesb": onesb, "eye": eye}
        for k in range(NCORES)
    ]
    return in_maps


def kernel(x, W, bias, b0):
    from concourse.bass_utils import run_bass_kernel_spmd

    nc = _get_program()
    in_maps = _host_prep(x, W, bias)
    res = run_bass_kernel_spmd(nc, in_maps, list(range(NCORES)))
    out = np.concatenate([res.results[k]["v"] for k in range(NCORES)],
                         axis=0)
    return np.ascontiguousarray(out.reshape(NCORES * BLOC, J, D))


# revision 26
# speedup vs baseline: 3.0461x; 1.0387x over previous
# Trainium2 Bass kernel for nn_CapLayer (CapsNet grouped 1x1 conv + dynamic routing).
#
# Key algebraic restructuring: the huge intermediate pred[b, i=(g,s), (j,d)]
# (188MB for the full batch) is NEVER materialized. Routing is computed in a
# factored form:
#   pred[b,(g,s),(j,d)] = sum_c Wa[g,j,d,c] * xga[b,g,c,s]     (c augmented with
#                                                               a ones channel to
#                                                               absorb the bias)
#   t[b,j,g,c]  = sum_s c[b,j,(g,s)] * xga[b,g,c,s]
#   s[b,j,d]    = sum_{g,c} t[b,j,g,c] * Wa[g,j,d,c]
#   u[b,j,g,c]  = sum_d v[b,j,d] * Wa[g,j,d,c]
#   db[b,j,g,s] = sum_c u[b,j,g,c] * xga[b,g,c,s]
# Iteration 1 collapses (softmax of zeros is uniform): t1 = xsum / J.
# The squash factor f = |s|/(1+|s|^2) is folded into u (u = f * (s @ W))
# instead of materializing v, which takes the scalar squash chain off the
# critical path between the PE matmul and the next product.
#
# Sharding: pure data parallel, 32 samples per core across 8 cores.
# On-chip layout: partition p = (b4, g) with 4 samples x 32 groups = 128
# partitions; 8 chunks cover the 32 local samples.
#
# Engine placement:
#  - The two W-contractions ride the TensorEngine: the g+c contraction for
#    s[b,(j,d)] uses a block-diagonal ones matrix (which also replicates s
#    across the g-partitions for free), and the c-contraction for db uses
#    an identity matrix with PSUM accumulation over the 9 channel slices —
#    this replaces a whole DVE add-tree and absorbs the bias channel.
#  - Every big broadcast-product is split along j between DVE (bf16 2x mode)
#    and GPSIMD/Pool so both engines chew each product concurrently.
#  - The Activation engine does psum evacuation (including writing db
#    directly into the logits), exp, square, sqrt.
#
# Schedule: stages are emitted in a wavefront across chunk groups of `bufs`
# (stage-major order) so each engine always has independent work from other
# chunks while one chunk traverses the cross-engine chain.

import sys

import numpy as np

# concourse (Bass/Tile) ships with the container; make sure it's importable
# when the grader runs kernel.py from a bare directory.
for _p in ("/opt/trn_rl_repo", "/root/.axon_site/_ro/trn_rl_repo"):
    if _p not in sys.path:
        sys.path.insert(0, _p)

NS, J, D, C_IN, H, WID, RN = 32, 10, 16, 8, 6, 6, 3
S = H * WID            # 36 spatial positions
CA = C_IN + 1          # 9 channels including the ones channel
CP = 10                # padded channel stride (4B alignment for bf16 rows)
NCORES = 8
BLOC = 32              # samples per core
B4 = 4                 # samples per chunk
NCH = BLOC // B4       # 8 chunks

_CACHE = {}


def _build_program(split_waits=True, kb=(2, 2), ke=(2, 2), kc=(3, 3, 3),
                   ku=(2, 2), bufs=6, grp=4, dma_eng="sync"):
    kb = (kb, kb) if isinstance(kb, int) else kb
    ke = (ke, ke) if isinstance(ke, int) else ke
    kc = (kc, kc, kc) if isinstance(kc, int) else kc
    ku = (ku, ku) if isinstance(ku, int) else ku
    from contextlib import ExitStack

    import concourse.bass as bass
    import concourse.tile as tile
    from concourse import mybir

    f32 = mybir.dt.float32
    bf16 = mybir.dt.float16
    Alu = mybir.AluOpType
    Act = mybir.ActivationFunctionType
    AxX = mybir.AxisListType.X

    nc = bass.Bass("TRN2", target_bir_lowering=True, debug=False,
                   num_devices=NCORES)

    xcs_d = nc.dram_tensor("xcs", [128, NCH * CA * S], bf16,
                           kind="ExternalInput").ap()      # free = (ch, c, s)
    xsc_d = nc.dram_tensor("xsc", [128, NCH * S * CP], bf16,
                           kind="ExternalInput").ap()      # free = (ch, s, c10)
    wc_d = nc.dram_tensor("wc", [128, J * D * CP], bf16,
                          kind="ExternalInput").ap()       # free = (j, d, c10)
    wu_d = nc.dram_tensor("wu", [128, J * CA * D], bf16,
                          kind="ExternalInput").ap()       # free = (j, c, d)
    ones_d = nc.dram_tensor("onesb", [128, 128], bf16,
                            kind="ExternalInput").ap()     # blockdiag over b4
    eye_d = nc.dram_tensor("eye", [128, 128], bf16,
                           kind="ExternalInput").ap()      # identity
    v_d = nc.dram_tensor("v", [BLOC, J * D], f32,
                         kind="ExternalOutput").ap()

    dmae = {"gpsimd": nc.gpsimd, "sync": nc.sync}[dma_eng]
    with tile.TileContext(nc) as tc, ExitStack() as ctx:
        consts = ctx.enter_context(tc.tile_pool(name="consts", bufs=1))
        xpool = ctx.enter_context(tc.tile_pool(name="xpool", bufs=1))
        lpool = ctx.enter_context(tc.tile_pool(name="lpool", bufs=1))
        spool = ctx.enter_context(tc.tile_pool(name="scratch", bufs=bufs))
        small = ctx.enter_context(tc.tile_pool(name="small", bufs=bufs))
        vpool = ctx.enter_context(tc.tile_pool(name="vpool", bufs=bufs))
        psum = ctx.enter_context(tc.tile_pool(name="psum", bufs=4,
                                              space="PSUM"))

        # x first (the first compute stage needs it), consts after.
        xall = xpool.tile([128, NCH * CA * S], bf16, tag="xall", name="xall")
        CSZ = CA * S
        dmae.dma_start(xall[:, 0:CSZ], xcs_d[:, 0:CSZ])
        HX = 4 * CSZ
        dmae.dma_start(xall[:, CSZ:HX], xcs_d[:, CSZ:HX])
        wc_t = consts.tile([128, J * D * CP], bf16, tag="wc")
        dmae.dma_start(wc_t[:, :], wc_d[:, :])
        ones_t = consts.tile([128, 128], bf16, tag="onesb")
        dmae.dma_start(ones_t[:, :], ones_d[:, :])
        dmae.dma_start(xall[:, HX:], xcs_d[:, HX:])
        xall2 = xpool.tile([128, NCH * S * CP], bf16, tag="xall2",
                           name="xall2")
        dmae.dma_start(xall2[:, :], xsc_d[:, :])
        wu_t = consts.tile([128, J * CA * D], bf16, tag="wu")
        dmae.dma_start(wu_t[:, :], wu_d[:, :])
        eye_t = consts.tile([128, 128], bf16, tag="eye")
        dmae.dma_start(eye_t[:, :], eye_d[:, :])

        Xcs = [xall[:, ch * CA * S:(ch + 1) * CA * S] for ch in range(NCH)]
        Xsc = [xall2[:, ch * S * CP:(ch + 1) * S * CP] for ch in range(NCH)]
        L = []     # routing logits b, layout [p, (j, s)] bf16
        for ch in range(NCH):
            L.append(lpool.tile([128, J * S], bf16, tag=f"L{ch}",
                                name=f"L{ch}"))

        # Per-chunk in-flight state handed between stages.
        ST = [dict() for _ in range(NCH)]
        IT = [0]   # current iteration index (0-based), set at emission

        def split_tt(out4, in0, in1, k):
            """Emit a broadcast-product split along the j (outermost free)
            dim: DVE takes j < J-k, Pool takes the last k rows. Both write
            disjoint j-ranges of the same tile."""
            if k < J:
                nc.vector.tensor_tensor(out4[:, 0:J - k], in0[:, 0:J - k],
                                        in1[:, 0:J - k], Alu.mult)
            if k > 0:
                nc.gpsimd.tensor_tensor(out4[:, J - k:J], in0[:, J - k:J],
                                        in1[:, J - k:J], Alu.mult)

        # ---------------- stage functions ----------------

        def st_xsum(ch):
            xsum = small.tile([128, CA], bf16, tag="xsum", name="xsum")
            with nc.allow_low_precision("bf16 routing intermediates"):
                nc.vector.tensor_reduce(
                    xsum[:, :],
                    Xcs[ch].rearrange("p (c s) -> p c s", c=CA), AxX,
                    Alu.add)
            ST[ch]["xsum"] = xsum

        def st_xs1(ch):
            xs1 = small.tile([128, CA], bf16, tag="xsum1", name="xs1")
            with nc.allow_low_precision("bf16 routing intermediates"):
                nc.vector.tensor_scalar_mul(xs1[:, :], ST[ch]["xsum"][:, :],
                                            1.0 / J)
            ST[ch]["t_b"] = (xs1[:, :].unsqueeze(1).unsqueeze(1)
                             .broadcast_to([128, J, D, CA]))

        def st_pc(ch):
            # prodC[p, (j, d, c)] = t_bcast * Wa
            pc = spool.tile([128, J * D * CP], bf16, tag="prodC", name="pc")
            pc4 = (pc[:, :].rearrange("p (j d c) -> p j d c", j=J, d=D)
                   [:, :, :, 0:CA])
            wc4 = (wc_t[:, :].rearrange("p (j d c) -> p j d c", j=J, d=D)
                   [:, :, :, 0:CA])
            split_tt(pc4, ST[ch]["t_b"], wc4, kc[IT[0]])
            ST[ch]["pc"] = pc

        def st_mm(ch):
            # PE contracts g (partitions, via blockdiag ones) AND c (PSUM
            # accumulation over the 9 channel slices).
            pcz = ST[ch]["pc"][:, :].rearrange("p (a c) -> p a c", c=CP)
            ps = psum.tile([128, J * D], f32, tag="psum_s", name="ps")
            for c in range(CA):
                nc.tensor.matmul(ps[:, :], ones_t[:, :], pcz[:, :, c],
                                 start=(c == 0), stop=(c == CA - 1))
            ST[ch]["ps"] = ps

        def st_scpy(ch):
            # s in bf16 for the u-product, plus s^2 for the squash norm.
            ps = ST[ch]["ps"]
            sb = small.tile([128, J * D], bf16, tag="s_b16", name="sb")
            nc.scalar.copy(sb[:, :], ps[:, :])
            s2 = small.tile([128, J * D], f32, tag="s2", name="s2")
            nc.scalar.activation(s2[:, :], ps[:, :], Act.Square)
            ST[ch]["sb"] = sb
            ST[ch]["s2"] = s2

        def st_scpy3(ch):
            ps = ST[ch]["ps"]
            s_sb = small.tile([128, J * D], f32, tag="s_sb", name="s_sb")
            nc.scalar.copy(s_sb[:, :], ps[:, :])
            s2 = small.tile([128, J * D], f32, tag="s2", name="s2")
            nc.scalar.activation(s2[:, :], ps[:, :], Act.Square)
            ST[ch]["s_sb"] = s_sb
            ST[ch]["s2"] = s2

        def st_n2(ch):
            n2 = small.tile([128, J], f32, tag="n2", name="n2")
            nc.vector.tensor_reduce(
                n2[:, :],
                ST[ch]["s2"][:, :].rearrange("p (j d) -> p j d", j=J), AxX,
                Alu.add)
            ST[ch]["n2"] = n2

        def st_nr(ch):
            nr = small.tile([128, J], f32, tag="nrm", name="nr")
            nc.scalar.activation(nr[:, :], ST[ch]["n2"][:, :], Act.Sqrt)
            ST[ch]["nr"] = nr

        def st_rf(ch):
            n2p1 = small.tile([128, J], f32, tag="n2p1", name="n2p1")
            nc.vector.tensor_scalar_add(n2p1[:, :], ST[ch]["n2"][:, :], 1.0)
            r = small.tile([128, J], f32, tag="rcp", name="r")
            nc.vector.reciprocal(r[:, :], n2p1[:, :])
            f = small.tile([128, J], f32, tag="fac", name="f")
            nc.vector.tensor_tensor(f[:, :], ST[ch]["nr"][:, :], r[:, :],
                                    Alu.mult)
            ST[ch]["f"] = f

        def st_pu(ch):
            # produ[p, (j, c, d)] = s_bcast * Wa  (squash factor applied
            # later, on the d-reduced output)
            pu = spool.tile([128, J * CA * D], bf16, tag="produ", name="pu")
            pu4 = pu[:, :].rearrange("p (j c d) -> p j c d", j=J, c=CA)
            wu4 = wu_t[:, :].rearrange("p (j c d) -> p j c d", j=J, c=CA)
            sbb = (ST[ch]["sb"][:, :].rearrange("p (j d) -> p j d", j=J)
                   .unsqueeze(2).broadcast_to([128, J, CA, D]))
            split_tt(pu4, sbb, wu4, ku[IT[0]])
            ST[ch]["pu"] = pu

        def st_umm(ch):
            # d-reduction (16 slices) on PE via identity-matmul PSUM
            # accumulation, straight off the product.
            puz = ST[ch]["pu"][:, :].rearrange("p (a d) -> p a d", d=D)
            psu = psum.tile([128, J * CA], f32, tag="psum_e", name="psu")
            for k in range(D):
                nc.tensor.matmul(psu[:, :], eye_t[:, :], puz[:, :, k],
                                 start=(k == 0), stop=(k == D - 1))
            ST[ch]["psu"] = psu

        def st_ucp(ch):
            ut = small.tile([128, J * CP], bf16, tag="ut", name="ut")
            ut3 = ut[:, :].rearrange("p (j c) -> p j c", j=J)[:, :, 0:CA]
            nc.scalar.copy(
                ut3, ST[ch]["psu"][:, :].rearrange("p (j c) -> p j c", j=J))
            ST[ch]["ut"] = ut

        def st_uscale(ch):
            # u = f * (s @ W): fold the squash factor into u.
            u = small.tile([128, J * CP], bf16, tag="u", name="u")
            u3 = u[:, :].rearrange("p (j c) -> p j c", j=J)[:, :, 0:CA]
            ut3 = (ST[ch]["ut"][:, :].rearrange("p (j c) -> p j c", j=J)
                   [:, :, 0:CA])
            fb = (ST[ch]["f"][:, :].unsqueeze(2)
                  .broadcast_to([128, J, CA]))
            nc.vector.tensor_tensor(u3, ut3, fb, Alu.mult)
            ST[ch]["u"] = u

        def st_pe(ch):
            # prodE[p, (j, s, c)] over all 9 channels (the ones channel
            # carries u[j,8], summed into db by the PE c-contraction).
            pe = spool.tile([128, J * S * CP], bf16, tag="bigP", name="pe")
            pe4 = (pe[:, :].rearrange("p (j s c) -> p j s c", j=J, s=S)
                   [:, :, :, 0:CA])
            ub = (ST[ch]["u"][:, :].rearrange("p (j c) -> p j c", j=J)
                  [:, :, 0:CA].unsqueeze(2)
                  .broadcast_to([128, J, S, CA]))
            xb = (Xsc[ch].rearrange("p (s c) -> p s c", s=S)
                  [:, :, 0:CA].unsqueeze(1)
                  .broadcast_to([128, J, S, CA]))
            split_tt(pe4, ub, xb, ke[IT[0]])
            ST[ch]["pe"] = pe

        def st_emm(ch):
            # db[p, (j, s)] = sum_c prodE: identity matmul with PSUM
            # accumulation over the 9 channel slices (partition-preserving).
            pez = ST[ch]["pe"][:, :].rearrange("p (a c) -> p a c", c=CP)
            pse = psum.tile([128, J * S], f32, tag="psum_e", name="pse")
            for c in range(CA):
                nc.tensor.matmul(pse[:, :], eye_t[:, :], pez[:, :, c],
                                 start=(c == 0), stop=(c == CA - 1))
            ST[ch]["pse"] = pse

        def st_lcp1(ch):
            # iteration 1: L = db1 (b was zero) — straight psum evacuation.
            nc.scalar.copy(L[ch][:, :], ST[ch]["pse"][:, :])

        def st_lcp2(ch):
            db = small.tile([128, J * S], bf16, tag="db", name="db")
            nc.scalar.copy(db[:, :], ST[ch]["pse"][:, :])
            ST[ch]["db"] = db

        def st_ladd(ch):
            nc.vector.tensor_tensor(L[ch][:, :], L[ch][:, :],
                                    ST[ch]["db"][:, :], Alu.add)

        def st_exp(ch):
            et = spool.tile([128, J * S], bf16, tag="expt", name="et")
            nc.scalar.activation(et[:, :], L[ch][:, :], Act.Exp)
            ST[ch]["et"] = et

        def st_zred(ch):
            # z[s] = sum_j exp(L): 10 j-slices accumulated on PE.
            et3 = ST[ch]["et"][:, :].rearrange("p (j s) -> p j s", j=J)
            psz = psum.tile([128, S], f32, tag="psum_s", name="psz")
            for k in range(J):
                nc.tensor.matmul(psz[:, :], eye_t[:, :], et3[:, k, :],
                                 start=(k == 0), stop=(k == J - 1))
            ST[ch]["z"] = psz

        def st_zr(ch):
            zr = small.tile([128, S], bf16, tag="zr", name="zr")
            with nc.allow_low_precision("bf16 softmax normalizer"):
                nc.vector.reciprocal(zr[:, :], ST[ch]["z"][:, :])
            ST[ch]["zr"] = zr

        def st_ct(ch):
            ct = spool.tile([128, J * S], bf16, tag="ct", name="ct")
            zb = ST[ch]["zr"][:, :].unsqueeze(1).broadcast_to([128, J, S])
            nc.vector.tensor_tensor(
                ct[:, :].rearrange("p (j s) -> p j s", j=J),
                ST[ch]["et"][:, :].rearrange("p (j s) -> p j s", j=J), zb,
                Alu.mult)
            ST[ch]["ct"] = ct

        def st_pb(ch):
            pb = spool.tile([128, J * CA * S], bf16, tag="bigP", name="pb")
            pb4 = pb[:, :].rearrange("p (j c s) -> p j c s", j=J, c=CA)
            cb = (ST[ch]["ct"][:, :].rearrange("p (j s) -> p j s", j=J)
                  .unsqueeze(2).broadcast_to([128, J, CA, S]))
            xb = (Xcs[ch].rearrange("p (c s) -> p c s", c=CA)
                  .unsqueeze(1).broadcast_to([128, J, CA, S]))
            split_tt(pb4, cb, xb, kb[IT[0] - 1])
            ST[ch]["pb"] = pb

        def st_tmm(ch):
            # s-reduction (all 36 slices) on PE via identity-matmul PSUM
            # accumulation, straight off the product.
            pbz = ST[ch]["pb"][:, :].rearrange("p (a s) -> p a s", s=S)
            pst = psum.tile([128, J * CA], f32, tag="psum_e", name="pst")
            for k in range(S):
                nc.tensor.matmul(pst[:, :], eye_t[:, :], pbz[:, :, k],
                                 start=(k == 0), stop=(k == S - 1))
            ST[ch]["pst"] = pst

        def st_tcp(ch):
            t = small.tile([128, J * CP], bf16, tag="tt", name="t")
            t3 = t[:, :].rearrange("p (j c) -> p j c", j=J)[:, :, 0:CA]
            nc.scalar.copy(
                t3, ST[ch]["pst"][:, :].rearrange("p (j c) -> p j c", j=J))
            ST[ch]["t_b"] = (t[:, :].rearrange("p (j c) -> p j c", j=J)
                             [:, :, 0:CA].unsqueeze(2)
                             .broadcast_to([128, J, D, CA]))

        def st_vt(ch):
            # final v = s * f, fp32
            vt = vpool.tile([128, J * D], f32, tag="vtf", name="vt")
            fb = (ST[ch]["f"][:, :].unsqueeze(2)
                  .broadcast_to([128, J, D]))
            nc.vector.tensor_tensor(
                vt[:, :].rearrange("p (j d) -> p j d", j=J),
                ST[ch]["s_sb"][:, :].rearrange("p (j d) -> p j d", j=J), fb,
                Alu.mult)
            ST[ch]["vt"] = vt

        def st_out(ch):
            dmae.dma_start(v_d[ch * B4:(ch + 1) * B4, :],
                           ST[ch]["vt"][0:128:NS, :])

        # ---------------- emission: stage-major wavefront ----------------
        # Chunks are processed in groups of `bufs`: within a group, stages
        # are emitted stage-major (so every engine has independent work from
        # the other chunks of the group), and every tile's consumers are
        # emitted before the next group recycles its buffer.

        def emit(stages):
            for g0 in range(0, NCH, grp):
                for fn in stages:
                    for ch in range(g0, min(g0 + grp, NCH)):
                        fn(ch)

        c_sq_u = [st_pc, st_mm, st_scpy, st_n2, st_nr, st_rf, st_pu,
                  st_umm, st_ucp, st_uscale, st_pe, st_emm]
        softmax_b = [st_exp, st_zred, st_zr, st_ct, st_pb, st_tmm, st_tcp]

        IT[0] = 0
        emit([st_xsum, st_xs1] + c_sq_u)
        IT[0] = 1
        emit([st_lcp1] + softmax_b + c_sq_u)
        IT[0] = 2
        emit([st_lcp2, st_ladd] + softmax_b +
             [st_pc, st_mm, st_scpy3, st_n2, st_nr, st_rf, st_vt, st_out])

    if split_waits:
        _split_multi_waits(nc)
    return nc


def _split_multi_waits(nc):
    """Walrus's cayman codegen allows exactly ONE sync wait per TPB
    instruction (NEURON_ISA_TPB_EVENTS has a single wait slot). Tile's
    scheduler attaches several waits to dependency-merge instructions,
    which the native bass encoder handles but the neuronx-cc path rejects
    ("Too many sync wait commands"). Split the extras onto engine-local
    NoOp instructions inserted immediately before the owner so the wait
    semantics are unchanged.
    """
    from concourse import mybir

    for bbname, bbwrap in nc.bb_map.items():
        bb = bbwrap.bb
        insts = bb.instructions
        i = 0
        while i < len(insts):
            ins = insts[i]
            si = getattr(ins, "sync_info", None)
            if si is None or len(si.on_wait or []) <= 1:
                i += 1
                continue
            waits = list(si.on_wait)
            engine = ins.engine
            for w in waits[:-1]:
                nop = mybir.InstNoOp(
                    name=nc.get_next_instruction_name(),
                    engine=engine,
                    bass_nofuse=True,
                    sync_info=mybir.SyncInfo(on_wait=[w], on_update=[]),
                )
                insts.insert(i, nop)
                i += 1
            ins.sync_info = mybir.SyncInfo(on_wait=[waits[-1]],
                                           on_update=si.on_update)
            i += 1


def _get_program(split_waits=True, **kw):
    key = ("nc", split_waits, tuple(sorted(kw.items())))
    if key not in _CACHE:
        _CACHE[key] = _build_program(split_waits, **kw)
    return _CACHE[key]


def _host_prep(x, W, bias):
    """Build per-core input maps."""
    bf = np.float16
    x = np.ascontiguousarray(x, dtype=np.float32)
    W = np.ascontiguousarray(W, dtype=np.float32)
    bias = np.ascontiguousarray(bias, dtype=np.float32)
    bs = x.shape[0]

    xga = x.reshape(bs, NS, C_IN, S)
    xa = np.concatenate(
        [xga, np.ones((bs, NS, 1, S), dtype=np.float32)], axis=2)
    # [core, ch, b4, g, c, s] -> partition-major [core, b4, g, ch, c, s]
    x6 = xa.reshape(NCORES, NCH, B4, NS, CA, S)
    x6p = x6.transpose(0, 2, 3, 1, 4, 5)
    xcs = np.ascontiguousarray(x6p).reshape(
        NCORES, 128, NCH * CA * S).astype(bf)
    x6sc = x6p.transpose(0, 1, 2, 3, 5, 4)    # [.., ch, s, c]
    x6sp = np.concatenate(
        [x6sc, np.zeros(x6sc.shape[:-1] + (CP - CA,), np.float32)], axis=-1)
    xsc = np.ascontiguousarray(x6sp).reshape(
        NCORES, 128, NCH * S * CP).astype(bf)

    Wa = np.concatenate(
        [W.reshape(NS, J, D, C_IN),
         bias.reshape(NS, J, D, 1)], axis=3)            # [g, j, d, c]
    Wap = np.concatenate(
        [Wa, np.zeros(Wa.shape[:-1] + (CP - CA,), np.float32)], axis=-1)
    wc = np.tile(Wap.reshape(NS, J * D * CP), (B4, 1)).astype(bf)
    wu = np.tile(
        Wa.transpose(0, 1, 3, 2).reshape(NS, J * CA * D),
        (B4, 1)).astype(bf)                             # [128, (j,c,d)]
    onesb = np.kron(np.eye(B4, dtype=np.float32),
                    np.ones((NS, NS), dtype=np.float32)).astype(bf)
    eye = np.eye(128, dtype=np.float32).astype(bf)

    in_maps = [
        {"xcs": np.ascontiguousarray(xcs[k]),
         "xsc": np.ascontiguousarray(xsc[k]),
         "wc": wc, "wu": wu, "onesb": onesb, "eye": eye}
        for k in range(NCORES)
    ]
    return in_maps


def kernel(x, W, bias, b0):
    from concourse.bass_utils import run_bass_kernel_spmd

    nc = _get_program()
    in_maps = _host_prep(x, W, bias)
    res = run_bass_kernel_spmd(nc, in_maps, list(range(NCORES)))
    out = np.concatenate([res.results[k]["v"] for k in range(NCORES)],
                         axis=0)
    return np.ascontiguousarray(out.reshape(NCORES * BLOC, J, D))


# revision 32
# speedup vs baseline: 3.0511x; 1.0017x over previous
# Trainium2 Bass kernel for nn_CapLayer (CapsNet grouped 1x1 conv + dynamic routing).
#
# Key algebraic restructuring: the huge intermediate pred[b, i=(g,s), (j,d)]
# (188MB for the full batch) is NEVER materialized. Routing is computed in a
# factored form:
#   pred[b,(g,s),(j,d)] = sum_c Wa[g,j,d,c] * xga[b,g,c,s]     (c augmented with
#                                                               a ones channel to
#                                                               absorb the bias)
#   t[b,j,g,c]  = sum_s c[b,j,(g,s)] * xga[b,g,c,s]
#   s[b,j,d]    = sum_{g,c} t[b,j,g,c] * Wa[g,j,d,c]
#   u[b,j,g,c]  = sum_d v[b,j,d] * Wa[g,j,d,c]
#   db[b,j,g,s] = sum_c u[b,j,g,c] * xga[b,g,c,s]
# Iteration 1 collapses (softmax of zeros is uniform): t1 = xsum / J.
# The squash factor f = |s|/(1+|s|^2) is folded into u (u = f * (s @ W))
# instead of materializing v, which takes the scalar squash chain off the
# critical path between the PE matmul and the next product.
#
# Sharding: pure data parallel, 32 samples per core across 8 cores.
# On-chip layout: partition p = (b4, g) with 4 samples x 32 groups = 128
# partitions; 8 chunks cover the 32 local samples.
#
# Engine placement:
#  - The two W-contractions ride the TensorEngine: the g+c contraction for
#    s[b,(j,d)] uses a block-diagonal ones matrix (which also replicates s
#    across the g-partitions for free), and the c-contraction for db uses
#    an identity matrix with PSUM accumulation over the 9 channel slices —
#    this replaces a whole DVE add-tree and absorbs the bias channel.
#  - Every big broadcast-product is split along j between DVE (bf16 2x mode)
#    and GPSIMD/Pool so both engines chew each product concurrently.
#  - The Activation engine does psum evacuation (including writing db
#    directly into the logits), exp, square, sqrt.
#
# Schedule: stages are emitted in a wavefront across chunk groups of `bufs`
# (stage-major order) so each engine always has independent work from other
# chunks while one chunk traverses the cross-engine chain.

import sys

import numpy as np

# concourse (Bass/Tile) ships with the container; make sure it's importable
# when the grader runs kernel.py from a bare directory.
for _p in ("/opt/trn_rl_repo", "/root/.axon_site/_ro/trn_rl_repo"):
    if _p not in sys.path:
        sys.path.insert(0, _p)

NS, J, D, C_IN, H, WID, RN = 32, 10, 16, 8, 6, 6, 3
S = H * WID            # 36 spatial positions
CA = C_IN + 1          # 9 channels including the ones channel
CP = 10                # padded channel stride (4B alignment for bf16 rows)
NCORES = 8
BLOC = 32              # samples per core
B4 = 4                 # samples per chunk
NCH = BLOC // B4       # 8 chunks

_CACHE = {}


def _build_program(split_waits=True, kb=(2, 2), ke=(2, 2), kc=(3, 4, 3),
                   ku=(2, 2), kct=(0, 1), n2_pe=False, bufs=6, grp=4,
                   dma_eng="sync"):
    kb = (kb, kb) if isinstance(kb, int) else kb
    ke = (ke, ke) if isinstance(ke, int) else ke
    kc = (kc, kc, kc) if isinstance(kc, int) else kc
    ku = (ku, ku) if isinstance(ku, int) else ku
    kct = (kct, kct) if isinstance(kct, int) else kct
    from contextlib import ExitStack

    import concourse.bass as bass
    import concourse.tile as tile
    from concourse import mybir

    f32 = mybir.dt.float32
    bf16 = mybir.dt.float16
    Alu = mybir.AluOpType
    Act = mybir.ActivationFunctionType
    AxX = mybir.AxisListType.X

    nc = bass.Bass("TRN2", target_bir_lowering=True, debug=False,
                   num_devices=NCORES)

    xcs_d = nc.dram_tensor("xcs", [128, NCH * CA * S], bf16,
                           kind="ExternalInput").ap()      # free = (ch, c, s)
    xsc_d = nc.dram_tensor("xsc", [128, NCH * S * CP], bf16,
                           kind="ExternalInput").ap()      # free = (ch, s, c10)
    wc_d = nc.dram_tensor("wc", [128, J * D * CP], bf16,
                          kind="ExternalInput").ap()       # free = (j, d, c10)
    wu_d = nc.dram_tensor("wu", [128, J * CA * D], bf16,
                          kind="ExternalInput").ap()       # free = (j, c, d)
    ones_d = nc.dram_tensor("onesb", [128, 128], bf16,
                            kind="ExternalInput").ap()     # blockdiag over b4
    eye_d = nc.dram_tensor("eye", [128, 128], bf16,
                           kind="ExternalInput").ap()      # identity
    v_d = nc.dram_tensor("v", [BLOC, J * D], f32,
                         kind="ExternalOutput").ap()

    dmae = {"gpsimd": nc.gpsimd, "sync": nc.sync}[dma_eng]
    with tile.TileContext(nc) as tc, ExitStack() as ctx:
        consts = ctx.enter_context(tc.tile_pool(name="consts", bufs=1))
        xpool = ctx.enter_context(tc.tile_pool(name="xpool", bufs=1))
        lpool = ctx.enter_context(tc.tile_pool(name="lpool", bufs=1))
        spool = ctx.enter_context(tc.tile_pool(name="scratch", bufs=bufs))
        small = ctx.enter_context(tc.tile_pool(name="small", bufs=bufs))
        vpool = ctx.enter_context(tc.tile_pool(name="vpool", bufs=bufs))
        psum = ctx.enter_context(tc.tile_pool(name="psum", bufs=4,
                                              space="PSUM"))

        # x first (the first compute stage needs it), consts after.
        xall = xpool.tile([128, NCH * CA * S], bf16, tag="xall", name="xall")
        CSZ = CA * S
        dmae.dma_start(xall[:, 0:CSZ], xcs_d[:, 0:CSZ])
        HX = 4 * CSZ
        dmae.dma_start(xall[:, CSZ:HX], xcs_d[:, CSZ:HX])
        wc_t = consts.tile([128, J * D * CP], bf16, tag="wc")
        dmae.dma_start(wc_t[:, :], wc_d[:, :])
        ones_t = consts.tile([128, 128], bf16, tag="onesb")
        dmae.dma_start(ones_t[:, :], ones_d[:, :])
        dmae.dma_start(xall[:, HX:], xcs_d[:, HX:])
        xall2 = xpool.tile([128, NCH * S * CP], bf16, tag="xall2",
                           name="xall2")
        dmae.dma_start(xall2[:, :], xsc_d[:, :])
        wu_t = consts.tile([128, J * CA * D], bf16, tag="wu")
        dmae.dma_start(wu_t[:, :], wu_d[:, :])
        eye_t = consts.tile([128, 128], bf16, tag="eye")
        dmae.dma_start(eye_t[:, :], eye_d[:, :])

        Xcs = [xall[:, ch * CA * S:(ch + 1) * CA * S] for ch in range(NCH)]
        Xsc = [xall2[:, ch * S * CP:(ch + 1) * S * CP] for ch in range(NCH)]
        L = []     # routing logits b, layout [p, (j, s)] bf16
        for ch in range(NCH):
            L.append(lpool.tile([128, J * S], bf16, tag=f"L{ch}",
                                name=f"L{ch}"))

        # Per-chunk in-flight state handed between stages.
        ST = [dict() for _ in range(NCH)]
        IT = [0]   # current iteration index (0-based), set at emission

        def split_tt(out4, in0, in1, k):
            """Emit a broadcast-product split along the j (outermost free)
            dim: DVE takes j < J-k, Pool takes the last k rows. Both write
            disjoint j-ranges of the same tile."""
            if k < J:
                nc.vector.tensor_tensor(out4[:, 0:J - k], in0[:, 0:J - k],
                                        in1[:, 0:J - k], Alu.mult)
            if k > 0:
                nc.gpsimd.tensor_tensor(out4[:, J - k:J], in0[:, J - k:J],
                                        in1[:, J - k:J], Alu.mult)

        # ---------------- stage functions ----------------

        def st_xsum(ch):
            xsum = small.tile([128, CA], bf16, tag="xsum", name="xsum")
            with nc.allow_low_precision("bf16 routing intermediates"):
                nc.vector.tensor_reduce(
                    xsum[:, :],
                    Xcs[ch].rearrange("p (c s) -> p c s", c=CA), AxX,
                    Alu.add)
            ST[ch]["xsum"] = xsum

        def st_xs1(ch):
            xs1 = small.tile([128, CA], bf16, tag="xsum1", name="xs1")
            with nc.allow_low_precision("bf16 routing intermediates"):
                nc.vector.tensor_scalar_mul(xs1[:, :], ST[ch]["xsum"][:, :],
                                            1.0 / J)
            ST[ch]["t_b"] = (xs1[:, :].unsqueeze(1).unsqueeze(1)
                             .broadcast_to([128, J, D, CA]))

        def st_pc(ch):
            # prodC[p, (j, d, c)] = t_bcast * Wa
            pc = spool.tile([128, J * D * CP], bf16, tag="prodC", name="pc")
            pc4 = (pc[:, :].rearrange("p (j d c) -> p j d c", j=J, d=D)
                   [:, :, :, 0:CA])
            wc4 = (wc_t[:, :].rearrange("p (j d c) -> p j d c", j=J, d=D)
                   [:, :, :, 0:CA])
            split_tt(pc4, ST[ch]["t_b"], wc4, kc[IT[0]])
            ST[ch]["pc"] = pc

        def st_mm(ch):
            # PE contracts g (partitions, via blockdiag ones) AND c (PSUM
            # accumulation over the 9 channel slices).
            pcz = ST[ch]["pc"][:, :].rearrange("p (a c) -> p a c", c=CP)
            ps = psum.tile([128, J * D], f32, tag="psum_s", name="ps")
            for c in range(CA):
                nc.tensor.matmul(ps[:, :], ones_t[:, :], pcz[:, :, c],
                                 start=(c == 0), stop=(c == CA - 1))
            ST[ch]["ps"] = ps

        def st_scpy(ch):
            # s in bf16 for the u-product, plus s^2 for the squash norm.
            ps = ST[ch]["ps"]
            sb = small.tile([128, J * D], bf16, tag="s_b16", name="sb")
            nc.scalar.copy(sb[:, :], ps[:, :])
            s2 = small.tile([128, J * D], bf16, tag="s2", name="s2")
            nc.scalar.activation(s2[:, :], ps[:, :], Act.Square)
            ST[ch]["sb"] = sb
            ST[ch]["s2"] = s2

        def st_scpy3(ch):
            ps = ST[ch]["ps"]
            s_sb = small.tile([128, J * D], f32, tag="s_sb", name="s_sb")
            nc.scalar.copy(s_sb[:, :], ps[:, :])
            s2 = small.tile([128, J * D], bf16, tag="s2", name="s2")
            nc.scalar.activation(s2[:, :], ps[:, :], Act.Square)
            ST[ch]["s_sb"] = s_sb
            ST[ch]["s2"] = s2

        def st_n2(ch):
            if n2_pe:
                s23 = ST[ch]["s2"][:, :].rearrange("p (j d) -> p j d", j=J)
                psn = psum.tile([128, J], f32, tag="psum_s", name="psn")
                for k in range(D):
                    nc.tensor.matmul(psn[:, :], eye_t[:, :], s23[:, :, k],
                                     start=(k == 0), stop=(k == D - 1))
                n2 = small.tile([128, J], f32, tag="n2", name="n2")
                nc.scalar.copy(n2[:, :], psn[:, :])
            else:
                n2 = small.tile([128, J], f32, tag="n2", name="n2")
                nc.vector.tensor_reduce(
                    n2[:, :],
                    ST[ch]["s2"][:, :].rearrange("p (j d) -> p j d", j=J),
                    AxX, Alu.add)
            ST[ch]["n2"] = n2

        def st_nr(ch):
            nr = small.tile([128, J], f32, tag="nrm", name="nr")
            nc.scalar.activation(nr[:, :], ST[ch]["n2"][:, :], Act.Sqrt)
            ST[ch]["nr"] = nr

        def st_rf(ch):
            n2p1 = small.tile([128, J], f32, tag="n2p1", name="n2p1")
            nc.vector.tensor_scalar_add(n2p1[:, :], ST[ch]["n2"][:, :], 1.0)
            r = small.tile([128, J], f32, tag="rcp", name="r")
            nc.vector.reciprocal(r[:, :], n2p1[:, :])
            f = small.tile([128, J], f32, tag="fac", name="f")
            nc.vector.tensor_tensor(f[:, :], ST[ch]["nr"][:, :], r[:, :],
                                    Alu.mult)
            ST[ch]["f"] = f

        def st_pu(ch):
            # produ[p, (j, c, d)] = s_bcast * Wa  (squash factor applied
            # later, on the d-reduced output)
            pu = spool.tile([128, J * CA * D], bf16, tag="produ", name="pu")
            pu4 = pu[:, :].rearrange("p (j c d) -> p j c d", j=J, c=CA)
            wu4 = wu_t[:, :].rearrange("p (j c d) -> p j c d", j=J, c=CA)
            sbb = (ST[ch]["sb"][:, :].rearrange("p (j d) -> p j d", j=J)
                   .unsqueeze(2).broadcast_to([128, J, CA, D]))
            split_tt(pu4, sbb, wu4, ku[IT[0]])
            ST[ch]["pu"] = pu

        def st_umm(ch):
            # d-reduction (16 slices) on PE via identity-matmul PSUM
            # accumulation, straight off the product.
            puz = ST[ch]["pu"][:, :].rearrange("p (a d) -> p a d", d=D)
            psu = psum.tile([128, J * CA], f32, tag="psum_e", name="psu")
            for k in range(D):
                nc.tensor.matmul(psu[:, :], eye_t[:, :], puz[:, :, k],
                                 start=(k == 0), stop=(k == D - 1))
            ST[ch]["psu"] = psu

        def st_ucp(ch):
            ut = small.tile([128, J * CP], bf16, tag="ut", name="ut")
            ut3 = ut[:, :].rearrange("p (j c) -> p j c", j=J)[:, :, 0:CA]
            nc.scalar.copy(
                ut3, ST[ch]["psu"][:, :].rearrange("p (j c) -> p j c", j=J))
            ST[ch]["ut"] = ut

        def st_uscale(ch):
            # u = f * (s @ W): fold the squash factor into u.
            u = small.tile([128, J * CP], bf16, tag="u", name="u")
            u3 = u[:, :].rearrange("p (j c) -> p j c", j=J)[:, :, 0:CA]
            ut3 = (ST[ch]["ut"][:, :].rearrange("p (j c) -> p j c", j=J)
                   [:, :, 0:CA])
            fb = (ST[ch]["f"][:, :].unsqueeze(2)
                  .broadcast_to([128, J, CA]))
            nc.vector.tensor_tensor(u3, ut3, fb, Alu.mult)
            ST[ch]["u"] = u

        def st_pe(ch):
            # prodE[p, (j, s, c)] over all 9 channels (the ones channel
            # carries u[j,8], summed into db by the PE c-contraction).
            pe = spool.tile([128, J * S * CP], bf16, tag="bigP", name="pe")
            pe4 = (pe[:, :].rearrange("p (j s c) -> p j s c", j=J, s=S)
                   [:, :, :, 0:CA])
            ub = (ST[ch]["u"][:, :].rearrange("p (j c) -> p j c", j=J)
                  [:, :, 0:CA].unsqueeze(2)
                  .broadcast_to([128, J, S, CA]))
            xb = (Xsc[ch].rearrange("p (s c) -> p s c", s=S)
                  [:, :, 0:CA].unsqueeze(1)
                  .broadcast_to([128, J, S, CA]))
            split_tt(pe4, ub, xb, ke[IT[0]])
            ST[ch]["pe"] = pe

        def st_emm(ch):
            # db[p, (j, s)] = sum_c prodE: identity matmul with PSUM
            # accumulation over the 9 channel slices (partition-preserving).
            pez = ST[ch]["pe"][:, :].rearrange("p (a c) -> p a c", c=CP)
            pse = psum.tile([128, J * S], f32, tag="psum_e", name="pse")
            for c in range(CA):
                nc.tensor.matmul(pse[:, :], eye_t[:, :], pez[:, :, c],
                                 start=(c == 0), stop=(c == CA - 1))
            ST[ch]["pse"] = pse

        def st_lcp1(ch):
            # iteration 1: L = db1 (b was zero) — straight psum evacuation.
            nc.scalar.copy(L[ch][:, :], ST[ch]["pse"][:, :])

        def st_lcp2(ch):
            db = small.tile([128, J * S], bf16, tag="db", name="db")
            nc.scalar.copy(db[:, :], ST[ch]["pse"][:, :])
            ST[ch]["db"] = db

        def st_ladd(ch):
            nc.vector.tensor_tensor(L[ch][:, :], L[ch][:, :],
                                    ST[ch]["db"][:, :], Alu.add)

        def st_exp(ch):
            et = spool.tile([128, J * S], bf16, tag="expt", name="et")
            nc.scalar.activation(et[:, :], L[ch][:, :], Act.Exp)
            ST[ch]["et"] = et

        def st_zred(ch):
            # z[s] = sum_j exp(L): 10 j-slices accumulated on PE.
            et3 = ST[ch]["et"][:, :].rearrange("p (j s) -> p j s", j=J)
            psz = psum.tile([128, S], f32, tag="psum_s", name="psz")
            for k in range(J):
                nc.tensor.matmul(psz[:, :], eye_t[:, :], et3[:, k, :],
                                 start=(k == 0), stop=(k == J - 1))
            ST[ch]["z"] = psz

        def st_zcp(ch):
            zs = small.tile([128, S], f32, tag="zs", name="zs")
            nc.scalar.copy(zs[:, :], ST[ch]["z"][:, :])
            ST[ch]["zs"] = zs

        def st_zr(ch):
            zr = small.tile([128, S], bf16, tag="zr", name="zr")
            with nc.allow_low_precision("bf16 softmax normalizer"):
                nc.vector.reciprocal(zr[:, :], ST[ch]["zs"][:, :])
            ST[ch]["zr"] = zr

        def st_ct(ch):
            ct = spool.tile([128, J * S], bf16, tag="ct", name="ct")
            zb = ST[ch]["zr"][:, :].unsqueeze(1).broadcast_to([128, J, S])
            split_tt(ct[:, :].rearrange("p (j s) -> p j s", j=J),
                     ST[ch]["et"][:, :].rearrange("p (j s) -> p j s", j=J),
                     zb, kct[IT[0] - 1])
            ST[ch]["ct"] = ct

        def st_pb(ch):
            pb = spool.tile([128, J * CA * S], bf16, tag="bigP", name="pb")
            pb4 = pb[:, :].rearrange("p (j c s) -> p j c s", j=J, c=CA)
            cb = (ST[ch]["ct"][:, :].rearrange("p (j s) -> p j s", j=J)
                  .unsqueeze(2).broadcast_to([128, J, CA, S]))
            xb = (Xcs[ch].rearrange("p (c s) -> p c s", c=CA)
                  .unsqueeze(1).broadcast_to([128, J, CA, S]))
            split_tt(pb4, cb, xb, kb[IT[0] - 1])
            ST[ch]["pb"] = pb

        def st_tmm(ch):
            # s-reduction (all 36 slices) on PE via identity-matmul PSUM
            # accumulation, straight off the product.
            pbz = ST[ch]["pb"][:, :].rearrange("p (a s) -> p a s", s=S)
            pst = psum.tile([128, J * CA], f32, tag="psum_e", name="pst")
            for k in range(S):
                nc.tensor.matmul(pst[:, :], eye_t[:, :], pbz[:, :, k],
                                 start=(k == 0), stop=(k == S - 1))
            ST[ch]["pst"] = pst

        def st_tcp(ch):
            t = small.tile([128, J * CP], bf16, tag="tt", name="t")
            t3 = t[:, :].rearrange("p (j c) -> p j c", j=J)[:, :, 0:CA]
            nc.scalar.copy(
                t3, ST[ch]["pst"][:, :].rearrange("p (j c) -> p j c", j=J))
            ST[ch]["t_b"] = (t[:, :].rearrange("p (j c) -> p j c", j=J)
                             [:, :, 0:CA].unsqueeze(2)
                             .broadcast_to([128, J, D, CA]))

        def st_vt(ch):
            # final v = s * f, fp32
            vt = vpool.tile([128, J * D], f32, tag="vtf", name="vt")
            fb = (ST[ch]["f"][:, :].unsqueeze(2)
                  .broadcast_to([128, J, D]))
            nc.vector.tensor_tensor(
                vt[:, :].rearrange("p (j d) -> p j d", j=J),
                ST[ch]["s_sb"][:, :].rearrange("p (j d) -> p j d", j=J), fb,
                Alu.mult)
            ST[ch]["vt"] = vt

        def st_out(ch):
            dmae.dma_start(v_d[ch * B4:(ch + 1) * B4, :],
                           ST[ch]["vt"][0:128:NS, :])

        # ---------------- emission: stage-major wavefront ----------------
        # Chunks are processed in groups of `bufs`: within a group, stages
        # are emitted stage-major (so every engine has independent work from
        # the other chunks of the group), and every tile's consumers are
        # emitted before the next group recycles its buffer.

        def emit(stages):
            for g0 in range(0, NCH, grp):
                for fn in stages:
                    for ch in range(g0, min(g0 + grp, NCH)):
                        fn(ch)

        c_sq_u = [st_pc, st_mm, st_scpy, st_n2, st_nr, st_rf, st_pu,
                  st_umm, st_ucp, st_uscale, st_pe, st_emm]
        softmax_b = [st_exp, st_zred, st_zcp, st_zr, st_ct, st_pb, st_tmm,
                     st_tcp]

        IT[0] = 0
        emit([st_xsum, st_xs1] + c_sq_u)
        IT[0] = 1
        emit([st_lcp1] + softmax_b + c_sq_u)
        IT[0] = 2
        emit([st_lcp2, st_ladd] + softmax_b +
             [st_pc, st_mm, st_scpy3, st_n2, st_nr, st_rf, st_vt, st_out])

    if split_waits:
        _split_multi_waits(nc)
    return nc


def _split_multi_waits(nc):
    """Walrus's cayman codegen allows exactly ONE sync wait per TPB
    instruction (NEURON_ISA_TPB_EVENTS has a single wait slot). Tile's
    scheduler attaches several waits to dependency-merge instructions,
    which the native bass encoder handles but the neuronx-cc path rejects
    ("Too many sync wait commands"). Split the extras onto engine-local
    NoOp instructions inserted immediately before the owner so the wait
    semantics are unchanged.
    """
    from concourse import mybir

    for bbname, bbwrap in nc.bb_map.items():
        bb = bbwrap.bb
        insts = bb.instructions
        i = 0
        while i < len(insts):
            ins = insts[i]
            si = getattr(ins, "sync_info", None)
            if si is None or len(si.on_wait or []) <= 1:
                i += 1
                continue
            waits = list(si.on_wait)
            engine = ins.engine
            for w in waits[:-1]:
                nop = mybir.InstNoOp(
                    name=nc.get_next_instruction_name(),
                    engine=engine,
                    bass_nofuse=True,
                    sync_info=mybir.SyncInfo(on_wait=[w], on_update=[]),
                )
                insts.insert(i, nop)
                i += 1
            ins.sync_info = mybir.SyncInfo(on_wait=[waits[-1]],
                                           on_update=si.on_update)
            i += 1


def _get_program(split_waits=True, **kw):
    key = ("nc", split_waits, tuple(sorted(kw.items())))
    if key not in _CACHE:
        _CACHE[key] = _build_program(split_waits, **kw)
    return _CACHE[key]


def _host_prep(x, W, bias):
    """Build per-core input maps."""
    bf = np.float16
    x = np.ascontiguousarray(x, dtype=np.float32)
    W = np.ascontiguousarray(W, dtype=np.float32)
    bias = np.ascontiguousarray(bias, dtype=np.float32)
    bs = x.shape[0]

    xga = x.reshape(bs, NS, C_IN, S)
    xa = np.concatenate(
        [xga, np.ones((bs, NS, 1, S), dtype=np.float32)], axis=2)
    # [core, ch, b4, g, c, s] -> partition-major [core, b4, g, ch, c, s]
    x6 = xa.reshape(NCORES, NCH, B4, NS, CA, S)
    x6p = x6.transpose(0, 2, 3, 1, 4, 5)
    xcs = np.ascontiguousarray(x6p).reshape(
        NCORES, 128, NCH * CA * S).astype(bf)
    x6sc = x6p.transpose(0, 1, 2, 3, 5, 4)    # [.., ch, s, c]
    x6sp = np.concatenate(
        [x6sc, np.zeros(x6sc.shape[:-1] + (CP - CA,), np.float32)], axis=-1)
    xsc = np.ascontiguousarray(x6sp).reshape(
        NCORES, 128, NCH * S * CP).astype(bf)

    Wa = np.concatenate(
        [W.reshape(NS, J, D, C_IN),
         bias.reshape(NS, J, D, 1)], axis=3)            # [g, j, d, c]
    Wap = np.concatenate(
        [Wa, np.zeros(Wa.shape[:-1] + (CP - CA,), np.float32)], axis=-1)
    wc = np.tile(Wap.reshape(NS, J * D * CP), (B4, 1)).astype(bf)
    wu = np.tile(
        Wa.transpose(0, 1, 3, 2).reshape(NS, J * CA * D),
        (B4, 1)).astype(bf)                             # [128, (j,c,d)]
    onesb = np.kron(np.eye(B4, dtype=np.float32),
                    np.ones((NS, NS), dtype=np.float32)).astype(bf)
    eye = np.eye(128, dtype=np.float32).astype(bf)

    in_maps = [
        {"xcs": np.ascontiguousarray(xcs[k]),
         "xsc": np.ascontiguousarray(xsc[k]),
         "wc": wc, "wu": wu, "onesb": onesb, "eye": eye}
        for k in range(NCORES)
    ]
    return in_maps


def kernel(x, W, bias, b0):
    from concourse.bass_utils import run_bass_kernel_spmd

    nc = _get_program()
    in_maps = _host_prep(x, W, bias)
    res = run_bass_kernel_spmd(nc, in_maps, list(range(NCORES)))
    out = np.concatenate([res.results[k]["v"] for k in range(NCORES)],
                         axis=0)
    return np.ascontiguousarray(out.reshape(NCORES * BLOC, J, D))
